# revision 1
# baseline (speedup 1.0000x reference)
"""MoE (8 routed experts, top-2, + shared expert) on 8 NeuronCores.

Strategy: data-parallel over tokens (1024 tokens/core), gate + all expert
weights replicated. The shared expert (hidden 4096) is split into two
H=2048 halves so the kernel is a uniform loop over 10 "virtual experts".
Dense formulation: every expert processes every token, scaled by the
(zero for unrouted) renormalized top-2 combine weight, fused into the
PSUM eviction. Gate runs in fp32 so routing decisions match the
reference; expert matmuls run in bf16 with fp32 accumulation.
"""

import numpy as np
import ml_dtypes

import concourse.bacc as bacc
import concourse.bass as bass
import concourse.tile as tile
import concourse.mybir as mybir
from concourse.bass_utils import run_bass_kernel_spmd

BF16 = ml_dtypes.bfloat16
F32 = mybir.dt.float32
BF = mybir.dt.bfloat16
AF = mybir.ActivationFunctionType
OP = mybir.AluOpType

P = 128


class Cfg:
    def __init__(self, D=1024, H=2048, E=8, n_sh=2, T=1024, n_cores=8, capm=96):
        self.D, self.H, self.E, self.n_sh, self.T = D, H, E, n_sh, T
        self.NV = E + n_sh          # virtual experts
        self.HS = n_sh * H          # shared hidden
        self.KD = D // P            # K chunks over D
        self.HCN = H // P           # h chunks over H
        self.TT = T // P            # token 128-tiles
        self.DT = (D + 511) // 512  # output d 512-tiles
        self.FT = (T + 511) // 512  # layer-1 free 512-tiles
        self.n_cores = n_cores
        self.capm = capm            # per-(expert, tile-pair) dispatch capacity
        self.NP = self.TT // 2      # token-tile pairs
        self.CAPE = self.NP * capm  # slots per expert
        self.ST = (self.CAPE + P - 1) // P  # slot 128-tiles per expert


def build_nc(cfg: Cfg):
    D, H, E, NV, T = cfg.D, cfg.H, cfg.E, cfg.NV, cfg.T
    KD, HCN, TT, DT, FT = cfg.KD, cfg.HCN, cfg.TT, cfg.DT, cfg.FT

    nc = bacc.Bacc("TRN2", target_bir_lowering=False)

    xT = nc.dram_tensor("xT", [P, KD, T], F32, kind="ExternalInput")
    w1t = nc.dram_tensor("w1t", [NV, HCN, P, KD, P], BF, kind="ExternalInput")
    w3t = nc.dram_tensor("w3t", [NV, HCN, P, KD, P], BF, kind="ExternalInput")
    w2t = nc.dram_tensor("w2t", [NV, P, HCN, D], BF, kind="ExternalInput")
    b1a = nc.dram_tensor("b1a", [NV, P, HCN], F32, kind="ExternalInput")
    b3a = nc.dram_tensor("b3a", [NV, P, HCN], F32, kind="ExternalInput")
    b2r = nc.dram_tensor("b2r", [1, NV, D], BF, kind="ExternalInput")
    gwt = nc.dram_tensor("gwt", [P, KD, E], F32, kind="ExternalInput")
    gb = nc.dram_tensor("gb", [1, E], F32, kind="ExternalInput")
    ones1 = nc.dram_tensor("ones1", [1, P], BF, kind="ExternalInput")
    y = nc.dram_tensor("y", [P, TT, D], F32, kind="ExternalOutput")

    with tile.TileContext(nc) as tc:
        with (
            tc.tile_pool(name="const1", bufs=1) as const1,
            tc.tile_pool(name="gchunk", bufs=2) as gchunk,
            tc.tile_pool(name="gtmp", bufs=4) as gtmp,
            tc.tile_pool(name="w1s", bufs=3) as w1s,
            tc.tile_pool(name="b13", bufs=2) as b13,
            tc.tile_pool(name="w2s", bufs=2) as w2s,
            tc.tile_pool(name="hpool", bufs=1) as hpool,
            tc.tile_pool(name="s1p", bufs=3) as s1p,
            tc.tile_pool(name="ps_l1", bufs=2, space="PSUM") as ps_l1,
            tc.tile_pool(name="ps_y", bufs=2, space="PSUM") as ps_y,
            tc.tile_pool(name="ps_g", bufs=2, space="PSUM") as ps_g,
        ):
            # ---- resident constants ----
            xTb = const1.tile([P, KD, T], BF)
            cw = const1.tile([P, TT, NV], F32)
            yacc = const1.tile([P, TT, D], F32)
            b2r_sb = const1.tile([1, NV, D], BF)
            ones_sb = const1.tile([1, P], BF)
            gwt_sb = const1.tile([P, KD, E], F32)
            gb_sb = const1.tile([1, E], F32)
            zerob = const1.tile([P, 1], F32)
            onesf = const1.tile([1, P], F32)

            nc.sync.dma_start(out=b2r_sb[:], in_=b2r[:])
            nc.sync.dma_start(out=ones_sb[:], in_=ones1[:])
            nc.sync.dma_start(out=gwt_sb[:], in_=gwt[:])
            nc.sync.dma_start(out=gb_sb[:], in_=gb[:])
            nc.vector.memset(zerob[:], 0.0)
            nc.vector.memset(onesf[:], 1.0)

            # ---- gate + bf16 cast of activations, per 128-token tile ----
            for m in range(TT):
                xchunk = gchunk.tile([P, KD, P], F32)
                nc.sync.dma_start(out=xchunk[:], in_=xT[:, :, m * P:(m + 1) * P])
                nc.vector.tensor_copy(xTb[:, :, m * P:(m + 1) * P], xchunk[:])

                pg = ps_g.tile([P, E], F32, space="PSUM")
                for k in range(KD):
                    nc.tensor.matmul(out=pg[:], lhsT=xchunk[:, k, :],
                                     rhs=gwt_sb[:, k, :],
                                     start=(k == 0), stop=False)
                # + gate bias via K=1 matmul with a ones row
                nc.tensor.matmul(out=pg[:], lhsT=onesf[:], rhs=gb_sb[:],
                                 start=False, stop=True)

                lg = gtmp.tile([P, E], F32)
                nc.scalar.activation(lg[:], pg[:], AF.Copy)
                m8 = gtmp.tile([P, 8], F32)
                nc.vector.max(m8[:], lg[:])
                # exp(l - max)
                ex = gtmp.tile([P, E], F32)
                nc.vector.tensor_scalar(out=ex[:], in0=lg[:],
                                        scalar1=m8[:, 0:1], scalar2=None,
                                        op0=OP.subtract)
                nc.scalar.activation(ex[:], ex[:], AF.Exp, bias=zerob[:])
                # top-2 mask
                mask = gtmp.tile([P, E], F32)
                nc.vector.tensor_scalar(out=mask[:], in0=lg[:],
                                        scalar1=m8[:, 1:2], scalar2=None,
                                        op0=OP.is_ge)
                # denom = 1 + exp(second - max);  cw = mask * ex / denom
                e2 = gtmp.tile([P, 1], F32)
                nc.vector.tensor_tensor(out=e2[:], in0=m8[:, 1:2], in1=m8[:, 0:1],
                                        op=OP.subtract)
                nc.scalar.activation(e2[:], e2[:], AF.Exp, bias=zerob[:])
                den = gtmp.tile([P, 1], F32)
                nc.vector.tensor_scalar(out=den[:], in0=e2[:], scalar1=1.0,
                                        scalar2=None, op0=OP.add)
                rec = gtmp.tile([P, 1], F32)
                nc.vector.reciprocal(rec[:], den[:])
                cwm = gtmp.tile([P, E], F32)
                nc.vector.tensor_mul(cwm[:], ex[:], mask[:])
                nc.vector.tensor_scalar(out=cw[:, m, 0:E], in0=cwm[:],
                                        scalar1=rec[:, 0:1], scalar2=None,
                                        op0=OP.mult)
                if NV > E:
                    nc.vector.memset(cw[:, m, E:NV], 1.0)

            # ---- virtual experts ----
            for e in range(NV):
                w2sb = w2s.tile([P, HCN, D], BF)
                nc.sync.dma_start(out=w2sb[:], in_=w2t[e])
                b1sb = b13.tile([P, HCN], F32)
                nc.sync.dma_start(out=b1sb[:], in_=b1a[e])
                b3sb = b13.tile([P, HCN], F32)
                nc.sync.dma_start(out=b3sb[:], in_=b3a[e])

                hT = hpool.tile([P, HCN, T], BF)

                # phase A: hT[h, t] = silu(W1 x + b1) * (W3 x + b3), feature-major
                for hc in range(HCN):
                    w1c = w1s.tile([P, KD, P], BF)
                    nc.sync.dma_start(out=w1c[:], in_=w1t[e, hc])
                    w3c = w1s.tile([P, KD, P], BF)
                    nc.sync.dma_start(out=w3c[:], in_=w3t[e, hc])
                    for ft in range(FT):
                        fsl = slice(ft * 512, min((ft + 1) * 512, T))
                        fw = fsl.stop - fsl.start
                        o1 = ps_l1.tile([P, 512], F32, space="PSUM", name="o1")
                        for k in range(KD):
                            nc.tensor.matmul(out=o1[:, :fw], lhsT=w1c[:, k, :],
                                             rhs=xTb[:, k, fsl],
                                             start=(k == 0), stop=(k == KD - 1))
                        # silu(v) = v * sigmoid(v), v = o1 + b1
                        s1 = s1p.tile([P, 512], F32)
                        nc.scalar.activation(s1[:, :fw], o1[:, :fw], AF.Sigmoid,
                                             bias=b1sb[:, hc:hc + 1])
                        t1 = s1p.tile([P, 512], F32)
                        nc.vector.scalar_tensor_tensor(
                            out=t1[:, :fw], in0=o1[:, :fw],
                            scalar=b1sb[:, hc:hc + 1], in1=s1[:, :fw],
                            op0=OP.add, op1=OP.mult)
                        o3 = ps_l1.tile([P, 512], F32, space="PSUM", name="o3")
                        for k in range(KD):
                            nc.tensor.matmul(out=o3[:, :fw], lhsT=w3c[:, k, :],
                                             rhs=xTb[:, k, fsl],
                                             start=(k == 0), stop=(k == KD - 1))
                        # h = (o3 + b3) * silu_out
                        nc.vector.scalar_tensor_tensor(
                            out=hT[:, hc, fsl], in0=o3[:, :fw],
                            scalar=b3sb[:, hc:hc + 1], in1=t1[:, :fw],
                            op0=OP.add, op1=OP.mult)

                # phase B: yacc[t, d] (+)= cw[t, e] * (hT^T @ W2^T + b2)
                for tt in range(TT):
                    tsl = slice(tt * P, (tt + 1) * P)
                    for dt in range(DT):
                        dsl = slice(dt * 512, min((dt + 1) * 512, D))
                        dw = dsl.stop - dsl.start
                        yp = ps_y.tile([P, 512], F32, space="PSUM", name="yp")
                        nc.tensor.matmul(out=yp[:, :dw], lhsT=ones_sb[:],
                                         rhs=b2r_sb[0:1, e, dsl],
                                         start=True, stop=False)
                        for hc in range(HCN):
                            nc.tensor.matmul(out=yp[:, :dw],
                                             lhsT=hT[:, hc, tsl],
                                             rhs=w2sb[:, hc, dsl],
                                             start=False, stop=(hc == HCN - 1))
                        if e == 0:
                            nc.vector.tensor_scalar(
                                out=yacc[:, tt, dsl], in0=yp[:, :dw],
                                scalar1=cw[:, tt, e:e + 1], scalar2=None,
                                op0=OP.mult)
                        else:
                            nc.vector.scalar_tensor_tensor(
                                out=yacc[:, tt, dsl], in0=yp[:, :dw],
                                scalar=cw[:, tt, e:e + 1],
                                in1=yacc[:, tt, dsl],
                                op0=OP.mult, op1=OP.add)

            nc.sync.dma_start(out=y[:], in_=yacc[:])

    nc.compile()
    return nc


def build_nc_dispatch(cfg: Cfg):
    """Dispatched (capacity-routed) variant, permutation-matmul dispatch.

    Token tiles are processed in pairs: per pair of 128-token tiles and
    routed expert e, a triangular-matmul prefix sum (plus a tiny
    count-broadcast matmul for the odd tile) assigns each routed token a
    slot in a capm-wide bucket. One-hot tiles Pe[t, j] = (slot[t] == j)
    then gather x feature-major via matmul (pad slots become zero
    columns). Each expert runs a dense SwiGLU over its CAPE slots and
    stores unscaled outputs (+b2) per slot in DRAM. The combine phase
    rebuilds Pe scaled by the renormalized gate weight, transposes it on
    the PE, and accumulates y_routed = sum_e Pe2w^T @ ye_bucket in PSUM;
    the shared expert (two H-half "virtual experts" over all tokens) is
    added on top. No indirect DMAs anywhere.
    """
    D, H, E, NV, T = cfg.D, cfg.H, cfg.E, cfg.NV, cfg.T
    KD, HCN, TT, DT, FT = cfg.KD, cfg.HCN, cfg.TT, cfg.DT, cfg.FT
    capm, CAPE, ST, NP = cfg.capm, cfg.CAPE, cfg.ST, cfg.NP

    nc = bacc.Bacc("TRN2", target_bir_lowering=False)

    xT = nc.dram_tensor("xT", [P, KD, T], F32, kind="ExternalInput")
    xtok = nc.dram_tensor("xtok", [P, TT, D], BF, kind="ExternalInput")
    xtb = nc.dram_tensor("xtb", [P, KD, T], BF, kind="ExternalInput")
    w1t = nc.dram_tensor("w1t", [NV, HCN, P, KD, P], BF, kind="ExternalInput")
    w3t = nc.dram_tensor("w3t", [NV, HCN, P, KD, P], BF, kind="ExternalInput")
    w2t = nc.dram_tensor("w2t", [NV, P, HCN, D], BF, kind="ExternalInput")
    b1a = nc.dram_tensor("b1a", [NV, P, HCN], F32, kind="ExternalInput")
    b3a = nc.dram_tensor("b3a", [NV, P, HCN], F32, kind="ExternalInput")
    b2r = nc.dram_tensor("b2r", [1, NV, D], BF, kind="ExternalInput")
    gwt = nc.dram_tensor("gwt", [P, KD, E], F32, kind="ExternalInput")
    gb = nc.dram_tensor("gb", [1, E], F32, kind="ExternalInput")
    ones1 = nc.dram_tensor("ones1", [1, P], BF, kind="ExternalInput")
    onesc = nc.dram_tensor("onesc", [P, 1], BF, kind="ExternalInput")
    lt = nc.dram_tensor("lt", [P, P], BF, kind="ExternalInput")
    ident = nc.dram_tensor("ident", [P, P], BF, kind="ExternalInput")
    iota = nc.dram_tensor("iota", [P, cfg.capm], F32, kind="ExternalInput")
    y = nc.dram_tensor("y", [P, TT, D], F32, kind="ExternalOutput")

    OOB = 3.0e6

    with tile.TileContext(nc) as tc:
        with (
            tc.tile_pool(name="const1", bufs=1) as const1,
            tc.tile_pool(name="gchunk", bufs=2) as gchunk,
            tc.tile_pool(name="gtmp", bufs=4) as gtmp,
            tc.tile_pool(name="w1s", bufs=3) as w1s,
            tc.tile_pool(name="b13", bufs=2) as b13,
            tc.tile_pool(name="w2s", bufs=1) as w2s,
            tc.tile_pool(name="hpool", bufs=1) as hpool,
            tc.tile_pool(name="s1p", bufs=2) as s1p,
            tc.tile_pool(name="yebp", bufs=5) as yebp,
            tc.tile_pool(name="xep", bufs=2) as xep,
            tc.tile_pool(name="pep", bufs=8) as pep,
            tc.tile_pool(name="comb", bufs=2) as comb,
            tc.tile_pool(name="dram", bufs=1, space="DRAM") as drp,
            tc.tile_pool(name="ps_l1", bufs=2, space="PSUM") as ps_l1,
            tc.tile_pool(name="ps_y", bufs=2, space="PSUM") as ps_y,
            tc.tile_pool(name="ps_sm", bufs=2, space="PSUM") as ps_sm,
        ):
            ye = drp.tile([E * CAPE, D], BF)   # per-slot expert outputs

            # ---- resident constants / state ----
            xTb = const1.tile([P, KD, T], BF)
            xtok_sb = const1.tile([P, TT, D], BF)
            yshared = const1.tile([P, TT, D], F32)
            cw = const1.tile([P, TT, E], F32)
            posb_all = const1.tile([P, TT, E], F32)
            ones_sb = const1.tile([1, P], BF)
            onesc_sb = const1.tile([P, 1], BF)
            gwt_sb = const1.tile([P, KD, E], F32)
            gb_sb = const1.tile([1, E], F32)
            zerob = const1.tile([P, 1], F32)
            onesf = const1.tile([1, P], F32)
            lt_sb = const1.tile([P, P], BF)
            id_sb = const1.tile([P, P], BF)
            iota_sb = const1.tile([P, capm], F32)

            nc.sync.dma_start(out=gwt_sb[:], in_=gwt[:])
            nc.sync.dma_start(out=gb_sb[:], in_=gb[:])
            nc.sync.dma_start(out=ones_sb[:], in_=ones1[:])
            nc.sync.dma_start(out=onesc_sb[:], in_=onesc[:])
            nc.sync.dma_start(out=lt_sb[:], in_=lt[:])
            nc.sync.dma_start(out=xTb[:], in_=xtb[:])
            nc.sync.dma_start(out=id_sb[:], in_=ident[:])
            nc.sync.dma_start(out=iota_sb[:], in_=iota[:])
            nc.vector.memset(zerob[:], 0.0)
            nc.vector.memset(onesf[:], 1.0)

            # prefetch the shared expert's first L1 weight chunks so its
            # matmuls can start while the gate phase runs
            pre_w = {}
            for hc in range(min(3, HCN)):
                w1c = w1s.tile([P, KD, P], BF, name="w1c", tag="w1c")
                nc.sync.dma_start(out=w1c[:], in_=w1t[E, hc])
                w3c = w1s.tile([P, KD, P], BF, name="w3c", tag="w3c")
                nc.sync.dma_start(out=w3c[:], in_=w3t[E, hc])
                pre_w[hc] = (w1c, w3c)
            nc.sync.dma_start(out=xtok_sb[:], in_=xtok[:])

            # ---- gate + routing, per token tile (paired buckets) ----
            cntb = None
            for m in range(TT):
                xchunk = gchunk.tile([P, KD, P], F32)
                nc.sync.dma_start(out=xchunk[:], in_=xT[:, :, m * P:(m + 1) * P])

                pg = ps_l1.tile([P, P], F32, space="PSUM", name="pg", tag="o1")
                for k in range(KD):
                    nc.tensor.matmul(out=pg[:, :E], lhsT=xchunk[:, k, :],
                                     rhs=gwt_sb[:, k, :],
                                     start=(k == 0), stop=False)
                nc.tensor.matmul(out=pg[:, :E], lhsT=onesf[:], rhs=gb_sb[:],
                                 start=False, stop=True)

                lg = gtmp.tile([P, E], F32)
                nc.scalar.activation(lg[:], pg[:, :E], AF.Copy)
                m8 = gtmp.tile([P, 8], F32)
                nc.vector.max(m8[:], lg[:])
                ex = gtmp.tile([P, E], F32)
                nc.vector.tensor_scalar(out=ex[:], in0=lg[:],
                                        scalar1=m8[:, 0:1], scalar2=None,
                                        op0=OP.subtract)
                nc.scalar.activation(ex[:], ex[:], AF.Exp, bias=zerob[:])
                mask = gtmp.tile([P, E], F32)
                nc.vector.tensor_scalar(out=mask[:], in0=lg[:],
                                        scalar1=m8[:, 1:2], scalar2=None,
                                        op0=OP.is_ge)
                e2 = gtmp.tile([P, 1], F32)
                nc.vector.tensor_tensor(out=e2[:], in0=m8[:, 1:2],
                                        in1=m8[:, 0:1], op=OP.subtract)
                nc.scalar.activation(e2[:], e2[:], AF.Exp, bias=zerob[:])
                den = gtmp.tile([P, 1], F32)
                nc.vector.tensor_scalar(out=den[:], in0=e2[:], scalar1=1.0,
                                        scalar2=None, op0=OP.add)
                rec = gtmp.tile([P, 1], F32)
                nc.vector.reciprocal(rec[:], den[:])
                cwm = gtmp.tile([P, E], F32)
                nc.vector.tensor_mul(cwm[:], ex[:], mask[:])
                nc.vector.tensor_scalar(out=cw[:, m, :], in0=cwm[:],
                                        scalar1=rec[:, 0:1], scalar2=None,
                                        op0=OP.mult)

                # bucket-local slot: pair prefix(mask) - mask; OOB unrouted
                maskb = gtmp.tile([P, E], BF)
                nc.vector.tensor_copy(maskb[:], mask[:])
                pp = ps_y.tile([P, P], F32, space="PSUM", name="pp", tag="yp")
                if m % 2 == 0:
                    nc.tensor.matmul(out=pp[:, :E], lhsT=lt_sb[:],
                                     rhs=maskb[:], start=True, stop=True)
                    # bucket count of the even tile, for the odd tile
                    cnt_ps = ps_sm.tile([1, P], F32, space="PSUM",
                                        name="cntp", tag="sm")
                    nc.tensor.matmul(out=cnt_ps[0:1, :E], lhsT=onesc_sb[:],
                                     rhs=maskb[:], start=True, stop=True)
                    cntb = gtmp.tile([1, E], BF, name="cntb")
                    nc.scalar.activation(cntb[:], cnt_ps[0:1, :E], AF.Copy)
                else:
                    nc.tensor.matmul(out=pp[:, :E], lhsT=lt_sb[:],
                                     rhs=maskb[:], start=True, stop=False)
                    nc.tensor.matmul(out=pp[:, :E], lhsT=ones_sb[:],
                                     rhs=cntb[:], start=False, stop=True)
                t1m = gtmp.tile([P, E], F32)
                nc.vector.scalar_tensor_tensor(out=t1m[:], in0=mask[:],
                                               scalar=-1.0, in1=pp[:, :E],
                                               op0=OP.mult, op1=OP.add)
                notm = gtmp.tile([P, E], F32)
                nc.vector.tensor_scalar(out=notm[:], in0=mask[:],
                                        scalar1=-1.0, scalar2=1.0,
                                        op0=OP.mult, op1=OP.add)
                nc.vector.scalar_tensor_tensor(out=posb_all[:, m, :],
                                               in0=notm[:], scalar=OOB,
                                               in1=t1m[:],
                                               op0=OP.mult, op1=OP.add)

            # ---- shared expert first (dense over all tokens) ----
            for sv in range(cfg.n_sh):
                e = E + sv
                w2sb = w2s.tile([P, HCN, D], BF)
                b1sb = b13.tile([P, HCN], F32)
                nc.sync.dma_start(out=b1sb[:], in_=b1a[e])
                b3sb = b13.tile([P, HCN], F32)
                nc.sync.dma_start(out=b3sb[:], in_=b3a[e])
                b2e = b13.tile([1, D], BF)
                nc.sync.dma_start(out=b2e[:], in_=b2r[0:1, e, :])

                hT = hpool.tile([P, HCN, T], BF, name="hT", tag="hT")
                for hc in range(HCN):
                    if sv == 0 and hc in pre_w:
                        w1c, w3c = pre_w[hc]
                    else:
                        w1c = w1s.tile([P, KD, P], BF, name="w1c", tag="w1c")
                        nc.sync.dma_start(out=w1c[:], in_=w1t[e, hc])
                        w3c = w1s.tile([P, KD, P], BF, name="w3c", tag="w3c")
                        nc.sync.dma_start(out=w3c[:], in_=w3t[e, hc])
                    for ft in range(FT):
                        fsl = slice(ft * 512, min((ft + 1) * 512, T))
                        fw = fsl.stop - fsl.start
                        o1 = ps_l1.tile([P, 512], F32, space="PSUM", name="o1")
                        for k in range(KD):
                            nc.tensor.matmul(out=o1[:, :fw], lhsT=w1c[:, k, :],
                                             rhs=xTb[:, k, fsl],
                                             start=(k == 0), stop=(k == KD - 1))
                        s1 = s1p.tile([P, 512], F32)
                        nc.scalar.activation(s1[:, :fw], o1[:, :fw], AF.Sigmoid,
                                             bias=b1sb[:, hc:hc + 1])
                        t1 = s1p.tile([P, 512], F32)
                        nc.vector.scalar_tensor_tensor(
                            out=t1[:, :fw], in0=o1[:, :fw],
                            scalar=b1sb[:, hc:hc + 1], in1=s1[:, :fw],
                            op0=OP.add, op1=OP.mult)
                        o3 = ps_l1.tile([P, 512], F32, space="PSUM", name="o3")
                        for k in range(KD):
                            nc.tensor.matmul(out=o3[:, :fw], lhsT=w3c[:, k, :],
                                             rhs=xTb[:, k, fsl],
                                             start=(k == 0), stop=(k == KD - 1))
                        nc.vector.scalar_tensor_tensor(
                            out=hT[:, hc, fsl], in0=o3[:, :fw],
                            scalar=b3sb[:, hc:hc + 1], in1=t1[:, :fw],
                            op0=OP.add, op1=OP.mult)

                nc.sync.dma_start(out=w2sb[:], in_=w2t[e])
                for tt in range(TT):
                    tsl = slice(tt * P, (tt + 1) * P)
                    for dt in range(DT):
                        dsl = slice(dt * 512, min((dt + 1) * 512, D))
                        dw = dsl.stop - dsl.start
                        yp = ps_y.tile([P, 512], F32, space="PSUM", name="yp")
                        nc.tensor.matmul(out=yp[:, :dw], lhsT=ones_sb[:],
                                         rhs=b2e[0:1, dsl],
                                         start=True, stop=False)
                        for hc in range(HCN):
                            nc.tensor.matmul(out=yp[:, :dw],
                                             lhsT=hT[:, hc, tsl],
                                             rhs=w2sb[:, hc, dsl],
                                             start=False, stop=(hc == HCN - 1))
                        if sv == 0:
                            nc.vector.tensor_copy(yshared[:, tt, dsl],
                                                  yp[:, :dw])
                        else:
                            nc.vector.tensor_add(yshared[:, tt, dsl],
                                                 yshared[:, tt, dsl],
                                                 yp[:, :dw])

            # ---- routed experts over dispatched slots ----
            for e in range(E):
                w2sb = w2s.tile([P, HCN, D], BF)
                b1sb = b13.tile([P, HCN], F32)
                nc.sync.dma_start(out=b1sb[:], in_=b1a[e])
                b3sb = b13.tile([P, HCN], F32)
                nc.sync.dma_start(out=b3sb[:], in_=b3a[e])
                b2e = b13.tile([1, D], BF)
                nc.sync.dma_start(out=b2e[:], in_=b2r[0:1, e, :])

                # matmul gather: xeT_k[:, pr, :] = sum_pair x_m^T @ Pe_m.
                # Per-k tiles so layer 1's k-th accumulation step only
                # depends on gather step k (gather pipelines under L1).
                pes = []
                for m in range(TT):
                    pe = pep.tile([P, capm], BF, name="pe", tag="pe")
                    nc.vector.tensor_scalar(
                        out=pe[:], in0=iota_sb[:],
                        scalar1=posb_all[:, m, e:e + 1],
                        scalar2=None, op0=OP.is_equal)
                    pes.append(pe)
                xeT_k = []
                for k in range(KD):
                    xk = xep.tile([P, NP, capm], BF, name=f"xeT{k}",
                                  tag=f"xeT{k}")
                    for pp0 in range(0, NP, 2):
                        npp = min(2, NP - pp0)
                        gx = ps_sm.tile([P, 2 * capm], F32, space="PSUM",
                                        name="gx", tag="sm")
                        for pi, m2 in [(a, b) for a in range(npp)
                                       for b in range(2)]:
                            pr = pp0 + pi
                            csl = slice(pi * capm, (pi + 1) * capm)
                            m = 2 * pr + m2
                            nc.tensor.matmul(
                                out=gx[:, csl],
                                lhsT=xtok_sb[:, m, k * P:(k + 1) * P],
                                rhs=pes[m][:], start=(m2 == 0),
                                stop=(m2 == 1))
                        nc.scalar.activation(
                            xk[:, pp0:pp0 + npp, :],
                            gx[:, :npp * capm], AF.Copy)
                    xeT_k.append(xk)

                hT = hpool.tile([P, HCN, T], BF, name="hT", tag="hT")
                for hc in range(HCN):
                    w1c = w1s.tile([P, KD, P], BF, name="w1c", tag="w1c")
                    nc.sync.dma_start(out=w1c[:], in_=w1t[e, hc])
                    w3c = w1s.tile([P, KD, P], BF, name="w3c", tag="w3c")
                    nc.sync.dma_start(out=w3c[:], in_=w3t[e, hc])
                    o1 = ps_l1.tile([P, 512], F32, space="PSUM", name="o1")
                    for k in range(KD):
                        nc.tensor.matmul(out=o1[:, :CAPE], lhsT=w1c[:, k, :],
                                         rhs=xeT_k[k][:, :, :],
                                         start=(k == 0), stop=(k == KD - 1))
                    s1 = s1p.tile([P, 512], F32)
                    nc.scalar.activation(s1[:, :CAPE], o1[:, :CAPE], AF.Sigmoid,
                                         bias=b1sb[:, hc:hc + 1])
                    t1 = s1p.tile([P, 512], F32)
                    nc.vector.scalar_tensor_tensor(
                        out=t1[:, :CAPE], in0=o1[:, :CAPE],
                        scalar=b1sb[:, hc:hc + 1], in1=s1[:, :CAPE],
                        op0=OP.add, op1=OP.mult)
                    o3 = ps_l1.tile([P, 512], F32, space="PSUM", name="o3")
                    for k in range(KD):
                        nc.tensor.matmul(out=o3[:, :CAPE], lhsT=w3c[:, k, :],
                                         rhs=xeT_k[k][:, :, :],
                                         start=(k == 0), stop=(k == KD - 1))
                    nc.vector.scalar_tensor_tensor(
                        out=hT[:, hc, :CAPE], in0=o3[:, :CAPE],
                        scalar=b3sb[:, hc:hc + 1], in1=t1[:, :CAPE],
                        op0=OP.add, op1=OP.mult)

                nc.sync.dma_start(out=w2sb[:], in_=w2t[e])
                for st in range(ST):
                    sw = min(P, CAPE - st * P)
                    ssl = slice(st * P, st * P + sw)
                    for dt in range(DT):
                        dsl = slice(dt * 512, min((dt + 1) * 512, D))
                        dw = dsl.stop - dsl.start
                        yp = ps_y.tile([P, 512], F32, space="PSUM", name="yp")
                        nc.tensor.matmul(out=yp[:sw, :dw], lhsT=ones_sb[:, :sw],
                                         rhs=b2e[0:1, dsl],
                                         start=True, stop=False)
                        for hc in range(HCN):
                            nc.tensor.matmul(out=yp[:sw, :dw],
                                             lhsT=hT[:, hc, ssl],
                                             rhs=w2sb[:, hc, dsl],
                                             start=False, stop=(hc == HCN - 1))
                        yeb = comb.tile([P, 512], BF, name="yeb")
                        nc.scalar.activation(yeb[:sw, :dw], yp[:sw, :dw],
                                             AF.Copy)
                        nc.sync.dma_start(
                            out=ye[e * CAPE + st * P: e * CAPE + st * P + sw,
                                   dsl],
                            in_=yeb[:sw, :dw])

            # ---- combine: y = yshared + sum_e cw_e * ye[slot] ----
            for m in range(TT):
                pr = m // 2
                yps = []
                pool_c, tag_c = (ps_y, "yp") if m % 2 == 0 else (ps_l1, "o1")
                for dt in range(DT):
                    yps.append(pool_c.tile([P, 512], F32, space="PSUM",
                                           name=f"ypc{dt}", tag=tag_c))
                for e in range(E):
                    yeb_sb = yebp.tile([capm, D], BF)
                    nc.sync.dma_start(
                        out=yeb_sb[:],
                        in_=ye[e * CAPE + pr * capm:
                               e * CAPE + (pr + 1) * capm, :])
                    pe = gtmp.tile([P, capm], BF, name="pe")
                    nc.vector.tensor_scalar(out=pe[:], in0=iota_sb[:],
                                            scalar1=posb_all[:, m, e:e + 1],
                                            scalar2=None, op0=OP.is_equal)
                    pew = gtmp.tile([P, capm], BF, name="pew")
                    nc.vector.tensor_scalar(out=pew[:], in0=pe[:],
                                            scalar1=cw[:, m, e:e + 1],
                                            scalar2=None, op0=OP.mult)
                    pool_t, tag_t = (ps_sm, "sm") if e % 2 == 0 else (ps_l1, "o3")
                    p2 = pool_t.tile([P, P], BF, space="PSUM",
                                     name="p2", tag=tag_t)
                    nc.tensor.transpose(out=p2[:capm, :], in_=pew[:],
                                        identity=id_sb[:])
                    p2s = gtmp.tile([capm, P], BF, name="p2s")
                    nc.scalar.activation(p2s[:], p2[:capm, :], AF.Copy)
                    for dt in range(DT):
                        dsl = slice(dt * 512, min((dt + 1) * 512, D))
                        dw = dsl.stop - dsl.start
                        nc.tensor.matmul(out=yps[dt][:, :dw], lhsT=p2s[:],
                                         rhs=yeb_sb[:, dsl],
                                         start=(e == 0), stop=(e == E - 1))
                for dt in range(DT):
                    dsl = slice(dt * 512, min((dt + 1) * 512, D))
                    dw = dsl.stop - dsl.start
                    yt = comb.tile([P, 512], F32, name="yt")
                    nc.vector.tensor_add(yt[:, :dw], yshared[:, m, dsl],
                                         yps[dt][:, :dw])
                    nc.sync.dma_start(out=y[:, m, dsl], in_=yt[:, :dw])

    nc.compile()
    return nc


# ---------------- host-side packing ----------------

def pack_static(cfg: Cfg, gate_w, gate_b, w1, b1, w2, b2, w3, b3,
                sw1, sb1, sw2, sb2, sw3, sb3):
    D, H, E, NV, n_sh = cfg.D, cfg.H, cfg.E, cfg.NV, cfg.n_sh
    KD, HCN = cfg.KD, cfg.HCN

    w1T = np.transpose(w1, (0, 2, 1))                      # [E, D, H]
    w3T = np.transpose(w3, (0, 2, 1))
    w2T = np.transpose(w2, (0, 2, 1))                      # [E, H, D]
    s1T = sw1.T.reshape(D, n_sh, H).transpose(1, 0, 2)     # [n_sh, D, H]
    s3T = sw3.T.reshape(D, n_sh, H).transpose(1, 0, 2)
    s2T = sw2.T.reshape(n_sh, H, D)                        # [n_sh, H, D]
    w1T_all = np.concatenate([w1T, s1T], 0)                # [NV, D, H]
    w3T_all = np.concatenate([w3T, s3T], 0)
    w2T_all = np.concatenate([w2T, s2T], 0)                # [NV, H, D]

    w1t = np.ascontiguousarray(
        w1T_all.reshape(NV, KD, P, HCN, P).transpose(0, 3, 2, 1, 4)).astype(BF16)
    w3t = np.ascontiguousarray(
        w3T_all.reshape(NV, KD, P, HCN, P).transpose(0, 3, 2, 1, 4)).astype(BF16)
    w2t = np.ascontiguousarray(
        w2T_all.reshape(NV, HCN, P, D).transpose(0, 2, 1, 3)).astype(BF16)

    b1_all = np.concatenate([b1, sb1.reshape(n_sh, H)], 0)  # [NV, H]
    b3_all = np.concatenate([b3, sb3.reshape(n_sh, H)], 0)
    b1a = np.ascontiguousarray(
        b1_all.reshape(NV, HCN, P).transpose(0, 2, 1)).astype(np.float32)
    b3a = np.ascontiguousarray(
        b3_all.reshape(NV, HCN, P).transpose(0, 2, 1)).astype(np.float32)

    b2_all = np.concatenate(
        [b2, sb2[None], np.zeros((n_sh - 1, D), np.float32)], 0)  # [NV, D]
    b2r = b2_all[None].astype(BF16)                         # [1, NV, D]

    gwt = np.ascontiguousarray(
        gate_w.T.reshape(KD, P, E).transpose(1, 0, 2)).astype(np.float32)
    gb = gate_b[None].astype(np.float32)
    ones1 = np.ones((1, P), BF16)

    return dict(w1t=w1t, w3t=w3t, w2t=w2t, b1a=b1a, b3a=b3a, b2r=b2r,
                gwt=gwt, gb=gb, ones1=ones1)


def pack_dispatch_consts(cfg: Cfg):
    lt = np.triu(np.ones((P, P))).astype(BF16)          # lt[k, j] = k <= j
    ident = np.eye(P).astype(BF16)
    iota = np.tile(np.arange(cfg.capm, dtype=np.float32), (P, 1))
    onesc = np.ones((P, 1), BF16)
    return dict(lt=lt, ident=ident, iota=iota, onesc=onesc)


def pack_xtok(cfg: Cfg, x_tokens):
    """x_tokens [T, D] fp32 -> token-major [P, TT, D] bf16."""
    T, D = x_tokens.shape
    xt = x_tokens.reshape(cfg.TT, P, D).transpose(1, 0, 2)
    return np.ascontiguousarray(xt).astype(BF16)


def pack_xT(cfg: Cfg, x_tokens):
    """x_tokens [T, D] fp32 -> xT device layout [P, KD, T]."""
    T, D = x_tokens.shape
    xT = x_tokens.T.reshape(cfg.KD, P, T).transpose(1, 0, 2)
    return np.ascontiguousarray(xT).astype(np.float32)


def unpack_y(cfg: Cfg, y_dev):
    """y device layout [P, TT, D] -> [T, D]."""
    return np.ascontiguousarray(y_dev.transpose(1, 0, 2).reshape(cfg.T, cfg.D))


_CACHE = {}
DISPATCH = True


def _get_nc(cfg: Cfg, dispatch=None):
    if dispatch is None:
        dispatch = DISPATCH
    key = (cfg.D, cfg.H, cfg.E, cfg.n_sh, cfg.T, cfg.capm, dispatch)
    if key not in _CACHE:
        _CACHE[key] = (build_nc_dispatch(cfg) if dispatch else build_nc(cfg))
    return _CACHE[key]


def make_in_maps(cfg: Cfg, inputs, dispatch=None):
    static = pack_static(
        cfg,
        np.asarray(inputs["gate_w"], np.float32), np.asarray(inputs["gate_b"], np.float32),
        np.asarray(inputs["w1"], np.float32), np.asarray(inputs["b1"], np.float32),
        np.asarray(inputs["w2"], np.float32), np.asarray(inputs["b2"], np.float32),
        np.asarray(inputs["w3"], np.float32), np.asarray(inputs["b3"], np.float32),
        np.asarray(inputs["sw1"], np.float32), np.asarray(inputs["sb1"], np.float32),
        np.asarray(inputs["sw2"], np.float32), np.asarray(inputs["sb2"], np.float32),
        np.asarray(inputs["sw3"], np.float32), np.asarray(inputs["sb3"], np.float32),
    )
    if dispatch is None:
        dispatch = DISPATCH
    if dispatch:
        static.update(pack_dispatch_consts(cfg))
    x = np.asarray(inputs["x"], np.float32)
    B, S, D = x.shape
    xf = x.reshape(-1, D)
    in_maps = []
    for c in range(cfg.n_cores):
        m = dict(static)
        xc = xf[c * cfg.T:(c + 1) * cfg.T]
        m["xT"] = pack_xT(cfg, xc)
        if dispatch:
            m["xtok"] = pack_xtok(cfg, xc)
            m["xtb"] = m["xT"].astype(BF16)
        in_maps.append(m)
    return in_maps


def kernel(**inputs) -> np.ndarray:
    x = np.asarray(inputs["x"], np.float32)
    B, S, D = x.shape
    N = B * S
    cfg = Cfg(D=D, T=N // 8, n_cores=8)
    nc = _get_nc(cfg)
    in_maps = make_in_maps(cfg, inputs)
    res = run_bass_kernel_spmd(nc, in_maps, list(range(cfg.n_cores)))
    outs = [unpack_y(cfg, res.results[c]["y"]) for c in range(cfg.n_cores)]
    return np.concatenate(outs, 0).reshape(B, S, D)



# revision 19
# speedup vs baseline: 1.1535x; 1.1535x over previous
"""MoE (8 routed experts, top-2, + shared expert) on 8 NeuronCores.

Data-parallel over tokens (1024/core), weights replicated. The host
load-balances token->bucket assignment (any sharding is allowed) so the
per-(256-token-bucket, expert) routed count is ~uniform (max 72 on this
model's routing), letting the capacity-dispatched kernel run with
capm=74 slots per bucket instead of the binomial-tail 96.

Device kernel (per core):
  1. Gate in fp32 (matches reference routing bit-for-bit for the
     observed >=1.7e-4 top-2/3 logit gaps), renormalized top-2 combine
     weights cw, and bucket-local slot positions via triangular-matmul
     prefix sums.
  2. Routed experts e=0..7: one-hot matmul gather of x into CAPE=296
     slots, SwiGLU L1 (feature-major, free dim = slots), L2 in
     d-partition orientation (out[d, slot], bias via activation), then
     PE transposes to slot-major ye tiles held in SBUF.
  3. Tail: shared expert (8 sub-experts of hidden 512) interleaved with
     the scatter-combine (transposed scaled one-hots x ye), everything
     accumulating into a token-major f32 yacc; last shared sub-expert's
     L2 fuses the final add and streams y out.

Matmuls are bf16 with fp32 accumulation; weight DMA is split across the
SP/Pool/Act queues to avoid head-of-line blocking on one DMA queue.
"""

import numpy as np
import ml_dtypes

import concourse.bacc as bacc
import concourse.bass as bass
import concourse.tile as tile
import concourse.mybir as mybir
from concourse.bass_utils import run_bass_kernel_spmd

BF16 = ml_dtypes.bfloat16
F32 = mybir.dt.float32
BF = mybir.dt.bfloat16
AF = mybir.ActivationFunctionType
OP = mybir.AluOpType

P = 128


class Cfg:
    def __init__(self, D=1024, H=2048, E=8, n_sh=2, T=1024, n_cores=8,
                 capms=(74,) * 8):
        self.D, self.H, self.E, self.n_sh, self.T = D, H, E, n_sh, T
        self.NV = E + n_sh          # packed weight rows (8 routed + 2 shared)
        self.HS = n_sh * H          # shared hidden total (4096)
        self.KD = D // P            # contraction chunks over D
        self.HCN = H // P           # h chunks per packed VE
        self.TT = T // P            # token 128-tiles per core
        self.FT = T // 512          # shared L1 free 512-tiles
        self.DT = D // 512          # 512-wide d tiles
        self.n_cores = n_cores
        self.capms = tuple(capms)   # slots per (256-token bucket, expert)
        self.capm = max(self.capms)  # iota / tile sizing width
        self.NP = self.TT // 2      # buckets per core (pair of tiles)
        self.NSH = 8                # shared sub-experts
        self.HQ = (self.HS // P) // self.NSH  # h-chunks per sub-expert (4)


def build_nc_v2(cfg: Cfg):
    D, H, E, T = cfg.D, cfg.H, cfg.E, cfg.T
    KD, HCN, TT, FT, DT = cfg.KD, cfg.HCN, cfg.TT, cfg.FT, cfg.DT
    capm, NP = cfg.capm, cfg.NP
    NSH, HQ = cfg.NSH, cfg.HQ

    nc = bacc.Bacc("TRN2", target_bir_lowering=False)

    xT = nc.dram_tensor("xT", [P, KD, T], F32, kind="ExternalInput")
    xtok = nc.dram_tensor("xtok", [P, TT, D], BF, kind="ExternalInput")
    xtb = nc.dram_tensor("xtb", [P, KD, T], BF, kind="ExternalInput")
    w1t = nc.dram_tensor("w1t", [cfg.NV, HCN, P, KD, P], BF, kind="ExternalInput")
    w3t = nc.dram_tensor("w3t", [cfg.NV, HCN, P, KD, P], BF, kind="ExternalInput")
    w2t = nc.dram_tensor("w2t", [cfg.NV, P, HCN, D], BF, kind="ExternalInput")
    b1a = nc.dram_tensor("b1a", [cfg.NV, P, HCN], F32, kind="ExternalInput")
    b3a = nc.dram_tensor("b3a", [cfg.NV, P, HCN], F32, kind="ExternalInput")
    b2r = nc.dram_tensor("b2r", [1, cfg.NV, D], BF, kind="ExternalInput")
    b2c = nc.dram_tensor("b2c", [P, E * KD], F32, kind="ExternalInput")
    gwt = nc.dram_tensor("gwt", [P, KD, E], F32, kind="ExternalInput")
    gb = nc.dram_tensor("gb", [1, E], F32, kind="ExternalInput")
    ones1 = nc.dram_tensor("ones1", [1, P], BF, kind="ExternalInput")
    onesc = nc.dram_tensor("onesc", [P, 1], BF, kind="ExternalInput")
    lt = nc.dram_tensor("lt", [P, P], BF, kind="ExternalInput")
    ident = nc.dram_tensor("ident", [P, P], BF, kind="ExternalInput")
    iota = nc.dram_tensor("iota", [P, capm], F32, kind="ExternalInput")
    y = nc.dram_tensor("y", [P, TT, D], F32, kind="ExternalOutput")

    OOB = 3.0e6

    from contextlib import ExitStack
    with tile.TileContext(nc) as tc:
        with ExitStack() as stack:
            pool_specs = dict(
                const1=dict(bufs=1), xbig=dict(bufs=1),
                gchunk=dict(bufs=2), gtmp=dict(bufs=4),
                w13=dict(bufs=3), w2p=dict(bufs=2), b13=dict(bufs=2),
                xep=dict(bufs=1), hrout=dict(bufs=1), hshp=dict(bufs=2),
                yeBp=dict(bufs=1), pep=dict(bufs=2), s1p=dict(bufs=2),
                combp=dict(bufs=8), ytp=dict(bufs=2),
                ps_o1=dict(bufs=2, space="PSUM"),
                ps_o3=dict(bufs=2, space="PSUM"),
                ps_l2=dict(bufs=2, space="PSUM"),
                ps_tr=dict(bufs=2, space="PSUM"),
            )
            pools = {n: stack.enter_context(tc.tile_pool(name=n, **kw))
                     for n, kw in pool_specs.items()}
            (const1, xbig, gchunk, gtmp, w13, w2p, b13, xep, hrout, hshp,
             yeBp, pep, s1p, combp, ytp, ps_o1, ps_o3, ps_l2, ps_tr) = (
                pools[n] for n in (
                    "const1", "xbig", "gchunk", "gtmp", "w13", "w2p", "b13",
                    "xep", "hrout", "hshp", "yeBp", "pep", "s1p", "combp",
                    "ytp", "ps_o1", "ps_o3", "ps_l2", "ps_tr"))
            # ---- resident state ----
            # xtok_sb and xTb share one 2MB buffer (tag xb): xtok is dead
            # after the last gather; xTb is DMA'd into the same space then.
            xtok_sb = xbig.tile([P, TT, D], BF, name="xtok_sb", tag="xb")
            yacc = const1.tile([P, TT, D], F32)
            cw = const1.tile([P, TT, E], F32)
            posb_all = const1.tile([P, TT, E], F32)
            ye_sb = const1.tile([P, E * NP, D], BF)
            gwt_sb = const1.tile([P, KD, E], F32)
            gb_sb = const1.tile([1, E], F32)
            ones_sb = const1.tile([1, P], BF)
            onesc_sb = const1.tile([P, 1], BF)
            lt_sb = const1.tile([P, P], BF)
            id_sb = const1.tile([P, P], BF)
            iota_sb = const1.tile([P, capm], F32)
            b2c_sb = const1.tile([P, E * KD], F32)
            b2sh_sb = const1.tile([1, D], BF)
            zerob = const1.tile([P, 1], F32)
            onesf = const1.tile([1, P], F32)

            nc.sync.dma_start(out=gwt_sb[:], in_=gwt[:])
            nc.sync.dma_start(out=gb_sb[:], in_=gb[:])
            nc.sync.dma_start(out=ones_sb[:], in_=ones1[:])
            nc.sync.dma_start(out=onesc_sb[:], in_=onesc[:])
            nc.sync.dma_start(out=lt_sb[:], in_=lt[:])
            nc.sync.dma_start(out=id_sb[:], in_=ident[:])
            nc.sync.dma_start(out=iota_sb[:], in_=iota[:])
            nc.sync.dma_start(out=b2c_sb[:], in_=b2c[:])
            nc.scalar.dma_start(out=b2sh_sb[:], in_=b2r[0:1, E, :])
            nc.scalar.dma_start(out=xtok_sb[:], in_=xtok[:])
            nc.vector.memset(zerob[:], 0.0)
            nc.vector.memset(onesf[:], 1.0)

            # prefetch expert 0's first L1 weight chunks
            pre_w = {}
            for hc in range(3):
                w1c = w13.tile([P, KD, P], BF, name="w1c", tag="w1c")
                nc.sync.dma_start(out=w1c[:], in_=w1t[0, hc])
                w3c = w13.tile([P, KD, P], BF, name="w3c", tag="w3c")
                nc.gpsimd.dma_start(out=w3c[:], in_=w3t[0, hc])
                pre_w[hc] = (w1c, w3c)

            # ---- gate + bucket positions, per 128-token tile ----
            cntb = None
            for m in range(TT):
                xchunk = gchunk.tile([P, KD, P], F32)
                nc.sync.dma_start(out=xchunk[:], in_=xT[:, :, m * P:(m + 1) * P])

                pg = ps_l2.tile([P, E], F32, space="PSUM", name="pg", tag="l2")
                for k in range(KD):
                    nc.tensor.matmul(out=pg[:], lhsT=xchunk[:, k, :],
                                     rhs=gwt_sb[:, k, :],
                                     start=(k == 0), stop=False)
                nc.tensor.matmul(out=pg[:], lhsT=onesf[:], rhs=gb_sb[:],
                                 start=False, stop=True)

                lg = gtmp.tile([P, E], F32)
                nc.scalar.activation(lg[:], pg[:], AF.Copy)
                m8 = gtmp.tile([P, 8], F32)
                nc.vector.max(m8[:], lg[:])
                ex = gtmp.tile([P, E], F32)
                nc.vector.tensor_scalar(out=ex[:], in0=lg[:],
                                        scalar1=m8[:, 0:1], scalar2=None,
                                        op0=OP.subtract)
                nc.scalar.activation(ex[:], ex[:], AF.Exp, bias=zerob[:])
                mask = gtmp.tile([P, E], F32)
                nc.vector.tensor_scalar(out=mask[:], in0=lg[:],
                                        scalar1=m8[:, 1:2], scalar2=None,
                                        op0=OP.is_ge)
                e2 = gtmp.tile([P, 1], F32)
                nc.vector.tensor_tensor(out=e2[:], in0=m8[:, 1:2],
                                        in1=m8[:, 0:1], op=OP.subtract)
                nc.scalar.activation(e2[:], e2[:], AF.Exp, bias=zerob[:])
                den = gtmp.tile([P, 1], F32)
                nc.vector.tensor_scalar(out=den[:], in0=e2[:], scalar1=1.0,
                                        scalar2=None, op0=OP.add)
                rec = gtmp.tile([P, 1], F32)
                nc.vector.reciprocal(rec[:], den[:])
                cwm = gtmp.tile([P, E], F32)
                nc.vector.tensor_mul(cwm[:], ex[:], mask[:])
                nc.vector.tensor_scalar(out=cw[:, m, :], in0=cwm[:],
                                        scalar1=rec[:, 0:1], scalar2=None,
                                        op0=OP.mult)

                # bucket-local slot: pair prefix(mask) - mask; OOB unrouted
                maskb = gtmp.tile([P, E], BF)
                nc.vector.tensor_copy(maskb[:], mask[:])
                pp = ps_tr.tile([P, E], F32, space="PSUM", name="pp", tag="tr")
                if m % 2 == 0:
                    nc.tensor.matmul(out=pp[:], lhsT=lt_sb[:],
                                     rhs=maskb[:], start=True, stop=True)
                    cnt_ps = ps_tr.tile([1, E], F32, space="PSUM",
                                        name="cntp", tag="tr")
                    nc.tensor.matmul(out=cnt_ps[:], lhsT=onesc_sb[:],
                                     rhs=maskb[:], start=True, stop=True)
                    cntb = gtmp.tile([1, E], BF, name="cntb")
                    nc.scalar.activation(cntb[:], cnt_ps[:], AF.Copy)
                else:
                    nc.tensor.matmul(out=pp[:], lhsT=lt_sb[:],
                                     rhs=maskb[:], start=True, stop=False)
                    nc.tensor.matmul(out=pp[:], lhsT=ones_sb[:],
                                     rhs=cntb[:], start=False, stop=True)
                t1m = gtmp.tile([P, E], F32)
                nc.vector.scalar_tensor_tensor(out=t1m[:], in0=mask[:],
                                               scalar=-1.0, in1=pp[:],
                                               op0=OP.mult, op1=OP.add)
                notm = gtmp.tile([P, E], F32)
                nc.vector.tensor_scalar(out=notm[:], in0=mask[:],
                                        scalar1=-1.0, scalar2=1.0,
                                        op0=OP.mult, op1=OP.add)
                nc.vector.scalar_tensor_tensor(out=posb_all[:, m, :],
                                               in0=notm[:], scalar=OOB,
                                               in1=t1m[:],
                                               op0=OP.mult, op1=OP.add)

            # ---- routed experts over dispatched slots ----
            for e in range(E):
                capm_e = cfg.capms[e]
                CAPE = NP * capm_e
                b1sb = b13.tile([P, HCN], F32, name="b1sb", tag="b1")
                nc.sync.dma_start(out=b1sb[:], in_=b1a[e])
                b3sb = b13.tile([P, HCN], F32, name="b3sb", tag="b3")
                nc.sync.dma_start(out=b3sb[:], in_=b3a[e])

                # one-hot dispatch tiles for all 8 token tiles
                pe_all = pep.tile([P, TT, capm_e], BF, name="pe_all", tag="pe")
                for m in range(TT):
                    nc.vector.tensor_scalar(
                        out=pe_all[:, m, :], in0=iota_sb[:, :capm_e],
                        scalar1=posb_all[:, m, e:e + 1],
                        scalar2=None, op0=OP.is_equal)

                # matmul gather: xeT[k][d, slot] = sum_m x_m^T @ Pe_m
                xeT = xep.tile([P, KD, CAPE], BF, name="xeT", tag="xeT")
                for k in range(KD):
                    gxp, gxt = (ps_l2, "l2") if k % 2 == 0 else (ps_tr, "tr")
                    gx = gxp.tile([P, CAPE], F32, space="PSUM",
                                  name="gx", tag=gxt)
                    for pr in range(NP):
                        for h in range(2):
                            m = 2 * pr + h
                            nc.tensor.matmul(
                                out=gx[:, pr * capm_e:(pr + 1) * capm_e],
                                lhsT=xtok_sb[:, m, k * P:(k + 1) * P],
                                rhs=pe_all[:, m, :],
                                start=(h == 0), stop=(h == 1))
                    nc.vector.tensor_copy(xeT[:, k, :], gx[:])

                # L1: hT[h, slot] = silu(W1 xe + b1) * (W3 xe + b3)
                hT = hrout.tile([P, HCN, CAPE], BF, name="hT", tag="hT")
                for hc in range(HCN):
                    if e == 0 and hc in pre_w:
                        w1c, w3c = pre_w[hc]
                    else:
                        w1c = w13.tile([P, KD, P], BF, name="w1c", tag="w1c")
                        nc.sync.dma_start(out=w1c[:], in_=w1t[e, hc])
                        w3c = w13.tile([P, KD, P], BF, name="w3c", tag="w3c")
                        nc.gpsimd.dma_start(out=w3c[:], in_=w3t[e, hc])
                    o1 = ps_o1.tile([P, CAPE], F32, space="PSUM",
                                    name="o1", tag="o1")
                    for k in range(KD):
                        nc.tensor.matmul(out=o1[:], lhsT=w1c[:, k, :],
                                         rhs=xeT[:, k, :],
                                         start=(k == 0), stop=(k == KD - 1))
                    s1 = s1p.tile([P, CAPE], F32, name="s1", tag="s1")
                    nc.scalar.activation(s1[:], o1[:], AF.Sigmoid,
                                         bias=b1sb[:, hc:hc + 1])
                    t1 = s1p.tile([P, CAPE], F32, name="t1", tag="t1")
                    nc.vector.scalar_tensor_tensor(
                        out=t1[:], in0=o1[:],
                        scalar=b1sb[:, hc:hc + 1], in1=s1[:],
                        op0=OP.add, op1=OP.mult)
                    o3 = ps_o3.tile([P, CAPE], F32, space="PSUM",
                                    name="o3", tag="o3")
                    for k in range(KD):
                        nc.tensor.matmul(out=o3[:], lhsT=w3c[:, k, :],
                                         rhs=xeT[:, k, :],
                                         start=(k == 0), stop=(k == KD - 1))
                    nc.vector.scalar_tensor_tensor(
                        out=hT[:, hc, :], in0=o3[:],
                        scalar=b3sb[:, hc:hc + 1], in1=t1[:],
                        op0=OP.add, op1=OP.mult)

                # L2 (d-partition orientation) + bias, then transpose to
                # slot-major ye tiles; transposes staggered one dc behind
                # the chains so their yeB reads never stall the PE.
                yeB = yeBp.tile([P, KD, CAPE], BF, name="yeB", tag="yeB")
                pend = []

                def emit_transp(dc, e=e, yeB=yeB, capm_e=capm_e):
                    for pr in range(NP):
                        p2t = ps_tr.tile([P, P], BF, space="PSUM",
                                         name="p2t", tag="tr")
                        nc.tensor.transpose(
                            out=p2t[:capm_e, :],
                            in_=yeB[:, dc, pr * capm_e:(pr + 1) * capm_e],
                            identity=id_sb[:])
                        nc.scalar.activation(
                            ye_sb[0:capm_e, e * NP + pr, dc * P:(dc + 1) * P],
                            p2t[:capm_e, :], AF.Copy)

                for dq in range(4):
                    w2q = w2p.tile([P, HCN, 256], BF, name="w2q", tag="w2q")
                    nc.scalar.dma_start(out=w2q[:],
                                        in_=w2t[e][:, :, dq * 256:(dq + 1) * 256])
                    for dc2 in range(2):
                        dc = dq * 2 + dc2
                        pl2 = ps_l2.tile([P, CAPE], F32, space="PSUM",
                                         name="pl2", tag="l2")
                        for hc in range(HCN):
                            nc.tensor.matmul(
                                out=pl2[:],
                                lhsT=w2q[:, hc, dc2 * P:(dc2 + 1) * P],
                                rhs=hT[:, hc, :],
                                start=(hc == 0), stop=(hc == HCN - 1))
                        nc.vector.tensor_scalar(
                            out=yeB[:, dc, :], in0=pl2[:],
                            scalar1=b2c_sb[:, e * KD + dc:e * KD + dc + 1],
                            scalar2=None, op0=OP.add)
                        if pend:
                            emit_transp(pend.pop())
                        pend.append(dc)
                while pend:
                    emit_transp(pend.pop())

            # ---- tail: shared sub-experts (hidden 512 each) + combine ----
            # xTb reuses xtok's buffer; this DMA waits for the last gather
            # read and completes under expert 7's compute.
            xTb = xbig.tile([P, KD, T], BF, name="xTb", tag="xb")
            nc.gpsimd.dma_start(out=xTb[:], in_=xtb[:])

            def emit_shared_l1_unit(s, hcl):
                sv = E + s // 4
                hcg = (s % 4) * HQ + hcl
                w1c = w13.tile([P, KD, P], BF, name="w1c", tag="w1c")
                nc.sync.dma_start(out=w1c[:], in_=w1t[sv, hcg])
                w3c = w13.tile([P, KD, P], BF, name="w3c", tag="w3c")
                nc.gpsimd.dma_start(out=w3c[:], in_=w3t[sv, hcg])
                hT_s = hts[s]
                for ft in range(FT):
                    fsl = slice(ft * 512, (ft + 1) * 512)
                    o1 = ps_o1.tile([P, 512], F32, space="PSUM",
                                    name="o1", tag="o1")
                    for k in range(KD):
                        nc.tensor.matmul(out=o1[:], lhsT=w1c[:, k, :],
                                         rhs=xTb[:, k, fsl],
                                         start=(k == 0), stop=(k == KD - 1))
                    s1 = s1p.tile([P, 512], F32, name="s1", tag="s1")
                    nc.scalar.activation(s1[:], o1[:], AF.Sigmoid,
                                         bias=bsh1[s // 4][:, hcg:hcg + 1])
                    t1 = s1p.tile([P, 512], F32, name="t1", tag="t1")
                    nc.vector.scalar_tensor_tensor(
                        out=t1[:], in0=o1[:],
                        scalar=bsh1[s // 4][:, hcg:hcg + 1], in1=s1[:],
                        op0=OP.add, op1=OP.mult)
                    o3 = ps_o3.tile([P, 512], F32, space="PSUM",
                                    name="o3", tag="o3")
                    for k in range(KD):
                        nc.tensor.matmul(out=o3[:], lhsT=w3c[:, k, :],
                                         rhs=xTb[:, k, fsl],
                                         start=(k == 0), stop=(k == KD - 1))
                    nc.vector.scalar_tensor_tensor(
                        out=hT_s[:, hcl, fsl], in0=o3[:],
                        scalar=bsh3[s // 4][:, hcg:hcg + 1], in1=t1[:],
                        op0=OP.add, op1=OP.mult)

            def emit_combine(m):
                pr = m // 2
                p2s_l = []
                for e in range(E):
                    capm_e = cfg.capms[e]
                    pe2 = gtmp.tile([P, capm_e], BF, name="pe2")
                    nc.vector.tensor_scalar(out=pe2[:], in0=iota_sb[:, :capm_e],
                                            scalar1=posb_all[:, m, e:e + 1],
                                            scalar2=None, op0=OP.is_equal)
                    pew = gtmp.tile([P, capm_e], BF, name="pew")
                    nc.vector.tensor_scalar(out=pew[:], in0=pe2[:],
                                            scalar1=cw[:, m, e:e + 1],
                                            scalar2=None, op0=OP.mult)
                    p2c = ps_tr.tile([P, P], BF, space="PSUM",
                                     name="p2c", tag="tr")
                    nc.tensor.transpose(out=p2c[:capm_e, :], in_=pew[:],
                                        identity=id_sb[:])
                    p2s = combp.tile([capm_e, P], BF, name="p2s")
                    nc.scalar.activation(p2s[:], p2c[:capm_e, :], AF.Copy)
                    p2s_l.append(p2s)
                for dt in range(DT):
                    dsl = slice(dt * 512, (dt + 1) * 512)
                    yp = ps_l2.tile([P, 512], F32, space="PSUM",
                                    name="yp", tag="l2")
                    for e in range(E):
                        nc.tensor.matmul(
                            out=yp[:], lhsT=p2s_l[e][:],
                            rhs=ye_sb[0:cfg.capms[e], e * NP + pr, dsl],
                            start=(e == 0), stop=(e == E - 1))
                    nc.vector.tensor_copy(yacc[:, m, dsl], yp[:])

            def emit_shared_l2(s):
                sv = E + s // 4
                hcg0 = (s % 4) * HQ
                w2q = w2p.tile([P, HQ, D], BF, name="w2qs", tag="w2q")
                nc.scalar.dma_start(out=w2q[:],
                                    in_=w2t[sv][:, hcg0:hcg0 + HQ, :])
                hT_s = hts[s]
                for tt in range(TT):
                    tsl = slice(tt * P, (tt + 1) * P)
                    for dt in range(DT):
                        dsl = slice(dt * 512, (dt + 1) * 512)
                        pl, tg = ((ps_l2, "l2") if (tt * DT + dt) % 2 == 0
                                  else (ps_tr, "tr"))
                        yp2 = pl.tile([P, 512], F32, space="PSUM",
                                      name="yp2", tag=tg)
                        if s == 0:
                            nc.tensor.matmul(out=yp2[:], lhsT=ones_sb[:],
                                             rhs=b2sh_sb[0:1, dsl],
                                             start=True, stop=False)
                        for hcl in range(HQ):
                            nc.tensor.matmul(
                                out=yp2[:], lhsT=hT_s[:, hcl, tsl],
                                rhs=w2q[:, hcl, dsl],
                                start=(s != 0 and hcl == 0),
                                stop=(hcl == HQ - 1))
                        if s < NSH - 1:
                            nc.vector.tensor_add(yacc[:, tt, dsl],
                                                 yacc[:, tt, dsl], yp2[:])
                        else:
                            yt = ytp.tile([P, 512], F32, name="yt")
                            nc.vector.tensor_add(yt[:], yacc[:, tt, dsl],
                                                 yp2[:])
                            nc.sync.dma_start(out=y[:, tt, dsl], in_=yt[:])

            bsh1 = []
            bsh3 = []
            for sv in range(2):
                b1s = b13.tile([P, HCN], F32, name="b1sh", tag="b1sh")
                nc.sync.dma_start(out=b1s[:], in_=b1a[E + sv])
                b3s = b13.tile([P, HCN], F32, name="b3sh", tag="b3sh")
                nc.sync.dma_start(out=b3s[:], in_=b3a[E + sv])
                bsh1.append(b1s)
                bsh3.append(b3s)

            hts = {}
            for s in range(NSH):
                if s < 2:
                    hts[s] = hshp.tile([P, HQ, T], BF, name=f"hTs{s}",
                                       tag="hTs")
            # interleave first two shared sub-experts' L1 with combine
            ci = 0
            for s in range(2):
                for hcl in range(HQ):
                    emit_shared_l1_unit(s, hcl)
                    emit_combine(ci)
                    ci += 1
            # pipeline: L2(s) || L1(s+2)
            for s in range(NSH):
                emit_shared_l2(s)
                if s + 2 < NSH:
                    hts[s + 2] = hshp.tile([P, HQ, T], BF, name=f"hTs{s+2}",
                                           tag="hTs")
                    for hcl in range(HQ):
                        emit_shared_l1_unit(s + 2, hcl)

    nc.compile()
    return nc


# ---------------- host-side packing ----------------

def pack_static(cfg: Cfg, gate_w, gate_b, w1, b1, w2, b2, w3, b3,
                sw1, sb1, sw2, sb2, sw3, sb3):
    D, H, E, NV, n_sh = cfg.D, cfg.H, cfg.E, cfg.NV, cfg.n_sh
    KD, HCN = cfg.KD, cfg.HCN

    w1T = np.transpose(w1, (0, 2, 1))                      # [E, D, H]
    w3T = np.transpose(w3, (0, 2, 1))
    w2T = np.transpose(w2, (0, 2, 1))                      # [E, H, D]
    s1T = sw1.T.reshape(D, n_sh, H).transpose(1, 0, 2)     # [n_sh, D, H]
    s3T = sw3.T.reshape(D, n_sh, H).transpose(1, 0, 2)
    s2T = sw2.T.reshape(n_sh, H, D)                        # [n_sh, H, D]
    w1T_all = np.concatenate([w1T, s1T], 0)                # [NV, D, H]
    w3T_all = np.concatenate([w3T, s3T], 0)
    w2T_all = np.concatenate([w2T, s2T], 0)                # [NV, H, D]

    w1t = np.ascontiguousarray(
        w1T_all.reshape(NV, KD, P, HCN, P).transpose(0, 3, 2, 1, 4)).astype(BF16)
    w3t = np.ascontiguousarray(
        w3T_all.reshape(NV, KD, P, HCN, P).transpose(0, 3, 2, 1, 4)).astype(BF16)
    w2t = np.ascontiguousarray(
        w2T_all.reshape(NV, HCN, P, D).transpose(0, 2, 1, 3)).astype(BF16)

    b1_all = np.concatenate([b1, sb1.reshape(n_sh, H)], 0)  # [NV, H]
    b3_all = np.concatenate([b3, sb3.reshape(n_sh, H)], 0)
    b1a = np.ascontiguousarray(
        b1_all.reshape(NV, HCN, P).transpose(0, 2, 1)).astype(np.float32)
    b3a = np.ascontiguousarray(
        b3_all.reshape(NV, HCN, P).transpose(0, 2, 1)).astype(np.float32)

    b2_all = np.concatenate(
        [b2, sb2[None], np.zeros((n_sh - 1, D), np.float32)], 0)  # [NV, D]
    b2r = b2_all[None].astype(BF16)                         # [1, NV, D]
    # routed b2 in d-partition layout: [P, E*KD], col e*KD+dc = b2[e, dc*128+p]
    b2c = np.ascontiguousarray(
        b2.reshape(E, KD, P).transpose(2, 0, 1).reshape(P, E * KD)
    ).astype(np.float32)

    gwt = np.ascontiguousarray(
        gate_w.T.reshape(KD, P, E).transpose(1, 0, 2)).astype(np.float32)
    gb = gate_b[None].astype(np.float32)
    ones1 = np.ones((1, P), BF16)
    onesc = np.ones((P, 1), BF16)
    lt = np.triu(np.ones((P, P))).astype(BF16)
    ident = np.eye(P).astype(BF16)
    iota = np.tile(np.arange(cfg.capm, dtype=np.float32), (P, 1))

    return dict(w1t=w1t, w3t=w3t, w2t=w2t, b1a=b1a, b3a=b3a, b2r=b2r,
                b2c=b2c, gwt=gwt, gb=gb, ones1=ones1, onesc=onesc,
                lt=lt, ident=ident, iota=iota)


def pack_xtok(cfg: Cfg, x_tokens):
    T, D = x_tokens.shape
    xt = x_tokens.reshape(cfg.TT, P, D).transpose(1, 0, 2)
    return np.ascontiguousarray(xt).astype(BF16)


def pack_xT(cfg: Cfg, x_tokens):
    T, D = x_tokens.shape
    xT = x_tokens.T.reshape(cfg.KD, P, T).transpose(1, 0, 2)
    return np.ascontiguousarray(xT).astype(np.float32)


def unpack_y(cfg: Cfg, y_dev):
    return np.ascontiguousarray(y_dev.transpose(1, 0, 2).reshape(cfg.T, cfg.D))


def balance_tokens(xf, gate_w, gate_b, E=8, margin=2):
    """Assign tokens to 256-token buckets so per-(bucket, expert) routed
    counts are near their per-expert means. Returns (perm, capms):
    bucket-major token order and per-expert slot capacities."""
    N = xf.shape[0]
    NB = N // 256
    logits = xf @ gate_w.T + gate_b
    idx = np.argsort(-logits, axis=1)[:, :2]
    tgt = np.zeros(E)
    for e in range(E):
        tgt[e] = ((idx[:, 0] == e) | (idx[:, 1] == e)).sum() / NB
    tgt = np.maximum(tgt, 1.0)
    cnt = np.zeros((NB, E), np.float64)
    fill = np.zeros(NB, np.int64)
    assign = np.empty(N, np.int32)
    rng = np.random.RandomState(0)
    BIG = 1 << 40
    for t in rng.permutation(N):
        a, b = idx[t]
        s = np.maximum((cnt[:, a] + 1) / tgt[a],
                       (cnt[:, b] + 1) / tgt[b]) * 4096 + fill
        s[fill >= 256] = BIG
        bb = int(np.argmin(s))
        assign[t] = bb
        cnt[bb, a] += 1
        cnt[bb, b] += 1
        fill[bb] += 1
    perm = np.argsort(assign.astype(np.int64) * N + np.arange(N))
    capms = tuple(int(c) + margin for c in cnt.max(0))
    return perm, capms


_CACHE = {}


def _get_nc(cfg: Cfg):
    key = (cfg.D, cfg.H, cfg.E, cfg.n_sh, cfg.T, cfg.capms)
    if key not in _CACHE:
        _CACHE[key] = build_nc_v2(cfg)
    return _CACHE[key]


def plan_cfg(inputs):
    """Balance tokens from the actual routing; returns (cfg, perm)."""
    x = np.asarray(inputs["x"], np.float32)
    B, S, D = x.shape
    xf = x.reshape(-1, D)
    perm, capms = balance_tokens(
        xf, np.asarray(inputs["gate_w"], np.float32),
        np.asarray(inputs["gate_b"], np.float32))
    cfg = Cfg(D=D, T=(B * S) // 8, n_cores=8, capms=capms)
    return cfg, perm


def make_in_maps(cfg: Cfg, inputs, perm):
    static = pack_static(
        cfg,
        np.asarray(inputs["gate_w"], np.float32), np.asarray(inputs["gate_b"], np.float32),
        np.asarray(inputs["w1"], np.float32), np.asarray(inputs["b1"], np.float32),
        np.asarray(inputs["w2"], np.float32), np.asarray(inputs["b2"], np.float32),
        np.asarray(inputs["w3"], np.float32), np.asarray(inputs["b3"], np.float32),
        np.asarray(inputs["sw1"], np.float32), np.asarray(inputs["sb1"], np.float32),
        np.asarray(inputs["sw2"], np.float32), np.asarray(inputs["sb2"], np.float32),
        np.asarray(inputs["sw3"], np.float32), np.asarray(inputs["sb3"], np.float32),
    )
    x = np.asarray(inputs["x"], np.float32)
    B, S, D = x.shape
    xp = x.reshape(-1, D)[perm]
    in_maps = []
    for c in range(cfg.n_cores):
        mm = dict(static)
        xc = xp[c * cfg.T:(c + 1) * cfg.T]
        mm["xT"] = pack_xT(cfg, xc)
        mm["xtok"] = pack_xtok(cfg, xc)
        mm["xtb"] = mm["xT"].astype(BF16)
        in_maps.append(mm)
    return in_maps


def kernel(**inputs) -> np.ndarray:
    x = np.asarray(inputs["x"], np.float32)
    B, S, D = x.shape
    cfg, perm = plan_cfg(inputs)
    nc = _get_nc(cfg)
    in_maps = make_in_maps(cfg, inputs, perm)
    res = run_bass_kernel_spmd(nc, in_maps, list(range(cfg.n_cores)))
    yp = np.concatenate(
        [unpack_y(cfg, res.results[c]["y"]) for c in range(cfg.n_cores)], 0)
    out = np.empty_like(yp)
    out[perm] = yp
    return out.reshape(B, S, D)


# revision 59
# speedup vs baseline: 1.2449x; 1.0793x over previous
"""MoE (8 routed experts, top-2, + shared expert) on 8 NeuronCores.

Data-parallel over tokens (1024/core), weights replicated. The host
load-balances token->bucket assignment (any sharding is allowed) so the
per-(256-token-bucket, expert) routed count is ~uniform (max 72 on this
model's routing), letting the capacity-dispatched kernel run with
capm=74 slots per bucket instead of the binomial-tail 96.

Device kernel (per core):
  1. Gate in fp32 (matches reference routing bit-for-bit for the
     observed >=1.7e-4 top-2/3 logit gaps), renormalized top-2 combine
     weights cw, and bucket-local slot positions via triangular-matmul
     prefix sums.
  2. Routed experts e=0..7: one-hot matmul gather of x into CAPE=296
     slots, SwiGLU L1 (feature-major, free dim = slots), L2 in
     d-partition orientation (out[d, slot], bias via activation), then
     PE transposes to slot-major ye tiles held in SBUF.
  3. Tail: shared expert (8 sub-experts of hidden 512) interleaved with
     the scatter-combine (transposed scaled one-hots x ye), everything
     accumulating into a token-major f32 yacc; last shared sub-expert's
     L2 fuses the final add and streams y out.

Matmuls are bf16 with fp32 accumulation; weight DMA is split across the
SP/Pool/Act queues to avoid head-of-line blocking on one DMA queue.
"""

import numpy as np
import ml_dtypes

import concourse.bacc as bacc
import concourse.bass as bass
import concourse.tile as tile
import concourse.mybir as mybir
from concourse.bass_utils import run_bass_kernel_spmd

BF16 = ml_dtypes.bfloat16
F32 = mybir.dt.float32
BF = mybir.dt.bfloat16
AF = mybir.ActivationFunctionType
OP = mybir.AluOpType

P = 128


class Cfg:
    def __init__(self, D=1024, H=2048, E=8, n_sh=2, T=1024, n_cores=8,
                 capms=(74,) * 8):
        self.D, self.H, self.E, self.n_sh, self.T = D, H, E, n_sh, T
        self.NV = E + n_sh          # packed weight rows (8 routed + 2 shared)
        self.HS = n_sh * H          # shared hidden total (4096)
        self.KD = D // P            # contraction chunks over D
        self.HCN = H // P           # h chunks per packed VE
        self.TT = T // P            # token 128-tiles per core
        self.FT = T // 512          # shared L1 free 512-tiles
        self.DT = D // 512          # 512-wide d tiles
        self.n_cores = n_cores
        self.capms = tuple(capms)   # slots per (256-token bucket, expert)
        self.capm = max(self.capms)  # iota / tile sizing width
        self.NP = self.TT // 2      # buckets per core (pair of tiles)
        self.NSH = 8                # shared sub-experts
        self.HQ = (self.HS // P) // self.NSH  # h-chunks per sub-expert (4)


def build_nc_v2(cfg: Cfg):
    D, H, E, T = cfg.D, cfg.H, cfg.E, cfg.T
    KD, HCN, TT, FT, DT = cfg.KD, cfg.HCN, cfg.TT, cfg.FT, cfg.DT
    capm, NP = cfg.capm, cfg.NP
    NSH, HQ = cfg.NSH, cfg.HQ

    nc = bacc.Bacc("TRN2", target_bir_lowering=False)

    xT = nc.dram_tensor("xT", [P, KD, T], F32, kind="ExternalInput")
    xtok = nc.dram_tensor("xtok", [P, TT, D], BF, kind="ExternalInput")
    xtb = nc.dram_tensor("xtb", [P, KD, T], BF, kind="ExternalInput")
    w1t = nc.dram_tensor("w1t", [cfg.NV, HCN, P, KD, P], BF, kind="ExternalInput")
    w3t = nc.dram_tensor("w3t", [cfg.NV, HCN, P, KD, P], BF, kind="ExternalInput")
    w2t = nc.dram_tensor("w2t", [cfg.NV, P, HCN, D], BF, kind="ExternalInput")
    b1a = nc.dram_tensor("b1a", [cfg.NV, P, HCN], F32, kind="ExternalInput")
    b3a = nc.dram_tensor("b3a", [cfg.NV, P, HCN], F32, kind="ExternalInput")
    b2r = nc.dram_tensor("b2r", [1, cfg.NV, D], BF, kind="ExternalInput")
    b2c = nc.dram_tensor("b2c", [P, E * KD], F32, kind="ExternalInput")
    gwt = nc.dram_tensor("gwt", [P, KD, E], F32, kind="ExternalInput")
    gb = nc.dram_tensor("gb", [1, E], F32, kind="ExternalInput")
    ones1 = nc.dram_tensor("ones1", [1, P], BF, kind="ExternalInput")
    onesc = nc.dram_tensor("onesc", [P, 1], BF, kind="ExternalInput")
    lt = nc.dram_tensor("lt", [P, P], BF, kind="ExternalInput")
    ident = nc.dram_tensor("ident", [P, P], BF, kind="ExternalInput")
    iota = nc.dram_tensor("iota", [P, capm], F32, kind="ExternalInput")
    y = nc.dram_tensor("y", [P, TT, D], BF, kind="ExternalOutput")

    OOB = 3.0e6

    from contextlib import ExitStack
    with tile.TileContext(nc) as tc:
        with ExitStack() as stack:
            pool_specs = dict(
                const1=dict(bufs=1), xbig=dict(bufs=1),
                gchunk=dict(bufs=2), gtmp=dict(bufs=4),
                w13=dict(bufs=3), w2p=dict(bufs=2), b13=dict(bufs=2),
                xep=dict(bufs=1), hrout=dict(bufs=1), hshp=dict(bufs=2),
                yeBp=dict(bufs=1), pep=dict(bufs=2), s1p=dict(bufs=2),
                combp=dict(bufs=8), ytp=dict(bufs=2),
                ps_o1=dict(bufs=2, space="PSUM"),
                ps_o3=dict(bufs=2, space="PSUM"),
                ps_l2=dict(bufs=2, space="PSUM"),
                ps_tr=dict(bufs=2, space="PSUM"),
            )
            pools = {n: stack.enter_context(tc.tile_pool(name=n, **kw))
                     for n, kw in pool_specs.items()}
            (const1, xbig, gchunk, gtmp, w13, w2p, b13, xep, hrout,
             hshp, yeBp, pep, s1p, combp, ytp, ps_o1, ps_o3, ps_l2,
             ps_tr) = (
                pools[n] for n in (
                    "const1", "xbig", "gchunk", "gtmp", "w13", "w2p", "b13",
                    "xep", "hrout", "hshp", "yeBp", "pep", "s1p",
                    "combp", "ytp", "ps_o1", "ps_o3", "ps_l2", "ps_tr"))
            # ---- resident state ----
            # xtok_sb and xTb share one 2MB buffer (tag xb): xtok is dead
            # after the last gather; xTb is DMA'd into the same space then.
            xtok_sb = xbig.tile([P, TT, D], BF, name="xtok_sb", tag="xb")
            yacc = const1.tile([P, TT, D], F32)
            cw = const1.tile([P, TT, E], F32)
            posb_all = const1.tile([P, TT, E], F32)
            ye_sb = const1.tile([P, E * NP, D], BF)
            gwt_sb = const1.tile([P, KD, E], F32)
            gb_sb = const1.tile([1, E], F32)
            ones_sb = const1.tile([1, P], BF)
            onesc_sb = const1.tile([P, 1], BF)
            lt_sb = const1.tile([P, P], BF)
            id_sb = const1.tile([P, P], BF)
            iota_sb = const1.tile([P, capm], F32)
            b2c_sb = const1.tile([P, E * KD], F32)
            b2sh_sb = const1.tile([1, D], BF)
            zerob = const1.tile([P, 1], F32)
            onesf = const1.tile([1, P], F32)

            # first two gate tiles + gate weights lead the DMA queues so
            # the gate starts without sitting behind the bulk prologue
            nc.sync.dma_start(out=gwt_sb[:], in_=gwt[:])
            nc.sync.dma_start(out=gb_sb[:], in_=gb[:])
            xc_pre = {}
            for m in range(2):
                xc = gchunk.tile([P, KD, P], F32, name="xchunk")
                nc.sync.dma_start(out=xc[:], in_=xT[:, :, m * P:(m + 1) * P])
                xc_pre[m] = xc
            # secondary consts flow on the Act queue in parallel so the
            # gate's per-tile xchunk stream on sync isn't delayed
            nc.scalar.dma_start(out=ones_sb[:], in_=ones1[:])
            nc.scalar.dma_start(out=onesc_sb[:], in_=onesc[:])
            nc.scalar.dma_start(out=lt_sb[:], in_=lt[:])
            nc.scalar.dma_start(out=id_sb[:], in_=ident[:])
            nc.scalar.dma_start(out=iota_sb[:], in_=iota[:])
            nc.scalar.dma_start(out=b2c_sb[:], in_=b2c[:])
            nc.scalar.dma_start(out=b2sh_sb[:], in_=b2r[0:1, E, :])
            nc.vector.memset(zerob[:], 0.0)
            nc.vector.memset(onesf[:], 1.0)

            # prefetch expert 0's first L1 weight chunks
            pre_w = {}
            for hc in range(3):
                w1c = w13.tile([P, KD, P], BF, name="w1c", tag="w1c")
                nc.sync.dma_start(out=w1c[:], in_=w1t[0, hc])
                w3c = w13.tile([P, KD, P], BF, name="w3c", tag="w3c")
                nc.gpsimd.dma_start(out=w3c[:], in_=w3t[0, hc])
                pre_w[hc] = (w1c, w3c)
            # xtok hands off to the DMA engines late (pool-queue tail) so
            # its 2MB transfer neither starves the small gate-const loads
            # nor blocks the per-tile gate xchunk stream on sync; it is
            # first needed by expert 0's gather, well after the gate.
            nc.gpsimd.dma_start(out=xtok_sb[:], in_=xtok[:])

            # ---- gate + bucket positions, per 128-token tile ----
            cntb = None
            for m in range(TT):
                if m in xc_pre:
                    xchunk = xc_pre[m]
                else:
                    xchunk = gchunk.tile([P, KD, P], F32)
                    nc.sync.dma_start(out=xchunk[:],
                                      in_=xT[:, :, m * P:(m + 1) * P])

                pg = ps_l2.tile([P, E], F32, space="PSUM", name="pg", tag="l2")
                for k in range(KD):
                    nc.tensor.matmul(out=pg[:], lhsT=xchunk[:, k, :],
                                     rhs=gwt_sb[:, k, :],
                                     start=(k == 0), stop=False)
                nc.tensor.matmul(out=pg[:], lhsT=onesf[:], rhs=gb_sb[:],
                                 start=False, stop=True)

                lg = gtmp.tile([P, E], F32)
                nc.scalar.activation(lg[:], pg[:], AF.Copy)
                m8 = gtmp.tile([P, 8], F32)
                nc.vector.max(m8[:], lg[:])
                ex = gtmp.tile([P, E], F32)
                nc.vector.tensor_scalar(out=ex[:], in0=lg[:],
                                        scalar1=m8[:, 0:1], scalar2=None,
                                        op0=OP.subtract)
                nc.scalar.activation(ex[:], ex[:], AF.Exp, bias=zerob[:])
                mask = gtmp.tile([P, E], F32)
                nc.vector.tensor_scalar(out=mask[:], in0=lg[:],
                                        scalar1=m8[:, 1:2], scalar2=None,
                                        op0=OP.is_ge)
                e2 = gtmp.tile([P, 1], F32)
                nc.vector.tensor_tensor(out=e2[:], in0=m8[:, 1:2],
                                        in1=m8[:, 0:1], op=OP.subtract)
                nc.scalar.activation(e2[:], e2[:], AF.Exp, bias=zerob[:])
                den = gtmp.tile([P, 1], F32)
                nc.vector.tensor_scalar(out=den[:], in0=e2[:], scalar1=1.0,
                                        scalar2=None, op0=OP.add)
                rec = gtmp.tile([P, 1], F32)
                nc.vector.reciprocal(rec[:], den[:])
                cwm = gtmp.tile([P, E], F32)
                nc.vector.tensor_mul(cwm[:], ex[:], mask[:])
                nc.vector.tensor_scalar(out=cw[:, m, :], in0=cwm[:],
                                        scalar1=rec[:, 0:1], scalar2=None,
                                        op0=OP.mult)

                # bucket-local slot: pair prefix(mask) - mask; OOB unrouted
                maskb = gtmp.tile([P, E], BF)
                nc.vector.tensor_copy(maskb[:], mask[:])
                pp = ps_tr.tile([P, E], F32, space="PSUM", name="pp", tag="tr")
                if m % 2 == 0:
                    nc.tensor.matmul(out=pp[:], lhsT=lt_sb[:],
                                     rhs=maskb[:], start=True, stop=True)
                    cnt_ps = ps_tr.tile([1, E], F32, space="PSUM",
                                        name="cntp", tag="tr")
                    nc.tensor.matmul(out=cnt_ps[:], lhsT=onesc_sb[:],
                                     rhs=maskb[:], start=True, stop=True)
                    cntb = gtmp.tile([1, E], BF, name="cntb")
                    nc.scalar.activation(cntb[:], cnt_ps[:], AF.Copy)
                else:
                    nc.tensor.matmul(out=pp[:], lhsT=lt_sb[:],
                                     rhs=maskb[:], start=True, stop=False)
                    nc.tensor.matmul(out=pp[:], lhsT=ones_sb[:],
                                     rhs=cntb[:], start=False, stop=True)
                t1m = gtmp.tile([P, E], F32)
                nc.vector.scalar_tensor_tensor(out=t1m[:], in0=mask[:],
                                               scalar=-1.0, in1=pp[:],
                                               op0=OP.mult, op1=OP.add)
                notm = gtmp.tile([P, E], F32)
                nc.vector.tensor_scalar(out=notm[:], in0=mask[:],
                                        scalar1=-1.0, scalar2=1.0,
                                        op0=OP.mult, op1=OP.add)
                nc.vector.scalar_tensor_tensor(out=posb_all[:, m, :],
                                               in0=notm[:], scalar=OOB,
                                               in1=t1m[:],
                                               op0=OP.mult, op1=OP.add)

            # ---- routed experts over dispatched slots ----
            pre_sh = {}
            for e in range(E):
                capm_e = cfg.capms[e]
                CAPE = NP * capm_e
                b1sb = b13.tile([P, HCN], F32, name="b1sb", tag="b1")
                nc.sync.dma_start(out=b1sb[:], in_=b1a[e])
                b3sb = b13.tile([P, HCN], F32, name="b3sb", tag="b3")
                nc.sync.dma_start(out=b3sb[:], in_=b3a[e])

                # prefetch the first two w2 quarters; they land during L1
                w2qs_pre = []
                for dq in range(2):
                    w2q0 = w2p.tile([P, HCN, 256], BF, name="w2q", tag="w2q")
                    nc.sync.dma_start(
                        out=w2q0[:],
                        in_=w2t[e][:, :, dq * 256:(dq + 1) * 256])
                    w2qs_pre.append(w2q0)

                # one-hot dispatch tiles for all 8 token tiles
                pe_all = pep.tile([P, TT, capm_e], BF, name="pe_all", tag="pe")
                for m in range(TT):
                    nc.vector.tensor_scalar(
                        out=pe_all[:, m, :], in0=iota_sb[:, :capm_e],
                        scalar1=posb_all[:, m, e:e + 1],
                        scalar2=None, op0=OP.is_equal)

                # matmul gather: xeT[k][d, slot] = sum_m x_m^T @ Pe_m
                xeT = xep.tile([P, KD, CAPE], BF, name="xeT", tag="xeT")
                for k in range(KD):
                    gxp, gxt = (ps_l2, "l2") if k % 2 == 0 else (ps_tr, "tr")
                    gx = gxp.tile([P, CAPE], F32, space="PSUM",
                                  name="gx", tag=gxt)
                    for pr in range(NP):
                        for h in range(2):
                            m = 2 * pr + h
                            nc.tensor.matmul(
                                out=gx[:, pr * capm_e:(pr + 1) * capm_e],
                                lhsT=xtok_sb[:, m, k * P:(k + 1) * P],
                                rhs=pe_all[:, m, :],
                                start=(h == 0), stop=(h == 1))
                    nc.vector.tensor_copy(xeT[:, k, :], gx[:])

                if e == E - 1:
                    # last xtok reader just emitted: reload the shared
                    # buffer with d-major x for the tail's shared expert;
                    # the transfer hides under expert 7's L1/L2.
                    xTb = xbig.tile([P, KD, T], BF, name="xTb", tag="xb")
                    nc.scalar.dma_start(out=xTb[:], in_=xtb[:])

                # L1: hT[h, slot] = silu(W1 xe + b1) * (W3 xe + b3)
                hT = hrout.tile([P, HCN, CAPE], BF, name="hT", tag="hT")
                for hc in range(HCN):
                    if e == 0 and hc in pre_w:
                        w1c, w3c = pre_w[hc]
                    else:
                        w1c = w13.tile([P, KD, P], BF, name="w1c", tag="w1c")
                        nc.sync.dma_start(out=w1c[:], in_=w1t[e, hc])
                        w3c = w13.tile([P, KD, P], BF, name="w3c", tag="w3c")
                        nc.gpsimd.dma_start(out=w3c[:], in_=w3t[e, hc])
                    o1 = ps_o1.tile([P, CAPE], F32, space="PSUM",
                                    name="o1", tag="o1")
                    for k in range(KD):
                        nc.tensor.matmul(out=o1[:], lhsT=w1c[:, k, :],
                                         rhs=xeT[:, k, :],
                                         start=(k == 0), stop=(k == KD - 1))
                    s1 = s1p.tile([P, CAPE], F32, name="s1", tag="s1")
                    nc.scalar.activation(s1[:], o1[:], AF.Sigmoid,
                                         bias=b1sb[:, hc:hc + 1])
                    t1 = s1p.tile([P, CAPE], F32, name="t1", tag="t1")
                    nc.vector.scalar_tensor_tensor(
                        out=t1[:], in0=o1[:],
                        scalar=b1sb[:, hc:hc + 1], in1=s1[:],
                        op0=OP.add, op1=OP.mult)
                    o3 = ps_o3.tile([P, CAPE], F32, space="PSUM",
                                    name="o3", tag="o3")
                    for k in range(KD):
                        nc.tensor.matmul(out=o3[:], lhsT=w3c[:, k, :],
                                         rhs=xeT[:, k, :],
                                         start=(k == 0), stop=(k == KD - 1))
                    nc.vector.scalar_tensor_tensor(
                        out=hT[:, hc, :], in0=o3[:],
                        scalar=b3sb[:, hc:hc + 1], in1=t1[:],
                        op0=OP.add, op1=OP.mult)

                # L2 (d-partition orientation) + bias, then transpose to
                # slot-major ye tiles; transposes staggered one dc behind
                # the chains so their yeB reads never stall the PE.
                yeB = yeBp.tile([P, KD, CAPE], BF, name="yeB", tag="yeB")
                pend = []

                def emit_transp(dc, e=e, yeB=yeB, capm_e=capm_e):
                    for pr in range(NP):
                        p2t = ps_tr.tile([P, P], BF, space="PSUM",
                                         name="p2t", tag="tr")
                        nc.tensor.transpose(
                            out=p2t[:capm_e, :],
                            in_=yeB[:, dc, pr * capm_e:(pr + 1) * capm_e],
                            identity=id_sb[:])
                        nc.vector.tensor_copy(
                            ye_sb[0:capm_e, e * NP + pr, dc * P:(dc + 1) * P],
                            p2t[:capm_e, :])

                for dq in range(4):
                    w2q = w2qs_pre[dq]
                    for dc2 in range(2):
                        dc = dq * 2 + dc2
                        pl2 = ps_l2.tile([P, CAPE], F32, space="PSUM",
                                         name="pl2", tag="l2")
                        for hc in range(HCN):
                            nc.tensor.matmul(
                                out=pl2[:],
                                lhsT=w2q[:, hc, dc2 * P:(dc2 + 1) * P],
                                rhs=hT[:, hc, :],
                                start=(hc == 0), stop=(hc == HCN - 1))
                        nc.vector.tensor_scalar(
                            out=yeB[:, dc, :], in0=pl2[:],
                            scalar1=b2c_sb[:, e * KD + dc:e * KD + dc + 1],
                            scalar2=None, op0=OP.add)
                        if pend:
                            emit_transp(pend.pop())
                        pend.append(dc)
                    if dq + 2 < 4:
                        # refill two quarters ahead (this quarter's chains
                        # just freed the buffer, so the queue-head wait is
                        # short; only next-expert w1c prefetches sit behind)
                        w2n = w2p.tile([P, HCN, 256], BF, name="w2q",
                                       tag="w2q")
                        nc.sync.dma_start(
                            out=w2n[:],
                            in_=w2t[e][:, :, (dq + 2) * 256:(dq + 3) * 256])
                        w2qs_pre.append(w2n)
                while pend:
                    emit_transp(pend.pop())

            # ---- tail: shared sub-experts (hidden 512 each) + combine ----
            def emit_shared_l1_unit(s, hcl):
                sv = E + s // 4
                hcg = (s % 4) * HQ + hcl
                if (s, hcl) in pre_sh:
                    w1c, w3c = pre_sh[(s, hcl)]
                else:
                    w1c = w13.tile([P, KD, P], BF, name="w1c", tag="w1c")
                    nc.sync.dma_start(out=w1c[:], in_=w1t[sv, hcg])
                    w3c = w13.tile([P, KD, P], BF, name="w3c", tag="w3c")
                    nc.gpsimd.dma_start(out=w3c[:], in_=w3t[sv, hcg])
                hT_s = hts[s]
                for ft in range(FT):
                    fsl = slice(ft * 512, (ft + 1) * 512)
                    o1 = ps_o1.tile([P, 512], F32, space="PSUM",
                                    name="o1", tag="o1")
                    for k in range(KD):
                        nc.tensor.matmul(out=o1[:], lhsT=w1c[:, k, :],
                                         rhs=xTb[:, k, fsl],
                                         start=(k == 0), stop=(k == KD - 1))
                    s1 = s1p.tile([P, 512], F32, name="s1", tag="s1")
                    nc.scalar.activation(s1[:], o1[:], AF.Sigmoid,
                                         bias=bsh1[s // 4][:, hcg:hcg + 1])
                    t1 = s1p.tile([P, 512], F32, name="t1", tag="t1")
                    nc.vector.scalar_tensor_tensor(
                        out=t1[:], in0=o1[:],
                        scalar=bsh1[s // 4][:, hcg:hcg + 1], in1=s1[:],
                        op0=OP.add, op1=OP.mult)
                    o3 = ps_o3.tile([P, 512], F32, space="PSUM",
                                    name="o3", tag="o3")
                    for k in range(KD):
                        nc.tensor.matmul(out=o3[:], lhsT=w3c[:, k, :],
                                         rhs=xTb[:, k, fsl],
                                         start=(k == 0), stop=(k == KD - 1))
                    nc.vector.scalar_tensor_tensor(
                        out=hT_s[:, hcl, fsl], in0=o3[:],
                        scalar=bsh3[s // 4][:, hcg:hcg + 1], in1=t1[:],
                        op0=OP.add, op1=OP.mult)

            def emit_combine(m):
                pr = m // 2
                p2s_l = []
                for e in range(E):
                    capm_e = cfg.capms[e]
                    pe2 = gtmp.tile([P, capm_e], BF, name="pe2")
                    nc.vector.tensor_scalar(out=pe2[:], in0=iota_sb[:, :capm_e],
                                            scalar1=posb_all[:, m, e:e + 1],
                                            scalar2=None, op0=OP.is_equal)
                    pew = gtmp.tile([P, capm_e], BF, name="pew")
                    nc.vector.tensor_scalar(out=pew[:], in0=pe2[:],
                                            scalar1=cw[:, m, e:e + 1],
                                            scalar2=None, op0=OP.mult)
                    p2c = ps_tr.tile([P, P], BF, space="PSUM",
                                     name="p2c", tag="tr")
                    nc.tensor.transpose(out=p2c[:capm_e, :], in_=pew[:],
                                        identity=id_sb[:])
                    p2s = combp.tile([capm_e, P], BF, name="p2s")
                    nc.scalar.activation(p2s[:], p2c[:capm_e, :], AF.Copy)
                    p2s_l.append(p2s)
                for dt in range(DT):
                    dsl = slice(dt * 512, (dt + 1) * 512)
                    yp = ps_l2.tile([P, 512], F32, space="PSUM",
                                    name="yp", tag="l2")
                    for e in range(E):
                        nc.tensor.matmul(
                            out=yp[:], lhsT=p2s_l[e][:],
                            rhs=ye_sb[0:cfg.capms[e], e * NP + pr, dsl],
                            start=(e == 0), stop=(e == E - 1))
                    nc.vector.tensor_copy(yacc[:, m, dsl], yp[:])

            def emit_shared_l2(s):
                sv = E + s // 4
                hcg0 = (s % 4) * HQ
                w2q = w2p.tile([P, HQ, D], BF, name="w2qs", tag="w2q")
                nc.scalar.dma_start(out=w2q[:],
                                    in_=w2t[sv][:, hcg0:hcg0 + HQ, :])
                hT_s = hts[s]
                for tt in range(TT):
                    tsl = slice(tt * P, (tt + 1) * P)
                    for dt in range(DT):
                        dsl = slice(dt * 512, (dt + 1) * 512)
                        if s >= NSH - 2:
                            # L1 is done by now: o1/o3 banks are free, use a
                            # deeper 3-pool rotation so chains never wait on
                            # the yacc-add evictions
                            pl, tg = [(ps_l2, "l2"), (ps_tr, "tr"),
                                      (ps_o1, "o1")][(tt * DT + dt) % 3]
                        else:
                            pl, tg = ((ps_l2, "l2") if (tt * DT + dt) % 2 == 0
                                      else (ps_tr, "tr"))
                        yp2 = pl.tile([P, 512], F32, space="PSUM",
                                      name="yp2", tag=tg)
                        if s == 0:
                            nc.tensor.matmul(out=yp2[:], lhsT=ones_sb[:],
                                             rhs=b2sh_sb[0:1, dsl],
                                             start=True, stop=False)
                        for hcl in range(HQ):
                            nc.tensor.matmul(
                                out=yp2[:], lhsT=hT_s[:, hcl, tsl],
                                rhs=w2q[:, hcl, dsl],
                                start=(s != 0 and hcl == 0),
                                stop=(hcl == HQ - 1))
                        if s < NSH - 1:
                            nc.vector.tensor_add(yacc[:, tt, dsl],
                                                 yacc[:, tt, dsl], yp2[:])
                        else:
                            yt = ytp.tile([P, 512], BF, name="yt", bufs=4)
                            nc.vector.tensor_add(yt[:], yacc[:, tt, dsl],
                                                 yp2[:])
                            nc.sync.dma_start(out=y[:, tt, dsl], in_=yt[:])

            bsh1 = []
            bsh3 = []
            for sv in range(2):
                b1s = b13.tile([P, HCN], F32, name="b1sh", tag="b1sh")
                nc.sync.dma_start(out=b1s[:], in_=b1a[E + sv])
                b3s = b13.tile([P, HCN], F32, name="b3sh", tag="b3sh")
                nc.sync.dma_start(out=b3s[:], in_=b3a[E + sv])
                bsh1.append(b1s)
                bsh3.append(b3s)

            hts = {}
            for s in range(NSH):
                if s < 2:
                    hts[s] = hshp.tile([P, HQ, T], BF, name=f"hTs{s}",
                                       tag="hTs")
            # interleave first two shared sub-experts' L1 with combine;
            # combine leads: its inputs (ye, cw, pos) are ready at routed
            # end, covering the xTb/w1c arrival for the shared L1
            ci = 0
            for s in range(2):
                for hcl in range(HQ):
                    emit_combine(ci)
                    ci += 1
                    emit_shared_l1_unit(s, hcl)
            # pipeline: L2(s) || L1(s+2)
            for s in range(NSH):
                emit_shared_l2(s)
                if s + 2 < NSH:
                    hts[s + 2] = hshp.tile([P, HQ, T], BF, name=f"hTs{s+2}",
                                           tag="hTs")
                    for hcl in range(HQ):
                        emit_shared_l1_unit(s + 2, hcl)

    nc.compile()
    return nc


# ---------------- host-side packing ----------------

def pack_static(cfg: Cfg, gate_w, gate_b, w1, b1, w2, b2, w3, b3,
                sw1, sb1, sw2, sb2, sw3, sb3):
    D, H, E, NV, n_sh = cfg.D, cfg.H, cfg.E, cfg.NV, cfg.n_sh
    KD, HCN = cfg.KD, cfg.HCN

    w1T = np.transpose(w1, (0, 2, 1))                      # [E, D, H]
    w3T = np.transpose(w3, (0, 2, 1))
    w2T = np.transpose(w2, (0, 2, 1))                      # [E, H, D]
    s1T = sw1.T.reshape(D, n_sh, H).transpose(1, 0, 2)     # [n_sh, D, H]
    s3T = sw3.T.reshape(D, n_sh, H).transpose(1, 0, 2)
    s2T = sw2.T.reshape(n_sh, H, D)                        # [n_sh, H, D]
    w1T_all = np.concatenate([w1T, s1T], 0)                # [NV, D, H]
    w3T_all = np.concatenate([w3T, s3T], 0)
    w2T_all = np.concatenate([w2T, s2T], 0)                # [NV, H, D]

    w1t = np.ascontiguousarray(
        w1T_all.reshape(NV, KD, P, HCN, P).transpose(0, 3, 2, 1, 4)).astype(BF16)
    w3t = np.ascontiguousarray(
        w3T_all.reshape(NV, KD, P, HCN, P).transpose(0, 3, 2, 1, 4)).astype(BF16)
    w2t = np.ascontiguousarray(
        w2T_all.reshape(NV, HCN, P, D).transpose(0, 2, 1, 3)).astype(BF16)

    b1_all = np.concatenate([b1, sb1.reshape(n_sh, H)], 0)  # [NV, H]
    b3_all = np.concatenate([b3, sb3.reshape(n_sh, H)], 0)
    b1a = np.ascontiguousarray(
        b1_all.reshape(NV, HCN, P).transpose(0, 2, 1)).astype(np.float32)
    b3a = np.ascontiguousarray(
        b3_all.reshape(NV, HCN, P).transpose(0, 2, 1)).astype(np.float32)

    b2_all = np.concatenate(
        [b2, sb2[None], np.zeros((n_sh - 1, D), np.float32)], 0)  # [NV, D]
    b2r = b2_all[None].astype(BF16)                         # [1, NV, D]
    # routed b2 in d-partition layout: [P, E*KD], col e*KD+dc = b2[e, dc*128+p]
    b2c = np.ascontiguousarray(
        b2.reshape(E, KD, P).transpose(2, 0, 1).reshape(P, E * KD)
    ).astype(np.float32)

    gwt = np.ascontiguousarray(
        gate_w.T.reshape(KD, P, E).transpose(1, 0, 2)).astype(np.float32)
    gb = gate_b[None].astype(np.float32)
    ones1 = np.ones((1, P), BF16)
    onesc = np.ones((P, 1), BF16)
    lt = np.triu(np.ones((P, P))).astype(BF16)
    ident = np.eye(P).astype(BF16)
    iota = np.tile(np.arange(cfg.capm, dtype=np.float32), (P, 1))

    return dict(w1t=w1t, w3t=w3t, w2t=w2t, b1a=b1a, b3a=b3a, b2r=b2r,
                b2c=b2c, gwt=gwt, gb=gb, ones1=ones1, onesc=onesc,
                lt=lt, ident=ident, iota=iota)


def pack_xtok(cfg: Cfg, x_tokens):
    T, D = x_tokens.shape
    xt = x_tokens.reshape(cfg.TT, P, D).transpose(1, 0, 2)
    return np.ascontiguousarray(xt).astype(BF16)


def pack_xT(cfg: Cfg, x_tokens):
    T, D = x_tokens.shape
    xT = x_tokens.T.reshape(cfg.KD, P, T).transpose(1, 0, 2)
    return np.ascontiguousarray(xT).astype(np.float32)


def unpack_y(cfg: Cfg, y_dev):
    return np.ascontiguousarray(
        y_dev.transpose(1, 0, 2).reshape(cfg.T, cfg.D)).astype(np.float32)


def balance_tokens(xf, gate_w, gate_b, E=8, margin=2):
    """Assign tokens to 256-token buckets so per-(bucket, expert) routed
    counts are near their per-expert means. Returns (perm, capms):
    bucket-major token order and per-expert slot capacities."""
    N = xf.shape[0]
    NB = N // 256
    logits = xf @ gate_w.T + gate_b
    idx = np.argsort(-logits, axis=1)[:, :2]
    tgt = np.zeros(E)
    for e in range(E):
        tgt[e] = ((idx[:, 0] == e) | (idx[:, 1] == e)).sum() / NB
    tgt = np.maximum(tgt, 1.0)
    cnt = np.zeros((NB, E), np.float64)
    fill = np.zeros(NB, np.int64)
    assign = np.empty(N, np.int32)
    rng = np.random.RandomState(0)
    BIG = 1 << 40
    for t in rng.permutation(N):
        a, b = idx[t]
        s = np.maximum((cnt[:, a] + 1) / tgt[a],
                       (cnt[:, b] + 1) / tgt[b]) * 4096 + fill
        s[fill >= 256] = BIG
        bb = int(np.argmin(s))
        assign[t] = bb
        cnt[bb, a] += 1
        cnt[bb, b] += 1
        fill[bb] += 1
    perm = np.argsort(assign.astype(np.int64) * N + np.arange(N))
    capms = tuple(int(c) + margin for c in cnt.max(0))
    return perm, capms


_CACHE = {}


def _get_nc(cfg: Cfg):
    key = (cfg.D, cfg.H, cfg.E, cfg.n_sh, cfg.T, cfg.capms)
    if key not in _CACHE:
        _CACHE[key] = build_nc_v2(cfg)
    return _CACHE[key]


def plan_cfg(inputs):
    """Balance tokens from the actual routing; returns (cfg, perm)."""
    x = np.asarray(inputs["x"], np.float32)
    B, S, D = x.shape
    xf = x.reshape(-1, D)
    perm, capms = balance_tokens(
        xf, np.asarray(inputs["gate_w"], np.float32),
        np.asarray(inputs["gate_b"], np.float32))
    cfg = Cfg(D=D, T=(B * S) // 8, n_cores=8, capms=capms)
    return cfg, perm


def make_in_maps(cfg: Cfg, inputs, perm):
    static = pack_static(
        cfg,
        np.asarray(inputs["gate_w"], np.float32), np.asarray(inputs["gate_b"], np.float32),
        np.asarray(inputs["w1"], np.float32), np.asarray(inputs["b1"], np.float32),
        np.asarray(inputs["w2"], np.float32), np.asarray(inputs["b2"], np.float32),
        np.asarray(inputs["w3"], np.float32), np.asarray(inputs["b3"], np.float32),
        np.asarray(inputs["sw1"], np.float32), np.asarray(inputs["sb1"], np.float32),
        np.asarray(inputs["sw2"], np.float32), np.asarray(inputs["sb2"], np.float32),
        np.asarray(inputs["sw3"], np.float32), np.asarray(inputs["sb3"], np.float32),
    )
    x = np.asarray(inputs["x"], np.float32)
    B, S, D = x.shape
    xp = x.reshape(-1, D)[perm]
    in_maps = []
    for c in range(cfg.n_cores):
        mm = dict(static)
        xc = xp[c * cfg.T:(c + 1) * cfg.T]
        mm["xT"] = pack_xT(cfg, xc)
        mm["xtok"] = pack_xtok(cfg, xc)
        mm["xtb"] = mm["xT"].astype(BF16)
        in_maps.append(mm)
    return in_maps


def kernel(**inputs) -> np.ndarray:
    x = np.asarray(inputs["x"], np.float32)
    B, S, D = x.shape
    cfg, perm = plan_cfg(inputs)
    nc = _get_nc(cfg)
    in_maps = make_in_maps(cfg, inputs, perm)
    res = run_bass_kernel_spmd(nc, in_maps, list(range(cfg.n_cores)))
    yp = np.concatenate(
        [unpack_y(cfg, res.results[c]["y"]) for c in range(cfg.n_cores)], 0)
    out = np.empty_like(yp)
    out[perm] = yp
    return out.reshape(B, S, D)


# revision 61
# speedup vs baseline: 1.2451x; 1.0001x over previous
"""MoE (8 routed experts, top-2, + shared expert) on 8 NeuronCores.

Data-parallel over tokens (1024/core), weights replicated. The host
load-balances the token->bucket assignment (any sharding is allowed) so
each (256-token bucket, expert) routed count sits at its per-expert
mean, letting the capacity-dispatched kernel run with per-expert
capacities capm_e = max bucket count + 2 (60..74 on this routing)
instead of the binomial-tail uniform 96.

Device kernel (per core):
  1. Gate in fp32 (matches reference routing bit-for-bit for the
     observed >=1.7e-4 top-2/3 logit gaps), renormalized top-2 combine
     weights cw, and bucket-local slot positions via triangular-matmul
     prefix sums.
  2. Routed experts e=0..7: one-hot matmul gather of x into CAPE=296
     slots, SwiGLU L1 (feature-major, free dim = slots), L2 in
     d-partition orientation (out[d, slot], bias via activation), then
     PE transposes to slot-major ye tiles held in SBUF.
  3. Tail: shared expert (8 sub-experts of hidden 512) interleaved with
     the scatter-combine (transposed scaled one-hots x ye), everything
     accumulating into a token-major f32 yacc; last shared sub-expert's
     L2 fuses the final add and streams y out.

Matmuls are bf16 with fp32 accumulation; weight DMA is split across the
SP/Pool/Act queues to avoid head-of-line blocking on one DMA queue.
"""

import numpy as np
import ml_dtypes

import concourse.bacc as bacc
import concourse.bass as bass
import concourse.tile as tile
import concourse.mybir as mybir
from concourse.bass_utils import run_bass_kernel_spmd

BF16 = ml_dtypes.bfloat16
F32 = mybir.dt.float32
BF = mybir.dt.bfloat16
AF = mybir.ActivationFunctionType
OP = mybir.AluOpType

P = 128


class Cfg:
    def __init__(self, D=1024, H=2048, E=8, n_sh=2, T=1024, n_cores=8,
                 capms=(74,) * 8):
        self.D, self.H, self.E, self.n_sh, self.T = D, H, E, n_sh, T
        self.NV = E + n_sh          # packed weight rows (8 routed + 2 shared)
        self.HS = n_sh * H          # shared hidden total (4096)
        self.KD = D // P            # contraction chunks over D
        self.HCN = H // P           # h chunks per packed VE
        self.TT = T // P            # token 128-tiles per core
        self.FT = T // 512          # shared L1 free 512-tiles
        self.DT = D // 512          # 512-wide d tiles
        self.n_cores = n_cores
        self.capms = tuple(capms)   # slots per (256-token bucket, expert)
        self.capm = max(self.capms)  # iota / tile sizing width
        self.NP = self.TT // 2      # buckets per core (pair of tiles)
        self.NSH = 8                # shared sub-experts
        self.HQ = (self.HS // P) // self.NSH  # h-chunks per sub-expert (4)


def build_nc_v2(cfg: Cfg):
    D, H, E, T = cfg.D, cfg.H, cfg.E, cfg.T
    KD, HCN, TT, FT, DT = cfg.KD, cfg.HCN, cfg.TT, cfg.FT, cfg.DT
    capm, NP = cfg.capm, cfg.NP
    NSH, HQ = cfg.NSH, cfg.HQ

    nc = bacc.Bacc("TRN2", target_bir_lowering=False)

    xT = nc.dram_tensor("xT", [P, KD, T], F32, kind="ExternalInput")
    xtok = nc.dram_tensor("xtok", [P, TT, D], BF, kind="ExternalInput")
    xtb = nc.dram_tensor("xtb", [P, KD, T], BF, kind="ExternalInput")
    w1t = nc.dram_tensor("w1t", [cfg.NV, HCN, P, KD, P], BF, kind="ExternalInput")
    w3t = nc.dram_tensor("w3t", [cfg.NV, HCN, P, KD, P], BF, kind="ExternalInput")
    w2t = nc.dram_tensor("w2t", [cfg.NV, P, HCN, D], BF, kind="ExternalInput")
    b1a = nc.dram_tensor("b1a", [cfg.NV, P, HCN], F32, kind="ExternalInput")
    b3a = nc.dram_tensor("b3a", [cfg.NV, P, HCN], F32, kind="ExternalInput")
    b2r = nc.dram_tensor("b2r", [1, cfg.NV, D], BF, kind="ExternalInput")
    b2c = nc.dram_tensor("b2c", [P, E * KD], F32, kind="ExternalInput")
    gwt = nc.dram_tensor("gwt", [P, KD, E], F32, kind="ExternalInput")
    gb = nc.dram_tensor("gb", [1, E], F32, kind="ExternalInput")
    ones1 = nc.dram_tensor("ones1", [1, P], BF, kind="ExternalInput")
    onesc = nc.dram_tensor("onesc", [P, 1], BF, kind="ExternalInput")
    lt = nc.dram_tensor("lt", [P, P], BF, kind="ExternalInput")
    ident = nc.dram_tensor("ident", [P, P], BF, kind="ExternalInput")
    iota = nc.dram_tensor("iota", [P, capm], F32, kind="ExternalInput")
    y = nc.dram_tensor("y", [P, TT, D], BF, kind="ExternalOutput")

    OOB = 3.0e6

    from contextlib import ExitStack
    with tile.TileContext(nc) as tc:
        with ExitStack() as stack:
            pool_specs = dict(
                const1=dict(bufs=1), xbig=dict(bufs=1),
                gchunk=dict(bufs=2), gtmp=dict(bufs=4),
                w13=dict(bufs=3), w2p=dict(bufs=2), b13=dict(bufs=2),
                xep=dict(bufs=1), hrout=dict(bufs=1), hshp=dict(bufs=2),
                yeBp=dict(bufs=1), pep=dict(bufs=2), s1p=dict(bufs=2),
                combp=dict(bufs=8), ytp=dict(bufs=2),
                ps_o1=dict(bufs=2, space="PSUM"),
                ps_o3=dict(bufs=2, space="PSUM"),
                ps_l2=dict(bufs=2, space="PSUM"),
                ps_tr=dict(bufs=2, space="PSUM"),
            )
            pools = {n: stack.enter_context(tc.tile_pool(name=n, **kw))
                     for n, kw in pool_specs.items()}
            (const1, xbig, gchunk, gtmp, w13, w2p, b13, xep, hrout,
             hshp, yeBp, pep, s1p, combp, ytp, ps_o1, ps_o3, ps_l2,
             ps_tr) = (
                pools[n] for n in (
                    "const1", "xbig", "gchunk", "gtmp", "w13", "w2p", "b13",
                    "xep", "hrout", "hshp", "yeBp", "pep", "s1p",
                    "combp", "ytp", "ps_o1", "ps_o3", "ps_l2", "ps_tr"))
            # ---- resident state ----
            # xtok_sb and xTb share one 2MB buffer (tag xb): xtok is dead
            # after the last gather; xTb is DMA'd into the same space then.
            xtok_sb = xbig.tile([P, TT, D], BF, name="xtok_sb", tag="xb")
            yacc = const1.tile([P, TT, D], F32)
            cw = const1.tile([P, TT, E], F32)
            posb_all = const1.tile([P, TT, E], F32)
            ye_sb = const1.tile([P, E * NP, D], BF)
            gwt_sb = const1.tile([P, KD, E], F32)
            gb_sb = const1.tile([1, E], F32)
            ones_sb = const1.tile([1, P], BF)
            onesc_sb = const1.tile([P, 1], BF)
            lt_sb = const1.tile([P, P], BF)
            id_sb = const1.tile([P, P], BF)
            iota_sb = const1.tile([P, capm], F32)
            b2c_sb = const1.tile([P, E * KD], F32)
            b2sh_sb = const1.tile([1, D], BF)
            zerob = const1.tile([P, 1], F32)
            onesf = const1.tile([1, P], F32)

            # first two gate tiles + gate weights lead the DMA queues so
            # the gate starts without sitting behind the bulk prologue
            xc_pre = {}
            for m in range(2):
                xc = gchunk.tile([P, KD, P], F32, name="xchunk")
                nc.sync.dma_start(out=xc[:], in_=xT[:, :, m * P:(m + 1) * P])
                xc_pre[m] = xc
            nc.sync.dma_start(out=gwt_sb[:], in_=gwt[:])
            nc.sync.dma_start(out=gb_sb[:], in_=gb[:])
            # secondary consts flow on the Act queue in parallel so the
            # gate's per-tile xchunk stream on sync isn't delayed
            nc.scalar.dma_start(out=ones_sb[:], in_=ones1[:])
            nc.scalar.dma_start(out=onesc_sb[:], in_=onesc[:])
            nc.scalar.dma_start(out=lt_sb[:], in_=lt[:])
            nc.scalar.dma_start(out=id_sb[:], in_=ident[:])
            nc.scalar.dma_start(out=iota_sb[:], in_=iota[:])
            nc.scalar.dma_start(out=b2c_sb[:], in_=b2c[:])
            nc.scalar.dma_start(out=b2sh_sb[:], in_=b2r[0:1, E, :])
            nc.vector.memset(zerob[:], 0.0)
            nc.vector.memset(onesf[:], 1.0)

            # prefetch expert 0's first L1 weight chunks
            pre_w = {}
            for hc in range(3):
                w1c = w13.tile([P, KD, P], BF, name="w1c", tag="w1c")
                nc.sync.dma_start(out=w1c[:], in_=w1t[0, hc])
                w3c = w13.tile([P, KD, P], BF, name="w3c", tag="w3c")
                nc.gpsimd.dma_start(out=w3c[:], in_=w3t[0, hc])
                pre_w[hc] = (w1c, w3c)
            # xtok hands off to the DMA engines late (pool-queue tail) so
            # its 2MB transfer neither starves the small gate-const loads
            # nor blocks the per-tile gate xchunk stream on sync; it is
            # first needed by expert 0's gather, well after the gate.
            nc.gpsimd.dma_start(out=xtok_sb[:], in_=xtok[:])

            # ---- gate + bucket positions, per 128-token tile ----
            cntb = None
            for m in range(TT):
                if m in xc_pre:
                    xchunk = xc_pre[m]
                else:
                    xchunk = gchunk.tile([P, KD, P], F32)
                    nc.sync.dma_start(out=xchunk[:],
                                      in_=xT[:, :, m * P:(m + 1) * P])

                pg = ps_l2.tile([P, E], F32, space="PSUM", name="pg", tag="l2")
                for k in range(KD):
                    nc.tensor.matmul(out=pg[:], lhsT=xchunk[:, k, :],
                                     rhs=gwt_sb[:, k, :],
                                     start=(k == 0), stop=False)
                nc.tensor.matmul(out=pg[:], lhsT=onesf[:], rhs=gb_sb[:],
                                 start=False, stop=True)

                lg = gtmp.tile([P, E], F32)
                nc.scalar.activation(lg[:], pg[:], AF.Copy)
                m8 = gtmp.tile([P, 8], F32)
                nc.vector.max(m8[:], lg[:])
                ex = gtmp.tile([P, E], F32)
                nc.vector.tensor_scalar(out=ex[:], in0=lg[:],
                                        scalar1=m8[:, 0:1], scalar2=None,
                                        op0=OP.subtract)
                nc.scalar.activation(ex[:], ex[:], AF.Exp, bias=zerob[:])
                mask = gtmp.tile([P, E], F32)
                nc.vector.tensor_scalar(out=mask[:], in0=lg[:],
                                        scalar1=m8[:, 1:2], scalar2=None,
                                        op0=OP.is_ge)
                e2 = gtmp.tile([P, 1], F32)
                nc.vector.tensor_tensor(out=e2[:], in0=m8[:, 1:2],
                                        in1=m8[:, 0:1], op=OP.subtract)
                nc.scalar.activation(e2[:], e2[:], AF.Exp, bias=zerob[:])
                den = gtmp.tile([P, 1], F32)
                nc.vector.tensor_scalar(out=den[:], in0=e2[:], scalar1=1.0,
                                        scalar2=None, op0=OP.add)
                rec = gtmp.tile([P, 1], F32)
                nc.vector.reciprocal(rec[:], den[:])
                cwm = gtmp.tile([P, E], F32)
                nc.vector.tensor_mul(cwm[:], ex[:], mask[:])
                nc.vector.tensor_scalar(out=cw[:, m, :], in0=cwm[:],
                                        scalar1=rec[:, 0:1], scalar2=None,
                                        op0=OP.mult)

                # bucket-local slot: pair prefix(mask) - mask; OOB unrouted
                maskb = gtmp.tile([P, E], BF)
                nc.vector.tensor_copy(maskb[:], mask[:])
                pp = ps_tr.tile([P, E], F32, space="PSUM", name="pp", tag="tr")
                if m % 2 == 0:
                    nc.tensor.matmul(out=pp[:], lhsT=lt_sb[:],
                                     rhs=maskb[:], start=True, stop=True)
                    cnt_ps = ps_tr.tile([1, E], F32, space="PSUM",
                                        name="cntp", tag="tr")
                    nc.tensor.matmul(out=cnt_ps[:], lhsT=onesc_sb[:],
                                     rhs=maskb[:], start=True, stop=True)
                    cntb = gtmp.tile([1, E], BF, name="cntb")
                    nc.scalar.activation(cntb[:], cnt_ps[:], AF.Copy)
                else:
                    nc.tensor.matmul(out=pp[:], lhsT=lt_sb[:],
                                     rhs=maskb[:], start=True, stop=False)
                    nc.tensor.matmul(out=pp[:], lhsT=ones_sb[:],
                                     rhs=cntb[:], start=False, stop=True)
                t1m = gtmp.tile([P, E], F32)
                nc.vector.scalar_tensor_tensor(out=t1m[:], in0=mask[:],
                                               scalar=-1.0, in1=pp[:],
                                               op0=OP.mult, op1=OP.add)
                notm = gtmp.tile([P, E], F32)
                nc.vector.tensor_scalar(out=notm[:], in0=mask[:],
                                        scalar1=-1.0, scalar2=1.0,
                                        op0=OP.mult, op1=OP.add)
                nc.vector.scalar_tensor_tensor(out=posb_all[:, m, :],
                                               in0=notm[:], scalar=OOB,
                                               in1=t1m[:],
                                               op0=OP.mult, op1=OP.add)

            # ---- routed experts over dispatched slots ----
            pre_sh = {}
            for e in range(E):
                capm_e = cfg.capms[e]
                CAPE = NP * capm_e
                b1sb = b13.tile([P, HCN], F32, name="b1sb", tag="b1")
                nc.sync.dma_start(out=b1sb[:], in_=b1a[e])
                b3sb = b13.tile([P, HCN], F32, name="b3sb", tag="b3")
                nc.sync.dma_start(out=b3sb[:], in_=b3a[e])

                # prefetch the first two w2 quarters; they land during L1
                w2qs_pre = []
                for dq in range(2):
                    w2q0 = w2p.tile([P, HCN, 256], BF, name="w2q", tag="w2q")
                    nc.sync.dma_start(
                        out=w2q0[:],
                        in_=w2t[e][:, :, dq * 256:(dq + 1) * 256])
                    w2qs_pre.append(w2q0)

                # one-hot dispatch tiles for all 8 token tiles
                pe_all = pep.tile([P, TT, capm_e], BF, name="pe_all", tag="pe")
                for m in range(TT):
                    nc.vector.tensor_scalar(
                        out=pe_all[:, m, :], in0=iota_sb[:, :capm_e],
                        scalar1=posb_all[:, m, e:e + 1],
                        scalar2=None, op0=OP.is_equal)

                # matmul gather: xeT[k][d, slot] = sum_m x_m^T @ Pe_m
                xeT = xep.tile([P, KD, CAPE], BF, name="xeT", tag="xeT")
                for k in range(KD):
                    gxp, gxt = (ps_l2, "l2") if k % 2 == 0 else (ps_tr, "tr")
                    gx = gxp.tile([P, CAPE], F32, space="PSUM",
                                  name="gx", tag=gxt)
                    for pr in range(NP):
                        for h in range(2):
                            m = 2 * pr + h
                            nc.tensor.matmul(
                                out=gx[:, pr * capm_e:(pr + 1) * capm_e],
                                lhsT=xtok_sb[:, m, k * P:(k + 1) * P],
                                rhs=pe_all[:, m, :],
                                start=(h == 0), stop=(h == 1))
                    nc.vector.tensor_copy(xeT[:, k, :], gx[:])

                if e == E - 1:
                    # last xtok reader just emitted: reload the shared
                    # buffer with d-major x for the tail's shared expert;
                    # the transfer hides under expert 7's L1/L2.
                    xTb = xbig.tile([P, KD, T], BF, name="xTb", tag="xb")
                    nc.scalar.dma_start(out=xTb[:], in_=xtb[:])

                # L1: hT[h, slot] = silu(W1 xe + b1) * (W3 xe + b3)
                hT = hrout.tile([P, HCN, CAPE], BF, name="hT", tag="hT")
                for hc in range(HCN):
                    if e == 0 and hc in pre_w:
                        w1c, w3c = pre_w[hc]
                    else:
                        w1c = w13.tile([P, KD, P], BF, name="w1c", tag="w1c")
                        nc.sync.dma_start(out=w1c[:], in_=w1t[e, hc])
                        w3c = w13.tile([P, KD, P], BF, name="w3c", tag="w3c")
                        nc.gpsimd.dma_start(out=w3c[:], in_=w3t[e, hc])
                    o1 = ps_o1.tile([P, CAPE], F32, space="PSUM",
                                    name="o1", tag="o1")
                    for k in range(KD):
                        nc.tensor.matmul(out=o1[:], lhsT=w1c[:, k, :],
                                         rhs=xeT[:, k, :],
                                         start=(k == 0), stop=(k == KD - 1))
                    s1 = s1p.tile([P, CAPE], F32, name="s1", tag="s1")
                    nc.scalar.activation(s1[:], o1[:], AF.Sigmoid,
                                         bias=b1sb[:, hc:hc + 1])
                    t1 = s1p.tile([P, CAPE], F32, name="t1", tag="t1")
                    nc.vector.scalar_tensor_tensor(
                        out=t1[:], in0=o1[:],
                        scalar=b1sb[:, hc:hc + 1], in1=s1[:],
                        op0=OP.add, op1=OP.mult)
                    o3 = ps_o3.tile([P, CAPE], F32, space="PSUM",
                                    name="o3", tag="o3")
                    for k in range(KD):
                        nc.tensor.matmul(out=o3[:], lhsT=w3c[:, k, :],
                                         rhs=xeT[:, k, :],
                                         start=(k == 0), stop=(k == KD - 1))
                    nc.vector.scalar_tensor_tensor(
                        out=hT[:, hc, :], in0=o3[:],
                        scalar=b3sb[:, hc:hc + 1], in1=t1[:],
                        op0=OP.add, op1=OP.mult)

                # L2 (d-partition orientation) + bias, then transpose to
                # slot-major ye tiles; transposes staggered one dc behind
                # the chains so their yeB reads never stall the PE.
                yeB = yeBp.tile([P, KD, CAPE], BF, name="yeB", tag="yeB")
                pend = []

                def emit_transp(dc, e=e, yeB=yeB, capm_e=capm_e):
                    for pr in range(NP):
                        p2t = ps_tr.tile([P, P], BF, space="PSUM",
                                         name="p2t", tag="tr")
                        nc.tensor.transpose(
                            out=p2t[:capm_e, :],
                            in_=yeB[:, dc, pr * capm_e:(pr + 1) * capm_e],
                            identity=id_sb[:])
                        nc.vector.tensor_copy(
                            ye_sb[0:capm_e, e * NP + pr, dc * P:(dc + 1) * P],
                            p2t[:capm_e, :])

                for dq in range(4):
                    w2q = w2qs_pre[dq]
                    for dc2 in range(2):
                        dc = dq * 2 + dc2
                        pl2 = ps_l2.tile([P, CAPE], F32, space="PSUM",
                                         name="pl2", tag="l2")
                        for hc in range(HCN):
                            nc.tensor.matmul(
                                out=pl2[:],
                                lhsT=w2q[:, hc, dc2 * P:(dc2 + 1) * P],
                                rhs=hT[:, hc, :],
                                start=(hc == 0), stop=(hc == HCN - 1))
                        nc.vector.tensor_scalar(
                            out=yeB[:, dc, :], in0=pl2[:],
                            scalar1=b2c_sb[:, e * KD + dc:e * KD + dc + 1],
                            scalar2=None, op0=OP.add)
                        if pend:
                            emit_transp(pend.pop())
                        pend.append(dc)
                    if dq + 2 < 4:
                        # refill two quarters ahead (this quarter's chains
                        # just freed the buffer, so the queue-head wait is
                        # short; only next-expert w1c prefetches sit behind)
                        w2n = w2p.tile([P, HCN, 256], BF, name="w2q",
                                       tag="w2q")
                        nc.sync.dma_start(
                            out=w2n[:],
                            in_=w2t[e][:, :, (dq + 2) * 256:(dq + 3) * 256])
                        w2qs_pre.append(w2n)
                while pend:
                    emit_transp(pend.pop())

            # ---- tail: shared sub-experts (hidden 512 each) + combine ----
            def emit_shared_l1_unit(s, hcl):
                sv = E + s // 4
                hcg = (s % 4) * HQ + hcl
                if (s, hcl) in pre_sh:
                    w1c, w3c = pre_sh[(s, hcl)]
                else:
                    w1c = w13.tile([P, KD, P], BF, name="w1c", tag="w1c")
                    nc.sync.dma_start(out=w1c[:], in_=w1t[sv, hcg])
                    w3c = w13.tile([P, KD, P], BF, name="w3c", tag="w3c")
                    nc.gpsimd.dma_start(out=w3c[:], in_=w3t[sv, hcg])
                hT_s = hts[s]
                for ft in range(FT):
                    fsl = slice(ft * 512, (ft + 1) * 512)
                    o1 = ps_o1.tile([P, 512], F32, space="PSUM",
                                    name="o1", tag="o1")
                    for k in range(KD):
                        nc.tensor.matmul(out=o1[:], lhsT=w1c[:, k, :],
                                         rhs=xTb[:, k, fsl],
                                         start=(k == 0), stop=(k == KD - 1))
                    s1 = s1p.tile([P, 512], F32, name="s1", tag="s1")
                    nc.scalar.activation(s1[:], o1[:], AF.Sigmoid,
                                         bias=bsh1[s // 4][:, hcg:hcg + 1])
                    t1 = s1p.tile([P, 512], F32, name="t1", tag="t1")
                    nc.vector.scalar_tensor_tensor(
                        out=t1[:], in0=o1[:],
                        scalar=bsh1[s // 4][:, hcg:hcg + 1], in1=s1[:],
                        op0=OP.add, op1=OP.mult)
                    o3 = ps_o3.tile([P, 512], F32, space="PSUM",
                                    name="o3", tag="o3")
                    for k in range(KD):
                        nc.tensor.matmul(out=o3[:], lhsT=w3c[:, k, :],
                                         rhs=xTb[:, k, fsl],
                                         start=(k == 0), stop=(k == KD - 1))
                    nc.vector.scalar_tensor_tensor(
                        out=hT_s[:, hcl, fsl], in0=o3[:],
                        scalar=bsh3[s // 4][:, hcg:hcg + 1], in1=t1[:],
                        op0=OP.add, op1=OP.mult)

            def emit_combine(m):
                pr = m // 2
                p2s_l = []
                for e in range(E):
                    capm_e = cfg.capms[e]
                    pe2 = gtmp.tile([P, capm_e], BF, name="pe2")
                    nc.vector.tensor_scalar(out=pe2[:], in0=iota_sb[:, :capm_e],
                                            scalar1=posb_all[:, m, e:e + 1],
                                            scalar2=None, op0=OP.is_equal)
                    pew = gtmp.tile([P, capm_e], BF, name="pew")
                    nc.vector.tensor_scalar(out=pew[:], in0=pe2[:],
                                            scalar1=cw[:, m, e:e + 1],
                                            scalar2=None, op0=OP.mult)
                    p2c = ps_tr.tile([P, P], BF, space="PSUM",
                                     name="p2c", tag="tr")
                    nc.tensor.transpose(out=p2c[:capm_e, :], in_=pew[:],
                                        identity=id_sb[:])
                    p2s = combp.tile([capm_e, P], BF, name="p2s")
                    nc.scalar.activation(p2s[:], p2c[:capm_e, :], AF.Copy)
                    p2s_l.append(p2s)
                for dt in range(DT):
                    dsl = slice(dt * 512, (dt + 1) * 512)
                    yp = ps_l2.tile([P, 512], F32, space="PSUM",
                                    name="yp", tag="l2")
                    for e in range(E):
                        nc.tensor.matmul(
                            out=yp[:], lhsT=p2s_l[e][:],
                            rhs=ye_sb[0:cfg.capms[e], e * NP + pr, dsl],
                            start=(e == 0), stop=(e == E - 1))
                    nc.vector.tensor_copy(yacc[:, m, dsl], yp[:])

            def emit_shared_l2(s):
                sv = E + s // 4
                hcg0 = (s % 4) * HQ
                w2q = w2p.tile([P, HQ, D], BF, name="w2qs", tag="w2q")
                nc.scalar.dma_start(out=w2q[:],
                                    in_=w2t[sv][:, hcg0:hcg0 + HQ, :])
                hT_s = hts[s]
                for tt in range(TT):
                    tsl = slice(tt * P, (tt + 1) * P)
                    for dt in range(DT):
                        dsl = slice(dt * 512, (dt + 1) * 512)
                        if s >= NSH - 2:
                            # L1 is done by now: o1/o3 banks are free, use a
                            # deeper 3-pool rotation so chains never wait on
                            # the yacc-add evictions
                            pl, tg = [(ps_l2, "l2"), (ps_tr, "tr"),
                                      (ps_o1, "o1")][(tt * DT + dt) % 3]
                        else:
                            pl, tg = ((ps_l2, "l2") if (tt * DT + dt) % 2 == 0
                                      else (ps_tr, "tr"))
                        yp2 = pl.tile([P, 512], F32, space="PSUM",
                                      name="yp2", tag=tg)
                        if s == 0:
                            nc.tensor.matmul(out=yp2[:], lhsT=ones_sb[:],
                                             rhs=b2sh_sb[0:1, dsl],
                                             start=True, stop=False)
                        for hcl in range(HQ):
                            nc.tensor.matmul(
                                out=yp2[:], lhsT=hT_s[:, hcl, tsl],
                                rhs=w2q[:, hcl, dsl],
                                start=(s != 0 and hcl == 0),
                                stop=(hcl == HQ - 1))
                        if s < NSH - 1:
                            nc.vector.tensor_add(yacc[:, tt, dsl],
                                                 yacc[:, tt, dsl], yp2[:])
                        else:
                            yt = ytp.tile([P, 512], BF, name="yt", bufs=4)
                            nc.vector.tensor_add(yt[:], yacc[:, tt, dsl],
                                                 yp2[:])
                            nc.sync.dma_start(out=y[:, tt, dsl], in_=yt[:])

            bsh1 = []
            bsh3 = []
            for sv in range(2):
                b1s = b13.tile([P, HCN], F32, name="b1sh", tag="b1sh")
                nc.sync.dma_start(out=b1s[:], in_=b1a[E + sv])
                b3s = b13.tile([P, HCN], F32, name="b3sh", tag="b3sh")
                nc.sync.dma_start(out=b3s[:], in_=b3a[E + sv])
                bsh1.append(b1s)
                bsh3.append(b3s)

            hts = {}
            for s in range(NSH):
                if s < 2:
                    hts[s] = hshp.tile([P, HQ, T], BF, name=f"hTs{s}",
                                       tag="hTs")
            # interleave first two shared sub-experts' L1 with combine;
            # combine leads: its inputs (ye, cw, pos) are ready at routed
            # end, covering the xTb/w1c arrival for the shared L1
            ci = 0
            for s in range(2):
                for hcl in range(HQ):
                    emit_combine(ci)
                    ci += 1
                    emit_shared_l1_unit(s, hcl)
            # pipeline: L2(s) || L1(s+2)
            for s in range(NSH):
                emit_shared_l2(s)
                if s + 2 < NSH:
                    hts[s + 2] = hshp.tile([P, HQ, T], BF, name=f"hTs{s+2}",
                                           tag="hTs")
                    for hcl in range(HQ):
                        emit_shared_l1_unit(s + 2, hcl)

    nc.compile()
    return nc


# ---------------- host-side packing ----------------

def pack_static(cfg: Cfg, gate_w, gate_b, w1, b1, w2, b2, w3, b3,
                sw1, sb1, sw2, sb2, sw3, sb3):
    D, H, E, NV, n_sh = cfg.D, cfg.H, cfg.E, cfg.NV, cfg.n_sh
    KD, HCN = cfg.KD, cfg.HCN

    w1T = np.transpose(w1, (0, 2, 1))                      # [E, D, H]
    w3T = np.transpose(w3, (0, 2, 1))
    w2T = np.transpose(w2, (0, 2, 1))                      # [E, H, D]
    s1T = sw1.T.reshape(D, n_sh, H).transpose(1, 0, 2)     # [n_sh, D, H]
    s3T = sw3.T.reshape(D, n_sh, H).transpose(1, 0, 2)
    s2T = sw2.T.reshape(n_sh, H, D)                        # [n_sh, H, D]
    w1T_all = np.concatenate([w1T, s1T], 0)                # [NV, D, H]
    w3T_all = np.concatenate([w3T, s3T], 0)
    w2T_all = np.concatenate([w2T, s2T], 0)                # [NV, H, D]

    w1t = np.ascontiguousarray(
        w1T_all.reshape(NV, KD, P, HCN, P).transpose(0, 3, 2, 1, 4)).astype(BF16)
    w3t = np.ascontiguousarray(
        w3T_all.reshape(NV, KD, P, HCN, P).transpose(0, 3, 2, 1, 4)).astype(BF16)
    w2t = np.ascontiguousarray(
        w2T_all.reshape(NV, HCN, P, D).transpose(0, 2, 1, 3)).astype(BF16)

    b1_all = np.concatenate([b1, sb1.reshape(n_sh, H)], 0)  # [NV, H]
    b3_all = np.concatenate([b3, sb3.reshape(n_sh, H)], 0)
    b1a = np.ascontiguousarray(
        b1_all.reshape(NV, HCN, P).transpose(0, 2, 1)).astype(np.float32)
    b3a = np.ascontiguousarray(
        b3_all.reshape(NV, HCN, P).transpose(0, 2, 1)).astype(np.float32)

    b2_all = np.concatenate(
        [b2, sb2[None], np.zeros((n_sh - 1, D), np.float32)], 0)  # [NV, D]
    b2r = b2_all[None].astype(BF16)                         # [1, NV, D]
    # routed b2 in d-partition layout: [P, E*KD], col e*KD+dc = b2[e, dc*128+p]
    b2c = np.ascontiguousarray(
        b2.reshape(E, KD, P).transpose(2, 0, 1).reshape(P, E * KD)
    ).astype(np.float32)

    gwt = np.ascontiguousarray(
        gate_w.T.reshape(KD, P, E).transpose(1, 0, 2)).astype(np.float32)
    gb = gate_b[None].astype(np.float32)
    ones1 = np.ones((1, P), BF16)
    onesc = np.ones((P, 1), BF16)
    lt = np.triu(np.ones((P, P))).astype(BF16)
    ident = np.eye(P).astype(BF16)
    iota = np.tile(np.arange(cfg.capm, dtype=np.float32), (P, 1))

    return dict(w1t=w1t, w3t=w3t, w2t=w2t, b1a=b1a, b3a=b3a, b2r=b2r,
                b2c=b2c, gwt=gwt, gb=gb, ones1=ones1, onesc=onesc,
                lt=lt, ident=ident, iota=iota)


def pack_xtok(cfg: Cfg, x_tokens):
    T, D = x_tokens.shape
    xt = x_tokens.reshape(cfg.TT, P, D).transpose(1, 0, 2)
    return np.ascontiguousarray(xt).astype(BF16)


def pack_xT(cfg: Cfg, x_tokens):
    T, D = x_tokens.shape
    xT = x_tokens.T.reshape(cfg.KD, P, T).transpose(1, 0, 2)
    return np.ascontiguousarray(xT).astype(np.float32)


def unpack_y(cfg: Cfg, y_dev):
    return np.ascontiguousarray(
        y_dev.transpose(1, 0, 2).reshape(cfg.T, cfg.D)).astype(np.float32)


def balance_tokens(xf, gate_w, gate_b, E=8, margin=2):
    """Assign tokens to 256-token buckets so per-(bucket, expert) routed
    counts are near their per-expert means. Returns (perm, capms):
    bucket-major token order and per-expert slot capacities."""
    N = xf.shape[0]
    NB = N // 256
    logits = xf @ gate_w.T + gate_b
    idx = np.argsort(-logits, axis=1)[:, :2]
    tgt = np.zeros(E)
    for e in range(E):
        tgt[e] = ((idx[:, 0] == e) | (idx[:, 1] == e)).sum() / NB
    tgt = np.maximum(tgt, 1.0)
    cnt = np.zeros((NB, E), np.float64)
    fill = np.zeros(NB, np.int64)
    assign = np.empty(N, np.int32)
    rng = np.random.RandomState(0)
    BIG = 1 << 40
    for t in rng.permutation(N):
        a, b = idx[t]
        s = np.maximum((cnt[:, a] + 1) / tgt[a],
                       (cnt[:, b] + 1) / tgt[b]) * 4096 + fill
        s[fill >= 256] = BIG
        bb = int(np.argmin(s))
        assign[t] = bb
        cnt[bb, a] += 1
        cnt[bb, b] += 1
        fill[bb] += 1
    perm = np.argsort(assign.astype(np.int64) * N + np.arange(N))
    capms = tuple(int(c) + margin for c in cnt.max(0))
    return perm, capms


_CACHE = {}


def _get_nc(cfg: Cfg):
    key = (cfg.D, cfg.H, cfg.E, cfg.n_sh, cfg.T, cfg.capms)
    if key not in _CACHE:
        _CACHE[key] = build_nc_v2(cfg)
    return _CACHE[key]


def plan_cfg(inputs):
    """Balance tokens from the actual routing; returns (cfg, perm)."""
    x = np.asarray(inputs["x"], np.float32)
    B, S, D = x.shape
    xf = x.reshape(-1, D)
    perm, capms = balance_tokens(
        xf, np.asarray(inputs["gate_w"], np.float32),
        np.asarray(inputs["gate_b"], np.float32))
    cfg = Cfg(D=D, T=(B * S) // 8, n_cores=8, capms=capms)
    return cfg, perm


def make_in_maps(cfg: Cfg, inputs, perm):
    static = pack_static(
        cfg,
        np.asarray(inputs["gate_w"], np.float32), np.asarray(inputs["gate_b"], np.float32),
        np.asarray(inputs["w1"], np.float32), np.asarray(inputs["b1"], np.float32),
        np.asarray(inputs["w2"], np.float32), np.asarray(inputs["b2"], np.float32),
        np.asarray(inputs["w3"], np.float32), np.asarray(inputs["b3"], np.float32),
        np.asarray(inputs["sw1"], np.float32), np.asarray(inputs["sb1"], np.float32),
        np.asarray(inputs["sw2"], np.float32), np.asarray(inputs["sb2"], np.float32),
        np.asarray(inputs["sw3"], np.float32), np.asarray(inputs["sb3"], np.float32),
    )
    x = np.asarray(inputs["x"], np.float32)
    B, S, D = x.shape
    xp = x.reshape(-1, D)[perm]
    in_maps = []
    for c in range(cfg.n_cores):
        mm = dict(static)
        xc = xp[c * cfg.T:(c + 1) * cfg.T]
        mm["xT"] = pack_xT(cfg, xc)
        mm["xtok"] = pack_xtok(cfg, xc)
        mm["xtb"] = mm["xT"].astype(BF16)
        in_maps.append(mm)
    return in_maps


def kernel(**inputs) -> np.ndarray:
    x = np.asarray(inputs["x"], np.float32)
    B, S, D = x.shape
    cfg, perm = plan_cfg(inputs)
    nc = _get_nc(cfg)
    in_maps = make_in_maps(cfg, inputs, perm)
    res = run_bass_kernel_spmd(nc, in_maps, list(range(cfg.n_cores)))
    yp = np.concatenate(
        [unpack_y(cfg, res.results[c]["y"]) for c in range(cfg.n_cores)], 0)
    out = np.empty_like(yp)
    out[perm] = yp
    return out.reshape(B, S, D)


# revision 62
# speedup vs baseline: 1.2508x; 1.0045x over previous
"""MoE (8 routed experts, top-2, + shared expert) on 8 NeuronCores.

Data-parallel over tokens (1024/core), weights replicated. The host
load-balances the token->bucket assignment (any sharding is allowed) so
each (256-token bucket, expert) routed count sits at its per-expert
mean, letting the capacity-dispatched kernel run with per-expert
capacities capm_e = max bucket count + 2 (60..74 on this routing)
instead of the binomial-tail uniform 96.

Device kernel (per core):
  1. Gate in fp32 (matches reference routing bit-for-bit for the
     observed >=1.7e-4 top-2/3 logit gaps), renormalized top-2 combine
     weights cw, and bucket-local slot positions via triangular-matmul
     prefix sums.
  2. Routed experts e=0..7: one-hot matmul gather of x into CAPE=296
     slots, SwiGLU L1 (feature-major, free dim = slots), L2 in
     d-partition orientation (out[d, slot], bias via activation), then
     PE transposes to slot-major ye tiles held in SBUF.
  3. Tail: shared expert (8 sub-experts of hidden 512) interleaved with
     the scatter-combine (transposed scaled one-hots x ye), everything
     accumulating into a token-major f32 yacc; last shared sub-expert's
     L2 fuses the final add and streams y out.

Matmuls are bf16 with fp32 accumulation; weight DMA is split across the
SP/Pool/Act queues to avoid head-of-line blocking on one DMA queue.
"""

import numpy as np
import ml_dtypes

import concourse.bacc as bacc
import concourse.bass as bass
import concourse.tile as tile
import concourse.mybir as mybir
from concourse.bass_utils import run_bass_kernel_spmd

BF16 = ml_dtypes.bfloat16
F32 = mybir.dt.float32
BF = mybir.dt.bfloat16
AF = mybir.ActivationFunctionType
OP = mybir.AluOpType

P = 128


class Cfg:
    def __init__(self, D=1024, H=2048, E=8, n_sh=2, T=1024, n_cores=8,
                 capms=(74,) * 8):
        self.D, self.H, self.E, self.n_sh, self.T = D, H, E, n_sh, T
        self.NV = E + n_sh          # packed weight rows (8 routed + 2 shared)
        self.HS = n_sh * H          # shared hidden total (4096)
        self.KD = D // P            # contraction chunks over D
        self.HCN = H // P           # h chunks per packed VE
        self.TT = T // P            # token 128-tiles per core
        self.FT = T // 512          # shared L1 free 512-tiles
        self.DT = D // 512          # 512-wide d tiles
        self.n_cores = n_cores
        self.capms = tuple(capms)   # slots per (256-token bucket, expert)
        self.capm = max(self.capms)  # iota / tile sizing width
        self.NP = self.TT // 2      # buckets per core (pair of tiles)
        self.NSH = 8                # shared sub-experts
        self.HQ = (self.HS // P) // self.NSH  # h-chunks per sub-expert (4)


def build_nc_v2(cfg: Cfg):
    D, H, E, T = cfg.D, cfg.H, cfg.E, cfg.T
    KD, HCN, TT, FT, DT = cfg.KD, cfg.HCN, cfg.TT, cfg.FT, cfg.DT
    capm, NP = cfg.capm, cfg.NP
    NSH, HQ = cfg.NSH, cfg.HQ

    nc = bacc.Bacc("TRN2", target_bir_lowering=False)

    xT = nc.dram_tensor("xT", [P, KD, T], F32, kind="ExternalInput")
    xtok = nc.dram_tensor("xtok", [P, TT, D], BF, kind="ExternalInput")
    xtb = nc.dram_tensor("xtb", [P, KD, T], BF, kind="ExternalInput")
    w1t = nc.dram_tensor("w1t", [cfg.NV, HCN, P, KD, P], BF, kind="ExternalInput")
    w3t = nc.dram_tensor("w3t", [cfg.NV, HCN, P, KD, P], BF, kind="ExternalInput")
    w2t = nc.dram_tensor("w2t", [cfg.NV, P, HCN, D], BF, kind="ExternalInput")
    b1a = nc.dram_tensor("b1a", [cfg.NV, P, HCN], F32, kind="ExternalInput")
    b3a = nc.dram_tensor("b3a", [cfg.NV, P, HCN], F32, kind="ExternalInput")
    b2r = nc.dram_tensor("b2r", [1, cfg.NV, D], BF, kind="ExternalInput")
    b2c = nc.dram_tensor("b2c", [P, E * KD], F32, kind="ExternalInput")
    gwt = nc.dram_tensor("gwt", [P, KD, E], F32, kind="ExternalInput")
    gb = nc.dram_tensor("gb", [1, E], F32, kind="ExternalInput")
    ones1 = nc.dram_tensor("ones1", [1, P], BF, kind="ExternalInput")
    onesc = nc.dram_tensor("onesc", [P, 1], BF, kind="ExternalInput")
    lt = nc.dram_tensor("lt", [P, P], BF, kind="ExternalInput")
    ident = nc.dram_tensor("ident", [P, P], BF, kind="ExternalInput")
    iota = nc.dram_tensor("iota", [P, capm], F32, kind="ExternalInput")
    y = nc.dram_tensor("y", [P, TT, D], BF, kind="ExternalOutput")

    OOB = 3.0e6

    from contextlib import ExitStack
    with tile.TileContext(nc) as tc:
        with ExitStack() as stack:
            pool_specs = dict(
                const1=dict(bufs=1), xbig=dict(bufs=1),
                gchunk=dict(bufs=2), gtmp=dict(bufs=4),
                w13=dict(bufs=3), w2p=dict(bufs=2), b13=dict(bufs=2),
                xep=dict(bufs=1), hrout=dict(bufs=1), hshp=dict(bufs=2),
                yeBp=dict(bufs=1), pep=dict(bufs=2), s1p=dict(bufs=2),
                combp=dict(bufs=8), ytp=dict(bufs=2),
                ps_o1=dict(bufs=2, space="PSUM"),
                ps_o3=dict(bufs=2, space="PSUM"),
                ps_l2=dict(bufs=2, space="PSUM"),
                ps_tr=dict(bufs=2, space="PSUM"),
            )
            pools = {n: stack.enter_context(tc.tile_pool(name=n, **kw))
                     for n, kw in pool_specs.items()}
            (const1, xbig, gchunk, gtmp, w13, w2p, b13, xep, hrout,
             hshp, yeBp, pep, s1p, combp, ytp, ps_o1, ps_o3, ps_l2,
             ps_tr) = (
                pools[n] for n in (
                    "const1", "xbig", "gchunk", "gtmp", "w13", "w2p", "b13",
                    "xep", "hrout", "hshp", "yeBp", "pep", "s1p",
                    "combp", "ytp", "ps_o1", "ps_o3", "ps_l2", "ps_tr"))
            # ---- resident state ----
            # xtok_sb and xTb share one 2MB buffer (tag xb): xtok is dead
            # after the last gather; xTb is DMA'd into the same space then.
            xtok_sb = xbig.tile([P, TT, D], BF, name="xtok_sb", tag="xb")
            yacc = const1.tile([P, TT, D], F32)
            cw = const1.tile([P, TT, E], F32)
            posb_all = const1.tile([P, TT, E], F32)
            ye_sb = const1.tile([P, E * NP, D], BF)
            gwt_sb = const1.tile([P, KD, E], F32)
            gb_sb = const1.tile([1, E], F32)
            ones_sb = const1.tile([1, P], BF)
            onesc_sb = const1.tile([P, 1], BF)
            lt_sb = const1.tile([P, P], BF)
            id_sb = const1.tile([P, P], BF)
            iota_sb = const1.tile([P, capm], F32)
            b2c_sb = const1.tile([P, E * KD], F32)
            b2sh_sb = const1.tile([1, D], BF)
            zerob = const1.tile([P, 1], F32)
            onesf = const1.tile([1, P], F32)

            # first two gate tiles + gate weights lead the DMA queues so
            # the gate starts without sitting behind the bulk prologue
            xc_pre = {}
            for m in range(2):
                xc = gchunk.tile([P, KD, P], F32, name="xchunk")
                nc.sync.dma_start(out=xc[:], in_=xT[:, :, m * P:(m + 1) * P])
                xc_pre[m] = xc
            nc.sync.dma_start(out=gwt_sb[:], in_=gwt[:])
            nc.sync.dma_start(out=gb_sb[:], in_=gb[:])
            # secondary consts flow on the Act queue in parallel so the
            # gate's per-tile xchunk stream on sync isn't delayed
            nc.scalar.dma_start(out=ones_sb[:], in_=ones1[:])
            nc.scalar.dma_start(out=onesc_sb[:], in_=onesc[:])
            nc.scalar.dma_start(out=lt_sb[:], in_=lt[:])
            nc.scalar.dma_start(out=id_sb[:], in_=ident[:])
            nc.scalar.dma_start(out=iota_sb[:], in_=iota[:])
            nc.scalar.dma_start(out=b2c_sb[:], in_=b2c[:])
            nc.scalar.dma_start(out=b2sh_sb[:], in_=b2r[0:1, E, :])
            nc.vector.memset(zerob[:], 0.0)
            nc.vector.memset(onesf[:], 1.0)

            # prefetch expert 0's first L1 weight chunks
            pre_w = {}
            for hc in range(3):
                w1c = w13.tile([P, KD, P], BF, name="w1c", tag="w1c")
                nc.sync.dma_start(out=w1c[:], in_=w1t[0, hc])
                w3c = w13.tile([P, KD, P], BF, name="w3c", tag="w3c")
                nc.gpsimd.dma_start(out=w3c[:], in_=w3t[0, hc])
                pre_w[hc] = (w1c, w3c)
            # xtok hands off to the DMA engines late (pool-queue tail) so
            # its 2MB transfer neither starves the small gate-const loads
            # nor blocks the per-tile gate xchunk stream on sync; it is
            # first needed by expert 0's gather, well after the gate.
            nc.gpsimd.dma_start(out=xtok_sb[:], in_=xtok[:])

            # ---- gate + bucket positions, per 128-token tile ----
            cntb = None
            for m in range(TT):
                if m in xc_pre:
                    xchunk = xc_pre[m]
                else:
                    xchunk = gchunk.tile([P, KD, P], F32)
                    nc.sync.dma_start(out=xchunk[:],
                                      in_=xT[:, :, m * P:(m + 1) * P])

                pg = ps_l2.tile([P, E], F32, space="PSUM", name="pg", tag="l2")
                for k in range(KD):
                    nc.tensor.matmul(out=pg[:], lhsT=xchunk[:, k, :],
                                     rhs=gwt_sb[:, k, :],
                                     start=(k == 0), stop=False)
                nc.tensor.matmul(out=pg[:], lhsT=onesf[:], rhs=gb_sb[:],
                                 start=False, stop=True)

                lg = gtmp.tile([P, E], F32)
                nc.scalar.activation(lg[:], pg[:], AF.Copy)
                m8 = gtmp.tile([P, 8], F32)
                nc.vector.max(m8[:], lg[:])
                ex = gtmp.tile([P, E], F32)
                nc.vector.tensor_scalar(out=ex[:], in0=lg[:],
                                        scalar1=m8[:, 0:1], scalar2=None,
                                        op0=OP.subtract)
                nc.scalar.activation(ex[:], ex[:], AF.Exp, bias=zerob[:])
                mask = gtmp.tile([P, E], F32)
                nc.vector.tensor_scalar(out=mask[:], in0=lg[:],
                                        scalar1=m8[:, 1:2], scalar2=None,
                                        op0=OP.is_ge)
                e2 = gtmp.tile([P, 1], F32)
                nc.vector.tensor_tensor(out=e2[:], in0=m8[:, 1:2],
                                        in1=m8[:, 0:1], op=OP.subtract)
                nc.scalar.activation(e2[:], e2[:], AF.Exp, bias=zerob[:])
                den = gtmp.tile([P, 1], F32)
                nc.vector.tensor_scalar(out=den[:], in0=e2[:], scalar1=1.0,
                                        scalar2=None, op0=OP.add)
                rec = gtmp.tile([P, 1], F32)
                nc.vector.reciprocal(rec[:], den[:])
                cwm = gtmp.tile([P, E], F32)
                nc.vector.tensor_mul(cwm[:], ex[:], mask[:])
                nc.vector.tensor_scalar(out=cw[:, m, :], in0=cwm[:],
                                        scalar1=rec[:, 0:1], scalar2=None,
                                        op0=OP.mult)

                # bucket-local slot: pair prefix(mask) - mask; OOB unrouted
                maskb = gtmp.tile([P, E], BF)
                nc.vector.tensor_copy(maskb[:], mask[:])
                pp = ps_tr.tile([P, E], F32, space="PSUM", name="pp", tag="tr")
                if m % 2 == 0:
                    nc.tensor.matmul(out=pp[:], lhsT=lt_sb[:],
                                     rhs=maskb[:], start=True, stop=True)
                    cnt_ps = ps_tr.tile([1, E], F32, space="PSUM",
                                        name="cntp", tag="tr")
                    nc.tensor.matmul(out=cnt_ps[:], lhsT=onesc_sb[:],
                                     rhs=maskb[:], start=True, stop=True)
                    cntb = gtmp.tile([1, E], BF, name="cntb")
                    nc.scalar.activation(cntb[:], cnt_ps[:], AF.Copy)
                else:
                    nc.tensor.matmul(out=pp[:], lhsT=lt_sb[:],
                                     rhs=maskb[:], start=True, stop=False)
                    nc.tensor.matmul(out=pp[:], lhsT=ones_sb[:],
                                     rhs=cntb[:], start=False, stop=True)
                t1m = gtmp.tile([P, E], F32)
                nc.vector.scalar_tensor_tensor(out=t1m[:], in0=mask[:],
                                               scalar=-1.0, in1=pp[:],
                                               op0=OP.mult, op1=OP.add)
                notm = gtmp.tile([P, E], F32)
                nc.vector.tensor_scalar(out=notm[:], in0=mask[:],
                                        scalar1=-1.0, scalar2=1.0,
                                        op0=OP.mult, op1=OP.add)
                nc.vector.scalar_tensor_tensor(out=posb_all[:, m, :],
                                               in0=notm[:], scalar=OOB,
                                               in1=t1m[:],
                                               op0=OP.mult, op1=OP.add)

            # ---- routed experts over dispatched slots ----
            pre_sh = {}
            for e in range(E):
                capm_e = cfg.capms[e]
                CAPE = NP * capm_e
                b1sb = b13.tile([P, HCN], F32, name="b1sb", tag="b1")
                nc.sync.dma_start(out=b1sb[:], in_=b1a[e])
                b3sb = b13.tile([P, HCN], F32, name="b3sb", tag="b3")
                nc.sync.dma_start(out=b3sb[:], in_=b3a[e])

                # prefetch the first two w2 quarters; they land during L1
                w2qs_pre = []
                for dq in range(2):
                    w2q0 = w2p.tile([P, HCN, 256], BF, name="w2q", tag="w2q")
                    nc.sync.dma_start(
                        out=w2q0[:],
                        in_=w2t[e][:, :, dq * 256:(dq + 1) * 256])
                    w2qs_pre.append(w2q0)

                # one-hot dispatch tiles for all 8 token tiles
                pe_all = pep.tile([P, TT, capm_e], BF, name="pe_all", tag="pe")
                for m in range(TT):
                    nc.vector.tensor_scalar(
                        out=pe_all[:, m, :], in0=iota_sb[:, :capm_e],
                        scalar1=posb_all[:, m, e:e + 1],
                        scalar2=None, op0=OP.is_equal)

                # matmul gather: xeT[k][d, slot] = sum_m x_m^T @ Pe_m
                xeT = xep.tile([P, KD, CAPE], BF, name="xeT", tag="xeT")
                for k in range(KD):
                    gxp, gxt = (ps_l2, "l2") if k % 2 == 0 else (ps_tr, "tr")
                    gx = gxp.tile([P, CAPE], F32, space="PSUM",
                                  name="gx", tag=gxt)
                    for pr in range(NP):
                        for h in range(2):
                            m = 2 * pr + h
                            nc.tensor.matmul(
                                out=gx[:, pr * capm_e:(pr + 1) * capm_e],
                                lhsT=xtok_sb[:, m, k * P:(k + 1) * P],
                                rhs=pe_all[:, m, :],
                                start=(h == 0), stop=(h == 1))
                    nc.vector.tensor_copy(xeT[:, k, :], gx[:])

                if e == E - 1:
                    # last xtok reader just emitted: reload the shared
                    # buffer with d-major x for the tail's shared expert;
                    # the transfer hides under expert 7's L1/L2.
                    xTb = xbig.tile([P, KD, T], BF, name="xTb", tag="xb")
                    nc.scalar.dma_start(out=xTb[:], in_=xtb[:])

                # L1: hT[h, slot] = silu(W1 xe + b1) * (W3 xe + b3)
                hT = hrout.tile([P, HCN, CAPE], BF, name="hT", tag="hT")
                for hc in range(HCN):
                    if e == 0 and hc in pre_w:
                        w1c, w3c = pre_w[hc]
                    else:
                        w1c = w13.tile([P, KD, P], BF, name="w1c", tag="w1c")
                        nc.sync.dma_start(out=w1c[:], in_=w1t[e, hc])
                        w3c = w13.tile([P, KD, P], BF, name="w3c", tag="w3c")
                        nc.gpsimd.dma_start(out=w3c[:], in_=w3t[e, hc])
                    o1 = ps_o1.tile([P, CAPE], F32, space="PSUM",
                                    name="o1", tag="o1")
                    for k in range(KD):
                        nc.tensor.matmul(out=o1[:], lhsT=w1c[:, k, :],
                                         rhs=xeT[:, k, :],
                                         start=(k == 0), stop=(k == KD - 1))
                    s1 = s1p.tile([P, CAPE], F32, name="s1", tag="s1")
                    nc.scalar.activation(s1[:], o1[:], AF.Sigmoid,
                                         bias=b1sb[:, hc:hc + 1])
                    t1 = s1p.tile([P, CAPE], F32, name="t1", tag="t1")
                    nc.vector.scalar_tensor_tensor(
                        out=t1[:], in0=o1[:],
                        scalar=b1sb[:, hc:hc + 1], in1=s1[:],
                        op0=OP.add, op1=OP.mult)
                    o3 = ps_o3.tile([P, CAPE], F32, space="PSUM",
                                    name="o3", tag="o3")
                    for k in range(KD):
                        nc.tensor.matmul(out=o3[:], lhsT=w3c[:, k, :],
                                         rhs=xeT[:, k, :],
                                         start=(k == 0), stop=(k == KD - 1))
                    nc.vector.scalar_tensor_tensor(
                        out=hT[:, hc, :], in0=o3[:],
                        scalar=b3sb[:, hc:hc + 1], in1=t1[:],
                        op0=OP.add, op1=OP.mult)

                # L2 (d-partition orientation) + bias, then transpose to
                # slot-major ye tiles; transposes staggered one dc behind
                # the chains so their yeB reads never stall the PE.
                yeB = yeBp.tile([P, KD, CAPE], BF, name="yeB", tag="yeB")
                pend = []

                def emit_transp(dc, e=e, yeB=yeB, capm_e=capm_e):
                    for pr in range(NP):
                        p2t = ps_tr.tile([P, P], BF, space="PSUM",
                                         name="p2t", tag="tr")
                        nc.tensor.transpose(
                            out=p2t[:capm_e, :],
                            in_=yeB[:, dc, pr * capm_e:(pr + 1) * capm_e],
                            identity=id_sb[:])
                        nc.vector.tensor_copy(
                            ye_sb[0:capm_e, e * NP + pr, dc * P:(dc + 1) * P],
                            p2t[:capm_e, :])

                for dq in range(4):
                    w2q = w2qs_pre[dq]
                    for dc2 in range(2):
                        dc = dq * 2 + dc2
                        pl2 = ps_l2.tile([P, CAPE], F32, space="PSUM",
                                         name="pl2", tag="l2")
                        for hc in range(HCN):
                            nc.tensor.matmul(
                                out=pl2[:],
                                lhsT=w2q[:, hc, dc2 * P:(dc2 + 1) * P],
                                rhs=hT[:, hc, :],
                                start=(hc == 0), stop=(hc == HCN - 1))
                        nc.vector.tensor_scalar(
                            out=yeB[:, dc, :], in0=pl2[:],
                            scalar1=b2c_sb[:, e * KD + dc:e * KD + dc + 1],
                            scalar2=None, op0=OP.add)
                        if pend:
                            emit_transp(pend.pop())
                        pend.append(dc)
                    if dq + 2 < 4:
                        # refill two quarters ahead (this quarter's chains
                        # just freed the buffer, so the queue-head wait is
                        # short; only next-expert w1c prefetches sit behind)
                        w2n = w2p.tile([P, HCN, 256], BF, name="w2q",
                                       tag="w2q")
                        nc.sync.dma_start(
                            out=w2n[:],
                            in_=w2t[e][:, :, (dq + 2) * 256:(dq + 3) * 256])
                        w2qs_pre.append(w2n)
                while pend:
                    emit_transp(pend.pop())

            # ---- tail: shared sub-experts (hidden 512 each) + combine ----
            def emit_shared_l1_unit(s, hcl):
                sv = E + s // 4
                hcg = (s % 4) * HQ + hcl
                if (s, hcl) in pre_sh:
                    w1c, w3c = pre_sh[(s, hcl)]
                else:
                    w1c = w13.tile([P, KD, P], BF, name="w1c", tag="w1c")
                    nc.sync.dma_start(out=w1c[:], in_=w1t[sv, hcg])
                    w3c = w13.tile([P, KD, P], BF, name="w3c", tag="w3c")
                    nc.gpsimd.dma_start(out=w3c[:], in_=w3t[sv, hcg])
                hT_s = hts[s]
                for ft in range(FT):
                    fsl = slice(ft * 512, (ft + 1) * 512)
                    o1 = ps_o1.tile([P, 512], F32, space="PSUM",
                                    name="o1", tag="o1")
                    for k in range(KD):
                        nc.tensor.matmul(out=o1[:], lhsT=w1c[:, k, :],
                                         rhs=xTb[:, k, fsl],
                                         start=(k == 0), stop=(k == KD - 1))
                    s1 = s1p.tile([P, 512], F32, name="s1", tag="s1")
                    nc.scalar.activation(s1[:], o1[:], AF.Sigmoid,
                                         bias=bsh1[s // 4][:, hcg:hcg + 1])
                    t1 = s1p.tile([P, 512], F32, name="t1", tag="t1")
                    nc.vector.scalar_tensor_tensor(
                        out=t1[:], in0=o1[:],
                        scalar=bsh1[s // 4][:, hcg:hcg + 1], in1=s1[:],
                        op0=OP.add, op1=OP.mult)
                    o3 = ps_o3.tile([P, 512], F32, space="PSUM",
                                    name="o3", tag="o3")
                    for k in range(KD):
                        nc.tensor.matmul(out=o3[:], lhsT=w3c[:, k, :],
                                         rhs=xTb[:, k, fsl],
                                         start=(k == 0), stop=(k == KD - 1))
                    nc.vector.scalar_tensor_tensor(
                        out=hT_s[:, hcl, fsl], in0=o3[:],
                        scalar=bsh3[s // 4][:, hcg:hcg + 1], in1=t1[:],
                        op0=OP.add, op1=OP.mult)

            def emit_combine(m):
                pr = m // 2
                p2s_l = []
                for e in range(E):
                    capm_e = cfg.capms[e]
                    pe2 = gtmp.tile([P, capm_e], BF, name="pe2")
                    nc.vector.tensor_scalar(out=pe2[:], in0=iota_sb[:, :capm_e],
                                            scalar1=posb_all[:, m, e:e + 1],
                                            scalar2=None, op0=OP.is_equal)
                    pew = gtmp.tile([P, capm_e], BF, name="pew")
                    nc.vector.tensor_scalar(out=pew[:], in0=pe2[:],
                                            scalar1=cw[:, m, e:e + 1],
                                            scalar2=None, op0=OP.mult)
                    p2c = ps_tr.tile([P, P], BF, space="PSUM",
                                     name="p2c", tag="tr")
                    nc.tensor.transpose(out=p2c[:capm_e, :], in_=pew[:],
                                        identity=id_sb[:])
                    p2s = combp.tile([capm_e, P], BF, name="p2s")
                    nc.scalar.activation(p2s[:], p2c[:capm_e, :], AF.Copy)
                    p2s_l.append(p2s)
                for dt in range(DT):
                    dsl = slice(dt * 512, (dt + 1) * 512)
                    yp = ps_l2.tile([P, 512], F32, space="PSUM",
                                    name="yp", tag="l2")
                    for e in range(E):
                        nc.tensor.matmul(
                            out=yp[:], lhsT=p2s_l[e][:],
                            rhs=ye_sb[0:cfg.capms[e], e * NP + pr, dsl],
                            start=(e == 0), stop=(e == E - 1))
                    nc.vector.tensor_copy(yacc[:, m, dsl], yp[:])

            def emit_shared_l2(s):
                sv = E + s // 4
                hcg0 = (s % 4) * HQ
                w2q = w2p.tile([P, HQ, D], BF, name="w2qs", tag="w2q")
                nc.scalar.dma_start(out=w2q[:],
                                    in_=w2t[sv][:, hcg0:hcg0 + HQ, :])
                hT_s = hts[s]
                for tt in range(TT):
                    tsl = slice(tt * P, (tt + 1) * P)
                    for dt in range(DT):
                        dsl = slice(dt * 512, (dt + 1) * 512)
                        if s >= NSH - 2:
                            # L1 is done by now: o1/o3 banks are free, use a
                            # deeper 3-pool rotation so chains never wait on
                            # the yacc-add evictions
                            pl, tg = [(ps_l2, "l2"), (ps_tr, "tr"),
                                      (ps_o1, "o1")][(tt * DT + dt) % 3]
                        else:
                            pl, tg = ((ps_l2, "l2") if (tt * DT + dt) % 2 == 0
                                      else (ps_tr, "tr"))
                        yp2 = pl.tile([P, 512], F32, space="PSUM",
                                      name="yp2", tag=tg)
                        if s == 0:
                            nc.tensor.matmul(out=yp2[:], lhsT=ones_sb[:],
                                             rhs=b2sh_sb[0:1, dsl],
                                             start=True, stop=False)
                        for hcl in range(HQ):
                            nc.tensor.matmul(
                                out=yp2[:], lhsT=hT_s[:, hcl, tsl],
                                rhs=w2q[:, hcl, dsl],
                                start=(s != 0 and hcl == 0),
                                stop=(hcl == HQ - 1))
                        if s < NSH - 1:
                            nc.vector.tensor_add(yacc[:, tt, dsl],
                                                 yacc[:, tt, dsl], yp2[:])
                        else:
                            yt = ytp.tile([P, 512], BF, name="yt", bufs=4)
                            nc.vector.tensor_add(yt[:], yacc[:, tt, dsl],
                                                 yp2[:])
                            nc.sync.dma_start(out=y[:, tt, dsl], in_=yt[:])

            bsh1 = []
            bsh3 = []
            for sv in range(2):
                b1s = b13.tile([P, HCN], F32, name="b1sh", tag="b1sh")
                nc.sync.dma_start(out=b1s[:], in_=b1a[E + sv])
                b3s = b13.tile([P, HCN], F32, name="b3sh", tag="b3sh")
                nc.sync.dma_start(out=b3s[:], in_=b3a[E + sv])
                bsh1.append(b1s)
                bsh3.append(b3s)

            hts = {}
            for s in range(NSH):
                if s < 2:
                    hts[s] = hshp.tile([P, HQ, T], BF, name=f"hTs{s}",
                                       tag="hTs")
            # interleave first two shared sub-experts' L1 with combine;
            # combine leads: its inputs (ye, cw, pos) are ready at routed
            # end, covering the xTb/w1c arrival for the shared L1
            ci = 0
            for s in range(2):
                for hcl in range(HQ):
                    emit_combine(ci)
                    ci += 1
                    emit_shared_l1_unit(s, hcl)
            # pipeline: L2(s) || L1(s+2)
            for s in range(NSH):
                emit_shared_l2(s)
                if s + 2 < NSH:
                    hts[s + 2] = hshp.tile([P, HQ, T], BF, name=f"hTs{s+2}",
                                           tag="hTs")
                    for hcl in range(HQ):
                        emit_shared_l1_unit(s + 2, hcl)

    nc.compile()
    return nc


# ---------------- host-side packing ----------------

def pack_static(cfg: Cfg, gate_w, gate_b, w1, b1, w2, b2, w3, b3,
                sw1, sb1, sw2, sb2, sw3, sb3):
    D, H, E, NV, n_sh = cfg.D, cfg.H, cfg.E, cfg.NV, cfg.n_sh
    KD, HCN = cfg.KD, cfg.HCN

    w1T = np.transpose(w1, (0, 2, 1))                      # [E, D, H]
    w3T = np.transpose(w3, (0, 2, 1))
    w2T = np.transpose(w2, (0, 2, 1))                      # [E, H, D]
    s1T = sw1.T.reshape(D, n_sh, H).transpose(1, 0, 2)     # [n_sh, D, H]
    s3T = sw3.T.reshape(D, n_sh, H).transpose(1, 0, 2)
    s2T = sw2.T.reshape(n_sh, H, D)                        # [n_sh, H, D]
    w1T_all = np.concatenate([w1T, s1T], 0)                # [NV, D, H]
    w3T_all = np.concatenate([w3T, s3T], 0)
    w2T_all = np.concatenate([w2T, s2T], 0)                # [NV, H, D]

    w1t = np.ascontiguousarray(
        w1T_all.reshape(NV, KD, P, HCN, P).transpose(0, 3, 2, 1, 4)).astype(BF16)
    w3t = np.ascontiguousarray(
        w3T_all.reshape(NV, KD, P, HCN, P).transpose(0, 3, 2, 1, 4)).astype(BF16)
    w2t = np.ascontiguousarray(
        w2T_all.reshape(NV, HCN, P, D).transpose(0, 2, 1, 3)).astype(BF16)

    b1_all = np.concatenate([b1, sb1.reshape(n_sh, H)], 0)  # [NV, H]
    b3_all = np.concatenate([b3, sb3.reshape(n_sh, H)], 0)
    b1a = np.ascontiguousarray(
        b1_all.reshape(NV, HCN, P).transpose(0, 2, 1)).astype(np.float32)
    b3a = np.ascontiguousarray(
        b3_all.reshape(NV, HCN, P).transpose(0, 2, 1)).astype(np.float32)

    b2_all = np.concatenate(
        [b2, sb2[None], np.zeros((n_sh - 1, D), np.float32)], 0)  # [NV, D]
    b2r = b2_all[None].astype(BF16)                         # [1, NV, D]
    # routed b2 in d-partition layout: [P, E*KD], col e*KD+dc = b2[e, dc*128+p]
    b2c = np.ascontiguousarray(
        b2.reshape(E, KD, P).transpose(2, 0, 1).reshape(P, E * KD)
    ).astype(np.float32)

    gwt = np.ascontiguousarray(
        gate_w.T.reshape(KD, P, E).transpose(1, 0, 2)).astype(np.float32)
    gb = gate_b[None].astype(np.float32)
    ones1 = np.ones((1, P), BF16)
    onesc = np.ones((P, 1), BF16)
    lt = np.triu(np.ones((P, P))).astype(BF16)
    ident = np.eye(P).astype(BF16)
    iota = np.tile(np.arange(cfg.capm, dtype=np.float32), (P, 1))

    return dict(w1t=w1t, w3t=w3t, w2t=w2t, b1a=b1a, b3a=b3a, b2r=b2r,
                b2c=b2c, gwt=gwt, gb=gb, ones1=ones1, onesc=onesc,
                lt=lt, ident=ident, iota=iota)


def pack_xtok(cfg: Cfg, x_tokens):
    T, D = x_tokens.shape
    xt = x_tokens.reshape(cfg.TT, P, D).transpose(1, 0, 2)
    return np.ascontiguousarray(xt).astype(BF16)


def pack_xT(cfg: Cfg, x_tokens):
    T, D = x_tokens.shape
    xT = x_tokens.T.reshape(cfg.KD, P, T).transpose(1, 0, 2)
    return np.ascontiguousarray(xT).astype(np.float32)


def unpack_y(cfg: Cfg, y_dev):
    return np.ascontiguousarray(
        y_dev.transpose(1, 0, 2).reshape(cfg.T, cfg.D)).astype(np.float32)


def balance_tokens(xf, gate_w, gate_b, E=8, margin=1):
    """Assign tokens to 256-token buckets so per-(bucket, expert) routed
    counts are near their per-expert means. Returns (perm, capms):
    bucket-major token order and per-expert slot capacities."""
    N = xf.shape[0]
    NB = N // 256
    logits = xf @ gate_w.T + gate_b
    idx = np.argsort(-logits, axis=1)[:, :2]
    tgt = np.zeros(E)
    for e in range(E):
        tgt[e] = ((idx[:, 0] == e) | (idx[:, 1] == e)).sum() / NB
    tgt = np.maximum(tgt, 1.0)
    cnt = np.zeros((NB, E), np.float64)
    fill = np.zeros(NB, np.int64)
    assign = np.empty(N, np.int32)
    rng = np.random.RandomState(0)
    BIG = 1 << 40
    for t in rng.permutation(N):
        a, b = idx[t]
        s = np.maximum((cnt[:, a] + 1) / tgt[a],
                       (cnt[:, b] + 1) / tgt[b]) * 4096 + fill
        s[fill >= 256] = BIG
        bb = int(np.argmin(s))
        assign[t] = bb
        cnt[bb, a] += 1
        cnt[bb, b] += 1
        fill[bb] += 1
    perm = np.argsort(assign.astype(np.int64) * N + np.arange(N))
    capms = tuple(int(c) + margin for c in cnt.max(0))
    return perm, capms


_CACHE = {}


def _get_nc(cfg: Cfg):
    key = (cfg.D, cfg.H, cfg.E, cfg.n_sh, cfg.T, cfg.capms)
    if key not in _CACHE:
        _CACHE[key] = build_nc_v2(cfg)
    return _CACHE[key]


def plan_cfg(inputs):
    """Balance tokens from the actual routing; returns (cfg, perm)."""
    x = np.asarray(inputs["x"], np.float32)
    B, S, D = x.shape
    xf = x.reshape(-1, D)
    perm, capms = balance_tokens(
        xf, np.asarray(inputs["gate_w"], np.float32),
        np.asarray(inputs["gate_b"], np.float32))
    cfg = Cfg(D=D, T=(B * S) // 8, n_cores=8, capms=capms)
    return cfg, perm


def make_in_maps(cfg: Cfg, inputs, perm):
    static = pack_static(
        cfg,
        np.asarray(inputs["gate_w"], np.float32), np.asarray(inputs["gate_b"], np.float32),
        np.asarray(inputs["w1"], np.float32), np.asarray(inputs["b1"], np.float32),
        np.asarray(inputs["w2"], np.float32), np.asarray(inputs["b2"], np.float32),
        np.asarray(inputs["w3"], np.float32), np.asarray(inputs["b3"], np.float32),
        np.asarray(inputs["sw1"], np.float32), np.asarray(inputs["sb1"], np.float32),
        np.asarray(inputs["sw2"], np.float32), np.asarray(inputs["sb2"], np.float32),
        np.asarray(inputs["sw3"], np.float32), np.asarray(inputs["sb3"], np.float32),
    )
    x = np.asarray(inputs["x"], np.float32)
    B, S, D = x.shape
    xp = x.reshape(-1, D)[perm]
    in_maps = []
    for c in range(cfg.n_cores):
        mm = dict(static)
        xc = xp[c * cfg.T:(c + 1) * cfg.T]
        mm["xT"] = pack_xT(cfg, xc)
        mm["xtok"] = pack_xtok(cfg, xc)
        mm["xtb"] = mm["xT"].astype(BF16)
        in_maps.append(mm)
    return in_maps


def kernel(**inputs) -> np.ndarray:
    x = np.asarray(inputs["x"], np.float32)
    B, S, D = x.shape
    cfg, perm = plan_cfg(inputs)
    nc = _get_nc(cfg)
    in_maps = make_in_maps(cfg, inputs, perm)
    res = run_bass_kernel_spmd(nc, in_maps, list(range(cfg.n_cores)))
    yp = np.concatenate(
        [unpack_y(cfg, res.results[c]["y"]) for c in range(cfg.n_cores)], 0)
    out = np.empty_like(yp)
    out[perm] = yp
    return out.reshape(B, S, D)


# revision 76
# speedup vs baseline: 1.2615x; 1.0086x over previous
"""MoE (8 routed experts, top-2, + shared expert) on 8 NeuronCores.

Data-parallel over tokens (1024/core), weights replicated. The host
load-balances the token->bucket assignment (any sharding is allowed) so
each (256-token bucket, expert) routed count sits at its per-expert
mean, letting the capacity-dispatched kernel run with per-expert
capacities capm_e = max bucket count + 2 (60..74 on this routing)
instead of the binomial-tail uniform 96.

Device kernel (per core):
  1. Gate in fp32 (matches reference routing bit-for-bit for the
     observed >=1.7e-4 top-2/3 logit gaps), renormalized top-2 combine
     weights cw, and bucket-local slot positions via triangular-matmul
     prefix sums.
  2. Routed experts e=0..7: one-hot matmul gather of x into CAPE=296
     slots, SwiGLU L1 (feature-major, free dim = slots), L2 in
     d-partition orientation (out[d, slot], bias via activation), then
     PE transposes to slot-major ye tiles held in SBUF.
  3. Tail: shared expert (8 sub-experts of hidden 512) interleaved with
     the scatter-combine (transposed scaled one-hots x ye), everything
     accumulating into a token-major f32 yacc; last shared sub-expert's
     L2 fuses the final add and streams y out.

Matmuls are bf16 with fp32 accumulation; weight DMA is split across the
SP/Pool/Act queues to avoid head-of-line blocking on one DMA queue.
"""

import numpy as np
import ml_dtypes

import concourse.bacc as bacc
import concourse.bass as bass
import concourse.tile as tile
import concourse.mybir as mybir
from concourse.bass_utils import run_bass_kernel_spmd

BF16 = ml_dtypes.bfloat16
F32 = mybir.dt.float32
BF = mybir.dt.bfloat16
AF = mybir.ActivationFunctionType
OP = mybir.AluOpType

P = 128


class Cfg:
    def __init__(self, D=1024, H=2048, E=8, n_sh=2, T=1024, n_cores=8,
                 capms=(74,) * 8):
        self.D, self.H, self.E, self.n_sh, self.T = D, H, E, n_sh, T
        self.NV = E + n_sh          # packed weight rows (8 routed + 2 shared)
        self.HS = n_sh * H          # shared hidden total (4096)
        self.KD = D // P            # contraction chunks over D
        self.HCN = H // P           # h chunks per packed VE
        self.TT = T // P            # token 128-tiles per core
        self.FT = T // 512          # shared L1 free 512-tiles
        self.DT = D // 512          # 512-wide d tiles
        self.n_cores = n_cores
        self.capms = tuple(capms)   # slots per (256-token bucket, expert)
        self.capm = max(self.capms)  # iota / tile sizing width
        self.NP = self.TT // 2      # buckets per core (pair of tiles)
        self.NSH = 8                # shared sub-experts
        self.HQ = (self.HS // P) // self.NSH  # h-chunks per sub-expert (4)


def build_nc_v2(cfg: Cfg):
    D, H, E, T = cfg.D, cfg.H, cfg.E, cfg.T
    KD, HCN, TT, FT, DT = cfg.KD, cfg.HCN, cfg.TT, cfg.FT, cfg.DT
    capm, NP = cfg.capm, cfg.NP
    NSH, HQ = cfg.NSH, cfg.HQ

    nc = bacc.Bacc("TRN2", target_bir_lowering=False)

    xT = nc.dram_tensor("xT", [P, KD, T], F32, kind="ExternalInput")
    xtok = nc.dram_tensor("xtok", [P, TT, D], BF, kind="ExternalInput")
    xtb = nc.dram_tensor("xtb", [P, KD, T], BF, kind="ExternalInput")
    w1t = nc.dram_tensor("w1t", [cfg.NV, HCN, P, KD, P], BF, kind="ExternalInput")
    w3t = nc.dram_tensor("w3t", [cfg.NV, HCN, P, KD, P], BF, kind="ExternalInput")
    w2t = nc.dram_tensor("w2t", [cfg.NV, P, HCN, D], BF, kind="ExternalInput")
    b1a = nc.dram_tensor("b1a", [cfg.NV, P, HCN], F32, kind="ExternalInput")
    b3a = nc.dram_tensor("b3a", [cfg.NV, P, HCN], F32, kind="ExternalInput")
    b2r = nc.dram_tensor("b2r", [1, cfg.NV, D], BF, kind="ExternalInput")
    b2c = nc.dram_tensor("b2c", [P, E * KD], F32, kind="ExternalInput")
    gwt = nc.dram_tensor("gwt", [P, KD, E], F32, kind="ExternalInput")
    gb = nc.dram_tensor("gb", [1, E], F32, kind="ExternalInput")
    ones1 = nc.dram_tensor("ones1", [1, P], BF, kind="ExternalInput")
    onesc = nc.dram_tensor("onesc", [P, 1], BF, kind="ExternalInput")
    lt = nc.dram_tensor("lt", [P, P], BF, kind="ExternalInput")
    ident = nc.dram_tensor("ident", [P, P], BF, kind="ExternalInput")
    iota = nc.dram_tensor("iota", [P, capm], F32, kind="ExternalInput")
    y = nc.dram_tensor("y", [P, TT, D], BF, kind="ExternalOutput")

    OOB = 3.0e6

    from contextlib import ExitStack
    with tile.TileContext(nc) as tc:
        with ExitStack() as stack:
            pool_specs = dict(
                const1=dict(bufs=1), xbig=dict(bufs=1),
                gchunk=dict(bufs=2), gtmp=dict(bufs=4),
                w13=dict(bufs=3), w2p=dict(bufs=2), b13=dict(bufs=2),
                xep=dict(bufs=1), hrout=dict(bufs=1), hshp=dict(bufs=2),
                yeBp=dict(bufs=1), pep=dict(bufs=2), s1p=dict(bufs=2),
                combp=dict(bufs=8), ytp=dict(bufs=2),
                ps_o1=dict(bufs=2, space="PSUM"),
                ps_o3=dict(bufs=2, space="PSUM"),
                ps_l2=dict(bufs=2, space="PSUM"),
                ps_tr=dict(bufs=2, space="PSUM"),
            )
            pools = {n: stack.enter_context(tc.tile_pool(name=n, **kw))
                     for n, kw in pool_specs.items()}
            (const1, xbig, gchunk, gtmp, w13, w2p, b13, xep, hrout,
             hshp, yeBp, pep, s1p, combp, ytp, ps_o1, ps_o3, ps_l2,
             ps_tr) = (
                pools[n] for n in (
                    "const1", "xbig", "gchunk", "gtmp", "w13", "w2p", "b13",
                    "xep", "hrout", "hshp", "yeBp", "pep", "s1p",
                    "combp", "ytp", "ps_o1", "ps_o3", "ps_l2", "ps_tr"))
            # ---- resident state ----
            # xtok_sb and xTb share one 2MB buffer (tag xb): xtok is dead
            # after the last gather; xTb is DMA'd into the same space then.
            xtok_sb = xbig.tile([P, TT, D], BF, name="xtok_sb", tag="xb")
            yacc = const1.tile([P, TT, D], F32)
            cw = const1.tile([P, TT, E], F32)
            posb_all = const1.tile([P, TT, E], F32)
            ye_sb = const1.tile([P, E * NP, D], BF)
            gwt_sb = const1.tile([P, KD, E], F32)
            gb_sb = const1.tile([1, E], F32)
            ones_sb = const1.tile([1, P], BF)
            onesc_sb = const1.tile([P, 1], BF)
            lt_sb = const1.tile([P, P], BF)
            id_sb = const1.tile([P, P], BF)
            iota_sb = const1.tile([P, capm], F32)
            b2c_sb = const1.tile([P, E * KD], F32)
            b2sh_sb = const1.tile([1, D], BF)
            zerob = const1.tile([P, 1], F32)
            onesf = const1.tile([1, P], F32)

            # first two gate tiles + gate weights lead the DMA queues so
            # the gate starts without sitting behind the bulk prologue
            xc_pre = {}
            for m in range(2):
                xc = gchunk.tile([P, KD, P], F32, name="xchunk")
                nc.sync.dma_start(out=xc[:], in_=xT[:, :, m * P:(m + 1) * P])
                xc_pre[m] = xc
            nc.sync.dma_start(out=gwt_sb[:], in_=gwt[:])
            nc.sync.dma_start(out=gb_sb[:], in_=gb[:])
            # secondary consts flow on the Act queue in parallel so the
            # gate's per-tile xchunk stream on sync isn't delayed
            nc.scalar.dma_start(out=ones_sb[:], in_=ones1[:])
            nc.scalar.dma_start(out=onesc_sb[:], in_=onesc[:])
            nc.scalar.dma_start(out=lt_sb[:], in_=lt[:])
            nc.scalar.dma_start(out=id_sb[:], in_=ident[:])
            nc.scalar.dma_start(out=iota_sb[:], in_=iota[:])
            nc.scalar.dma_start(out=b2c_sb[:], in_=b2c[:])
            nc.scalar.dma_start(out=b2sh_sb[:], in_=b2r[0:1, E, :])
            nc.vector.memset(zerob[:], 0.0)
            nc.vector.memset(onesf[:], 1.0)

            # prefetch expert 0's first L1 weight chunks
            pre_w = {}
            for hc in range(3):
                w1c = w13.tile([P, KD, P], BF, name="w1c", tag="w1c")
                nc.sync.dma_start(out=w1c[:], in_=w1t[0, hc])
                w3c = w13.tile([P, KD, P], BF, name="w3c", tag="w3c")
                nc.gpsimd.dma_start(out=w3c[:], in_=w3t[0, hc])
                pre_w[hc] = (w1c, w3c)
            # xtok hands off to the DMA engines late (pool-queue tail) so
            # its 2MB transfer neither starves the small gate-const loads
            # nor blocks the per-tile gate xchunk stream on sync; it is
            # first needed by expert 0's gather, well after the gate.
            nc.gpsimd.dma_start(out=xtok_sb[:], in_=xtok[:])

            # ---- gate + bucket positions, per 128-token tile ----
            cntb = None
            for m in range(TT):
                if m in xc_pre:
                    xchunk = xc_pre[m]
                else:
                    xchunk = gchunk.tile([P, KD, P], F32)
                    nc.sync.dma_start(out=xchunk[:],
                                      in_=xT[:, :, m * P:(m + 1) * P])

                pg = ps_l2.tile([P, E], F32, space="PSUM", name="pg", tag="l2")
                for k in range(KD):
                    nc.tensor.matmul(out=pg[:], lhsT=xchunk[:, k, :],
                                     rhs=gwt_sb[:, k, :],
                                     start=(k == 0), stop=False)
                nc.tensor.matmul(out=pg[:], lhsT=onesf[:], rhs=gb_sb[:],
                                 start=False, stop=True)

                lg = gtmp.tile([P, E], F32)
                nc.scalar.activation(lg[:], pg[:], AF.Copy)
                m8 = gtmp.tile([P, 8], F32)
                nc.vector.max(m8[:], lg[:])
                ex = gtmp.tile([P, E], F32)
                nc.vector.tensor_scalar(out=ex[:], in0=lg[:],
                                        scalar1=m8[:, 0:1], scalar2=None,
                                        op0=OP.subtract)
                nc.scalar.activation(ex[:], ex[:], AF.Exp, bias=zerob[:])
                mask = gtmp.tile([P, E], F32)
                nc.vector.tensor_scalar(out=mask[:], in0=lg[:],
                                        scalar1=m8[:, 1:2], scalar2=None,
                                        op0=OP.is_ge)
                e2 = gtmp.tile([P, 1], F32)
                nc.vector.tensor_tensor(out=e2[:], in0=m8[:, 1:2],
                                        in1=m8[:, 0:1], op=OP.subtract)
                nc.scalar.activation(e2[:], e2[:], AF.Exp, bias=zerob[:])
                den = gtmp.tile([P, 1], F32)
                nc.vector.tensor_scalar(out=den[:], in0=e2[:], scalar1=1.0,
                                        scalar2=None, op0=OP.add)
                rec = gtmp.tile([P, 1], F32)
                nc.vector.reciprocal(rec[:], den[:])
                cwm = gtmp.tile([P, E], F32)
                nc.vector.tensor_mul(cwm[:], ex[:], mask[:])
                nc.vector.tensor_scalar(out=cw[:, m, :], in0=cwm[:],
                                        scalar1=rec[:, 0:1], scalar2=None,
                                        op0=OP.mult)

                # bucket-local slot: pair prefix(mask) - mask; OOB unrouted
                maskb = gtmp.tile([P, E], BF)
                nc.vector.tensor_copy(maskb[:], mask[:])
                pp = ps_tr.tile([P, E], F32, space="PSUM", name="pp", tag="tr")
                if m % 2 == 0:
                    nc.tensor.matmul(out=pp[:], lhsT=lt_sb[:],
                                     rhs=maskb[:], start=True, stop=True)
                    cnt_ps = ps_tr.tile([1, E], F32, space="PSUM",
                                        name="cntp", tag="tr")
                    nc.tensor.matmul(out=cnt_ps[:], lhsT=onesc_sb[:],
                                     rhs=maskb[:], start=True, stop=True)
                    cntb = gtmp.tile([1, E], BF, name="cntb")
                    nc.scalar.activation(cntb[:], cnt_ps[:], AF.Copy)
                else:
                    nc.tensor.matmul(out=pp[:], lhsT=lt_sb[:],
                                     rhs=maskb[:], start=True, stop=False)
                    nc.tensor.matmul(out=pp[:], lhsT=ones_sb[:],
                                     rhs=cntb[:], start=False, stop=True)
                t1m = gtmp.tile([P, E], F32)
                nc.vector.scalar_tensor_tensor(out=t1m[:], in0=mask[:],
                                               scalar=-1.0, in1=pp[:],
                                               op0=OP.mult, op1=OP.add)
                notm = gtmp.tile([P, E], F32)
                nc.vector.tensor_scalar(out=notm[:], in0=mask[:],
                                        scalar1=-1.0, scalar2=1.0,
                                        op0=OP.mult, op1=OP.add)
                nc.vector.scalar_tensor_tensor(out=posb_all[:, m, :],
                                               in0=notm[:], scalar=OOB,
                                               in1=t1m[:],
                                               op0=OP.mult, op1=OP.add)

            # combine groups: stack pairs of experts with capm <= 64 into
            # one K<=128 chunk (second member at partition base 64 — PE
            # writes only allow bases 0/32/64). Gap rows are zeroed once.
            small = [e for e in range(E) if cfg.capms[e] <= 64]
            big = [e for e in range(E) if cfg.capms[e] > 64]
            groups = []
            for i in range(0, len(small) - 1, 2):
                groups.append((small[i], small[i + 1]))
            if len(small) % 2:
                groups.append((small[-1],))
            groups.extend((e,) for e in big)
            home = {}
            yoff = {}
            gap_zero = []
            for g in groups:
                for i, e in enumerate(g):
                    home[e] = g[0]
                    yoff[e] = 64 * i
                if len(g) == 2 and cfg.capms[g[0]] < 64:
                    gap_zero.append((cfg.capms[g[0]], g[0]))
            grp_k = {g: (64 + cfg.capms[g[1]] if len(g) == 2
                         else cfg.capms[g[0]]) for g in groups}

            # zero the ye/one-hot gap rows [capm_a, 64) of paired tiles so
            # the stacked K=64+capm_b combine chains read zeros there
            for cap_a, hm in gap_zero:
                nc.vector.memset(
                    ye_sb[cap_a:64, hm * NP:(hm + 1) * NP, :], 0.0)

            # ---- routed experts over dispatched slots ----
            pre_sh = {}
            for e in range(E):
                capm_e = cfg.capms[e]
                CAPE = NP * capm_e
                b1sb = b13.tile([P, HCN], F32, name="b1sb", tag="b1")
                nc.sync.dma_start(out=b1sb[:], in_=b1a[e])
                b3sb = b13.tile([P, HCN], F32, name="b3sb", tag="b3")
                nc.sync.dma_start(out=b3sb[:], in_=b3a[e])

                # prefetch the first two w2 quarters; they land during L1
                w2qs_pre = []
                for dq in range(2):
                    w2q0 = w2p.tile([P, HCN, 256], BF, name="w2q", tag="w2q")
                    nc.sync.dma_start(
                        out=w2q0[:],
                        in_=w2t[e][:, :, dq * 256:(dq + 1) * 256])
                    w2qs_pre.append(w2q0)

                # one-hot dispatch tiles for all 8 token tiles
                pe_all = pep.tile([P, TT, capm_e], BF, name="pe_all", tag="pe")
                for m in range(TT):
                    nc.vector.tensor_scalar(
                        out=pe_all[:, m, :], in0=iota_sb[:, :capm_e],
                        scalar1=posb_all[:, m, e:e + 1],
                        scalar2=None, op0=OP.is_equal)

                # matmul gather: xeT[k][d, slot] = sum_m x_m^T @ Pe_m
                xeT = xep.tile([P, KD, CAPE], BF, name="xeT", tag="xeT")
                for k in range(KD):
                    gxp, gxt = (ps_l2, "l2") if k % 2 == 0 else (ps_tr, "tr")
                    gx = gxp.tile([P, CAPE], F32, space="PSUM",
                                  name="gx", tag=gxt)
                    for pr in range(NP):
                        for h in range(2):
                            m = 2 * pr + h
                            nc.tensor.matmul(
                                out=gx[:, pr * capm_e:(pr + 1) * capm_e],
                                lhsT=xtok_sb[:, m, k * P:(k + 1) * P],
                                rhs=pe_all[:, m, :],
                                start=(h == 0), stop=(h == 1))
                    nc.vector.tensor_copy(xeT[:, k, :], gx[:])

                if e == E - 1:
                    # last xtok reader just emitted: reload the shared
                    # buffer with d-major x for the tail's shared expert;
                    # the transfer hides under expert 7's L1/L2.
                    xTb = xbig.tile([P, KD, T], BF, name="xTb", tag="xb")
                    nc.scalar.dma_start(out=xTb[:], in_=xtb[:])

                # L1: hT[h, slot] = silu(W1 xe + b1) * (W3 xe + b3)
                hT = hrout.tile([P, HCN, CAPE], BF, name="hT", tag="hT")
                for hc in range(HCN):
                    if e == 0 and hc in pre_w:
                        w1c, w3c = pre_w[hc]
                    else:
                        w1c = w13.tile([P, KD, P], BF, name="w1c", tag="w1c")
                        nc.sync.dma_start(out=w1c[:], in_=w1t[e, hc])
                        w3c = w13.tile([P, KD, P], BF, name="w3c", tag="w3c")
                        nc.gpsimd.dma_start(out=w3c[:], in_=w3t[e, hc])
                    o1 = ps_o1.tile([P, CAPE], F32, space="PSUM",
                                    name="o1", tag="o1")
                    for k in range(KD):
                        nc.tensor.matmul(out=o1[:], lhsT=w1c[:, k, :],
                                         rhs=xeT[:, k, :],
                                         start=(k == 0), stop=(k == KD - 1))
                    s1 = s1p.tile([P, CAPE], F32, name="s1", tag="s1")
                    nc.scalar.activation(s1[:], o1[:], AF.Sigmoid,
                                         bias=b1sb[:, hc:hc + 1])
                    t1 = s1p.tile([P, CAPE], F32, name="t1", tag="t1")
                    nc.vector.scalar_tensor_tensor(
                        out=t1[:], in0=o1[:],
                        scalar=b1sb[:, hc:hc + 1], in1=s1[:],
                        op0=OP.add, op1=OP.mult)
                    o3 = ps_o3.tile([P, CAPE], F32, space="PSUM",
                                    name="o3", tag="o3")
                    for k in range(KD):
                        nc.tensor.matmul(out=o3[:], lhsT=w3c[:, k, :],
                                         rhs=xeT[:, k, :],
                                         start=(k == 0), stop=(k == KD - 1))
                    nc.vector.scalar_tensor_tensor(
                        out=hT[:, hc, :], in0=o3[:],
                        scalar=b3sb[:, hc:hc + 1], in1=t1[:],
                        op0=OP.add, op1=OP.mult)

                # L2 (d-partition orientation) + bias, then transpose to
                # slot-major ye tiles; transposes staggered one dc behind
                # the chains so their yeB reads never stall the PE.
                yeB = yeBp.tile([P, KD, CAPE], BF, name="yeB", tag="yeB")
                pend = []

                def emit_transp(dc, e=e, yeB=yeB, capm_e=capm_e):
                    o = yoff[e]
                    tgt = home[e] * NP
                    for pr in range(NP):
                        p2t = ps_tr.tile([P, P], BF, space="PSUM",
                                         name="p2t", tag="tr")
                        nc.tensor.transpose(
                            out=p2t[o:o + capm_e, :],
                            in_=yeB[:, dc, pr * capm_e:(pr + 1) * capm_e],
                            identity=id_sb[:])
                        nc.vector.tensor_copy(
                            ye_sb[o:o + capm_e, tgt + pr, dc * P:(dc + 1) * P],
                            p2t[o:o + capm_e, :])

                for dq in range(4):
                    w2q = w2qs_pre[dq]
                    for dc2 in range(2):
                        dc = dq * 2 + dc2
                        pl2 = ps_l2.tile([P, CAPE], F32, space="PSUM",
                                         name="pl2", tag="l2")
                        for hc in range(HCN):
                            nc.tensor.matmul(
                                out=pl2[:],
                                lhsT=w2q[:, hc, dc2 * P:(dc2 + 1) * P],
                                rhs=hT[:, hc, :],
                                start=(hc == 0), stop=(hc == HCN - 1))
                        nc.vector.tensor_scalar(
                            out=yeB[:, dc, :], in0=pl2[:],
                            scalar1=b2c_sb[:, e * KD + dc:e * KD + dc + 1],
                            scalar2=None, op0=OP.add)
                        if pend:
                            emit_transp(pend.pop())
                        pend.append(dc)
                    if dq + 2 < 4:
                        # refill two quarters ahead (this quarter's chains
                        # just freed the buffer, so the queue-head wait is
                        # short; only next-expert w1c prefetches sit behind)
                        w2n = w2p.tile([P, HCN, 256], BF, name="w2q",
                                       tag="w2q")
                        nc.sync.dma_start(
                            out=w2n[:],
                            in_=w2t[e][:, :, (dq + 2) * 256:(dq + 3) * 256])
                        w2qs_pre.append(w2n)
                while pend:
                    emit_transp(pend.pop())

            # ---- tail: shared sub-experts (hidden 512 each) + combine ----
            def emit_shared_l1_unit(s, hcl):
                sv = E + s // 4
                hcg = (s % 4) * HQ + hcl
                if (s, hcl) in pre_sh:
                    w1c, w3c = pre_sh[(s, hcl)]
                else:
                    w1c = w13.tile([P, KD, P], BF, name="w1c", tag="w1c")
                    nc.sync.dma_start(out=w1c[:], in_=w1t[sv, hcg])
                    w3c = w13.tile([P, KD, P], BF, name="w3c", tag="w3c")
                    nc.gpsimd.dma_start(out=w3c[:], in_=w3t[sv, hcg])
                hT_s = hts[s]
                for ft in range(FT):
                    fsl = slice(ft * 512, (ft + 1) * 512)
                    o1 = ps_o1.tile([P, 512], F32, space="PSUM",
                                    name="o1", tag="o1")
                    for k in range(KD):
                        nc.tensor.matmul(out=o1[:], lhsT=w1c[:, k, :],
                                         rhs=xTb[:, k, fsl],
                                         start=(k == 0), stop=(k == KD - 1))
                    s1 = s1p.tile([P, 512], F32, name="s1", tag="s1")
                    nc.scalar.activation(s1[:], o1[:], AF.Sigmoid,
                                         bias=bsh1[s // 4][:, hcg:hcg + 1])
                    t1 = s1p.tile([P, 512], F32, name="t1", tag="t1")
                    nc.vector.scalar_tensor_tensor(
                        out=t1[:], in0=o1[:],
                        scalar=bsh1[s // 4][:, hcg:hcg + 1], in1=s1[:],
                        op0=OP.add, op1=OP.mult)
                    o3 = ps_o3.tile([P, 512], F32, space="PSUM",
                                    name="o3", tag="o3")
                    for k in range(KD):
                        nc.tensor.matmul(out=o3[:], lhsT=w3c[:, k, :],
                                         rhs=xTb[:, k, fsl],
                                         start=(k == 0), stop=(k == KD - 1))
                    nc.vector.scalar_tensor_tensor(
                        out=hT_s[:, hcl, fsl], in0=o3[:],
                        scalar=bsh3[s // 4][:, hcg:hcg + 1], in1=t1[:],
                        op0=OP.add, op1=OP.mult)

            def emit_combine(m):
                pr = m // 2
                p2s_l = []
                for g in groups:
                    kk = grp_k[g]
                    p2c = ps_tr.tile([P, P], BF, space="PSUM",
                                     name="p2c", tag="tr")
                    for e in g:
                        capm_e = cfg.capms[e]
                        o = yoff[e]
                        pe2 = gtmp.tile([P, capm_e], BF, name="pe2")
                        nc.vector.tensor_scalar(
                            out=pe2[:], in0=iota_sb[:, :capm_e],
                            scalar1=posb_all[:, m, e:e + 1],
                            scalar2=None, op0=OP.is_equal)
                        pew = gtmp.tile([P, capm_e], BF, name="pew")
                        nc.vector.tensor_scalar(out=pew[:], in0=pe2[:],
                                                scalar1=cw[:, m, e:e + 1],
                                                scalar2=None, op0=OP.mult)
                        nc.tensor.transpose(out=p2c[o:o + capm_e, :],
                                            in_=pew[:], identity=id_sb[:])
                    p2s = combp.tile([kk, P], BF, name="p2s")
                    nc.scalar.activation(p2s[:], p2c[:kk, :], AF.Copy)
                    if len(g) == 2 and cfg.capms[g[0]] < 64:
                        # rows [capm_a, 64) came from uninitialized PSUM;
                        # zero them (their ye rows are zero too, but NaN
                        # garbage would still poison the product)
                        nc.vector.memset(p2s[cfg.capms[g[0]]:64, :], 0.0)
                    p2s_l.append((p2s, kk))
                for dt in range(DT):
                    dsl = slice(dt * 512, (dt + 1) * 512)
                    yp = ps_l2.tile([P, 512], F32, space="PSUM",
                                    name="yp", tag="l2")
                    for i, g in enumerate(groups):
                        p2s, kk = p2s_l[i]
                        nc.tensor.matmul(
                            out=yp[:], lhsT=p2s[:],
                            rhs=ye_sb[0:kk, home[g[0]] * NP + pr, dsl],
                            start=(i == 0), stop=(i == len(groups) - 1))
                    nc.vector.tensor_copy(yacc[:, m, dsl], yp[:])

            def emit_shared_l2(s):
                sv = E + s // 4
                hcg0 = (s % 4) * HQ
                w2q = w2p.tile([P, HQ, D], BF, name="w2qs", tag="w2q")
                nc.scalar.dma_start(out=w2q[:],
                                    in_=w2t[sv][:, hcg0:hcg0 + HQ, :])
                hT_s = hts[s]
                for tt in range(TT):
                    tsl = slice(tt * P, (tt + 1) * P)
                    for dt in range(DT):
                        dsl = slice(dt * 512, (dt + 1) * 512)
                        if s >= NSH - 2:
                            # L1 is done by now: o1/o3 banks are free, use a
                            # deeper 3-pool rotation so chains never wait on
                            # the yacc-add evictions
                            pl, tg = [(ps_l2, "l2"), (ps_tr, "tr"),
                                      (ps_o1, "o1")][(tt * DT + dt) % 3]
                        else:
                            pl, tg = ((ps_l2, "l2") if (tt * DT + dt) % 2 == 0
                                      else (ps_tr, "tr"))
                        yp2 = pl.tile([P, 512], F32, space="PSUM",
                                      name="yp2", tag=tg)
                        if s == 0:
                            nc.tensor.matmul(out=yp2[:], lhsT=ones_sb[:],
                                             rhs=b2sh_sb[0:1, dsl],
                                             start=True, stop=False)
                        for hcl in range(HQ):
                            nc.tensor.matmul(
                                out=yp2[:], lhsT=hT_s[:, hcl, tsl],
                                rhs=w2q[:, hcl, dsl],
                                start=(s != 0 and hcl == 0),
                                stop=(hcl == HQ - 1))
                        if s < NSH - 1:
                            nc.vector.tensor_add(yacc[:, tt, dsl],
                                                 yacc[:, tt, dsl], yp2[:])
                        else:
                            yt = ytp.tile([P, 512], BF, name="yt", bufs=4)
                            nc.vector.tensor_add(yt[:], yacc[:, tt, dsl],
                                                 yp2[:])
                            nc.sync.dma_start(out=y[:, tt, dsl], in_=yt[:])

            bsh1 = []
            bsh3 = []
            for sv in range(2):
                b1s = b13.tile([P, HCN], F32, name="b1sh", tag="b1sh")
                nc.sync.dma_start(out=b1s[:], in_=b1a[E + sv])
                b3s = b13.tile([P, HCN], F32, name="b3sh", tag="b3sh")
                nc.sync.dma_start(out=b3s[:], in_=b3a[E + sv])
                bsh1.append(b1s)
                bsh3.append(b3s)

            hts = {}
            for s in range(NSH):
                if s < 2:
                    hts[s] = hshp.tile([P, HQ, T], BF, name=f"hTs{s}",
                                       tag="hTs")
            # interleave first two shared sub-experts' L1 with combine;
            # combine leads: its inputs (ye, cw, pos) are ready at routed
            # end, covering the xTb/w1c arrival for the shared L1
            ci = 0
            for s in range(2):
                for hcl in range(HQ):
                    emit_combine(ci)
                    ci += 1
                    emit_shared_l1_unit(s, hcl)
            # pipeline: L2(s) || L1(s+2)
            for s in range(NSH):
                emit_shared_l2(s)
                if s + 2 < NSH:
                    hts[s + 2] = hshp.tile([P, HQ, T], BF, name=f"hTs{s+2}",
                                           tag="hTs")
                    for hcl in range(HQ):
                        emit_shared_l1_unit(s + 2, hcl)

    nc.compile()
    return nc


# ---------------- host-side packing ----------------

def pack_static(cfg: Cfg, gate_w, gate_b, w1, b1, w2, b2, w3, b3,
                sw1, sb1, sw2, sb2, sw3, sb3):
    D, H, E, NV, n_sh = cfg.D, cfg.H, cfg.E, cfg.NV, cfg.n_sh
    KD, HCN = cfg.KD, cfg.HCN

    w1T = np.transpose(w1, (0, 2, 1))                      # [E, D, H]
    w3T = np.transpose(w3, (0, 2, 1))
    w2T = np.transpose(w2, (0, 2, 1))                      # [E, H, D]
    s1T = sw1.T.reshape(D, n_sh, H).transpose(1, 0, 2)     # [n_sh, D, H]
    s3T = sw3.T.reshape(D, n_sh, H).transpose(1, 0, 2)
    s2T = sw2.T.reshape(n_sh, H, D)                        # [n_sh, H, D]
    w1T_all = np.concatenate([w1T, s1T], 0)                # [NV, D, H]
    w3T_all = np.concatenate([w3T, s3T], 0)
    w2T_all = np.concatenate([w2T, s2T], 0)                # [NV, H, D]

    w1t = np.ascontiguousarray(
        w1T_all.reshape(NV, KD, P, HCN, P).transpose(0, 3, 2, 1, 4)).astype(BF16)
    w3t = np.ascontiguousarray(
        w3T_all.reshape(NV, KD, P, HCN, P).transpose(0, 3, 2, 1, 4)).astype(BF16)
    w2t = np.ascontiguousarray(
        w2T_all.reshape(NV, HCN, P, D).transpose(0, 2, 1, 3)).astype(BF16)

    b1_all = np.concatenate([b1, sb1.reshape(n_sh, H)], 0)  # [NV, H]
    b3_all = np.concatenate([b3, sb3.reshape(n_sh, H)], 0)
    b1a = np.ascontiguousarray(
        b1_all.reshape(NV, HCN, P).transpose(0, 2, 1)).astype(np.float32)
    b3a = np.ascontiguousarray(
        b3_all.reshape(NV, HCN, P).transpose(0, 2, 1)).astype(np.float32)

    b2_all = np.concatenate(
        [b2, sb2[None], np.zeros((n_sh - 1, D), np.float32)], 0)  # [NV, D]
    b2r = b2_all[None].astype(BF16)                         # [1, NV, D]
    # routed b2 in d-partition layout: [P, E*KD], col e*KD+dc = b2[e, dc*128+p]
    b2c = np.ascontiguousarray(
        b2.reshape(E, KD, P).transpose(2, 0, 1).reshape(P, E * KD)
    ).astype(np.float32)

    gwt = np.ascontiguousarray(
        gate_w.T.reshape(KD, P, E).transpose(1, 0, 2)).astype(np.float32)
    gb = gate_b[None].astype(np.float32)
    ones1 = np.ones((1, P), BF16)
    onesc = np.ones((P, 1), BF16)
    lt = np.triu(np.ones((P, P))).astype(BF16)
    ident = np.eye(P).astype(BF16)
    iota = np.tile(np.arange(cfg.capm, dtype=np.float32), (P, 1))

    return dict(w1t=w1t, w3t=w3t, w2t=w2t, b1a=b1a, b3a=b3a, b2r=b2r,
                b2c=b2c, gwt=gwt, gb=gb, ones1=ones1, onesc=onesc,
                lt=lt, ident=ident, iota=iota)


def pack_xtok(cfg: Cfg, x_tokens):
    T, D = x_tokens.shape
    xt = x_tokens.reshape(cfg.TT, P, D).transpose(1, 0, 2)
    return np.ascontiguousarray(xt).astype(BF16)


def pack_xT(cfg: Cfg, x_tokens):
    T, D = x_tokens.shape
    xT = x_tokens.T.reshape(cfg.KD, P, T).transpose(1, 0, 2)
    return np.ascontiguousarray(xT).astype(np.float32)


def unpack_y(cfg: Cfg, y_dev):
    return np.ascontiguousarray(
        y_dev.transpose(1, 0, 2).reshape(cfg.T, cfg.D)).astype(np.float32)


def balance_tokens(xf, gate_w, gate_b, E=8, margin=1):
    """Assign tokens to 256-token buckets so per-(bucket, expert) routed
    counts are near their per-expert means. Returns (perm, capms):
    bucket-major token order and per-expert slot capacities."""
    N = xf.shape[0]
    NB = N // 256
    logits = xf @ gate_w.T + gate_b
    idx = np.argsort(-logits, axis=1)[:, :2]
    tgt = np.zeros(E)
    for e in range(E):
        tgt[e] = ((idx[:, 0] == e) | (idx[:, 1] == e)).sum() / NB
    tgt = np.maximum(tgt, 1.0)
    cnt = np.zeros((NB, E), np.float64)
    fill = np.zeros(NB, np.int64)
    assign = np.empty(N, np.int32)
    rng = np.random.RandomState(0)
    BIG = 1 << 40
    for t in rng.permutation(N):
        a, b = idx[t]
        s = np.maximum((cnt[:, a] + 1) / tgt[a],
                       (cnt[:, b] + 1) / tgt[b]) * 4096 + fill
        s[fill >= 256] = BIG
        bb = int(np.argmin(s))
        assign[t] = bb
        cnt[bb, a] += 1
        cnt[bb, b] += 1
        fill[bb] += 1
    perm = np.argsort(assign.astype(np.int64) * N + np.arange(N))
    capms = tuple(int(c) + margin for c in cnt.max(0))
    return perm, capms


_CACHE = {}


def _get_nc(cfg: Cfg):
    key = (cfg.D, cfg.H, cfg.E, cfg.n_sh, cfg.T, cfg.capms)
    if key not in _CACHE:
        _CACHE[key] = build_nc_v2(cfg)
    return _CACHE[key]


def plan_cfg(inputs):
    """Balance tokens from the actual routing; returns (cfg, perm)."""
    x = np.asarray(inputs["x"], np.float32)
    B, S, D = x.shape
    xf = x.reshape(-1, D)
    perm, capms = balance_tokens(
        xf, np.asarray(inputs["gate_w"], np.float32),
        np.asarray(inputs["gate_b"], np.float32))
    cfg = Cfg(D=D, T=(B * S) // 8, n_cores=8, capms=capms)
    return cfg, perm


def make_in_maps(cfg: Cfg, inputs, perm):
    static = pack_static(
        cfg,
        np.asarray(inputs["gate_w"], np.float32), np.asarray(inputs["gate_b"], np.float32),
        np.asarray(inputs["w1"], np.float32), np.asarray(inputs["b1"], np.float32),
        np.asarray(inputs["w2"], np.float32), np.asarray(inputs["b2"], np.float32),
        np.asarray(inputs["w3"], np.float32), np.asarray(inputs["b3"], np.float32),
        np.asarray(inputs["sw1"], np.float32), np.asarray(inputs["sb1"], np.float32),
        np.asarray(inputs["sw2"], np.float32), np.asarray(inputs["sb2"], np.float32),
        np.asarray(inputs["sw3"], np.float32), np.asarray(inputs["sb3"], np.float32),
    )
    x = np.asarray(inputs["x"], np.float32)
    B, S, D = x.shape
    xp = x.reshape(-1, D)[perm]
    in_maps = []
    for c in range(cfg.n_cores):
        mm = dict(static)
        xc = xp[c * cfg.T:(c + 1) * cfg.T]
        mm["xT"] = pack_xT(cfg, xc)
        mm["xtok"] = pack_xtok(cfg, xc)
        mm["xtb"] = mm["xT"].astype(BF16)
        in_maps.append(mm)
    return in_maps


def kernel(**inputs) -> np.ndarray:
    x = np.asarray(inputs["x"], np.float32)
    B, S, D = x.shape
    cfg, perm = plan_cfg(inputs)
    nc = _get_nc(cfg)
    in_maps = make_in_maps(cfg, inputs, perm)
    res = run_bass_kernel_spmd(nc, in_maps, list(range(cfg.n_cores)))
    yp = np.concatenate(
        [unpack_y(cfg, res.results[c]["y"]) for c in range(cfg.n_cores)], 0)
    out = np.empty_like(yp)
    out[perm] = yp
    return out.reshape(B, S, D)


# revision 81
# speedup vs baseline: 1.2664x; 1.0039x over previous
"""MoE (8 routed experts, top-2, + shared expert) on 8 NeuronCores.

Data-parallel over tokens (1024/core), weights replicated. The host
load-balances the token->bucket assignment (any sharding is allowed) so
each (256-token bucket, expert) routed count sits at its per-expert
mean, letting the capacity-dispatched kernel run with per-expert
capacities capm_e = max bucket count + 1 (59..74 on this routing)
instead of the binomial-tail uniform 96. Combine chains for expert
pairs with capm <= 64 are stacked into single K<=128 chunks.

Device kernel (per core):
  1. Gate in fp32 (matches reference routing bit-for-bit for the
     observed >=1.7e-4 top-2/3 logit gaps), renormalized top-2 combine
     weights cw, and bucket-local slot positions via triangular-matmul
     prefix sums.
  2. Routed experts e=0..7: one-hot matmul gather of x into CAPE=296
     slots, SwiGLU L1 (feature-major, free dim = slots), L2 in
     d-partition orientation (out[d, slot], bias via activation), then
     PE transposes to slot-major ye tiles held in SBUF.
  3. Tail: shared expert (8 sub-experts of hidden 512) interleaved with
     the scatter-combine (transposed scaled one-hots x ye), everything
     accumulating into a token-major f32 yacc; last shared sub-expert's
     L2 fuses the final add and streams y out.

Matmuls are bf16 with fp32 accumulation; weight DMA is split across the
SP/Pool/Act queues to avoid head-of-line blocking on one DMA queue.
"""

import numpy as np
import ml_dtypes

import concourse.bacc as bacc
import concourse.bass as bass
import concourse.tile as tile
import concourse.mybir as mybir
from concourse.bass_utils import run_bass_kernel_spmd

BF16 = ml_dtypes.bfloat16
F32 = mybir.dt.float32
BF = mybir.dt.bfloat16
AF = mybir.ActivationFunctionType
OP = mybir.AluOpType

P = 128


class Cfg:
    def __init__(self, D=1024, H=2048, E=8, n_sh=2, T=1024, n_cores=8,
                 capms=(74,) * 8):
        self.D, self.H, self.E, self.n_sh, self.T = D, H, E, n_sh, T
        self.NV = E + n_sh          # packed weight rows (8 routed + 2 shared)
        self.HS = n_sh * H          # shared hidden total (4096)
        self.KD = D // P            # contraction chunks over D
        self.HCN = H // P           # h chunks per packed VE
        self.TT = T // P            # token 128-tiles per core
        self.FT = T // 512          # shared L1 free 512-tiles
        self.DT = D // 512          # 512-wide d tiles
        self.n_cores = n_cores
        self.capms = tuple(capms)   # slots per (256-token bucket, expert)
        self.capm = max(self.capms)  # iota / tile sizing width
        self.NP = self.TT // 2      # buckets per core (pair of tiles)
        self.NSH = 8                # shared sub-experts
        self.HQ = (self.HS // P) // self.NSH  # h-chunks per sub-expert (4)


def build_nc_v2(cfg: Cfg):
    D, H, E, T = cfg.D, cfg.H, cfg.E, cfg.T
    KD, HCN, TT, FT, DT = cfg.KD, cfg.HCN, cfg.TT, cfg.FT, cfg.DT
    capm, NP = cfg.capm, cfg.NP
    NSH, HQ = cfg.NSH, cfg.HQ

    nc = bacc.Bacc("TRN2", target_bir_lowering=False)

    xT = nc.dram_tensor("xT", [P, KD, T], F32, kind="ExternalInput")
    xtok = nc.dram_tensor("xtok", [P, TT, D], BF, kind="ExternalInput")
    xtb = nc.dram_tensor("xtb", [P, KD, T], BF, kind="ExternalInput")
    w1t = nc.dram_tensor("w1t", [cfg.NV, HCN, P, KD, P], BF, kind="ExternalInput")
    w3t = nc.dram_tensor("w3t", [cfg.NV, HCN, P, KD, P], BF, kind="ExternalInput")
    w2t = nc.dram_tensor("w2t", [cfg.NV, P, HCN, D], BF, kind="ExternalInput")
    b1a = nc.dram_tensor("b1a", [cfg.NV, P, HCN], F32, kind="ExternalInput")
    b3a = nc.dram_tensor("b3a", [cfg.NV, P, HCN], F32, kind="ExternalInput")
    b2c = nc.dram_tensor("b2c", [P, E * KD], F32, kind="ExternalInput")
    gwt = nc.dram_tensor("gwt", [P, KD, E], F32, kind="ExternalInput")
    gb = nc.dram_tensor("gb", [1, E], F32, kind="ExternalInput")
    ones1 = nc.dram_tensor("ones1", [1, P], BF, kind="ExternalInput")
    onesc = nc.dram_tensor("onesc", [P, 1], BF, kind="ExternalInput")
    lt = nc.dram_tensor("lt", [P, P], BF, kind="ExternalInput")
    ident = nc.dram_tensor("ident", [P, P], BF, kind="ExternalInput")
    iota = nc.dram_tensor("iota", [P, capm], F32, kind="ExternalInput")
    y = nc.dram_tensor("y", [P, TT, D], BF, kind="ExternalOutput")

    OOB = 3.0e6

    from contextlib import ExitStack
    with tile.TileContext(nc) as tc:
        with ExitStack() as stack:
            pool_specs = dict(
                const1=dict(bufs=1), xbig=dict(bufs=1),
                gchunk=dict(bufs=2), gtmp=dict(bufs=4),
                w13=dict(bufs=3), w2p=dict(bufs=2), b13=dict(bufs=2),
                xep=dict(bufs=1), hrout=dict(bufs=1), hshp=dict(bufs=2),
                yeBp=dict(bufs=1), pep=dict(bufs=2), s1p=dict(bufs=2),
                combp=dict(bufs=8), ytp=dict(bufs=2),
                ps_o1=dict(bufs=2, space="PSUM"),
                ps_o3=dict(bufs=2, space="PSUM"),
                ps_l2=dict(bufs=2, space="PSUM"),
                ps_tr=dict(bufs=2, space="PSUM"),
            )
            pools = {n: stack.enter_context(tc.tile_pool(name=n, **kw))
                     for n, kw in pool_specs.items()}
            (const1, xbig, gchunk, gtmp, w13, w2p, b13, xep, hrout,
             hshp, yeBp, pep, s1p, combp, ytp, ps_o1, ps_o3, ps_l2,
             ps_tr) = (
                pools[n] for n in (
                    "const1", "xbig", "gchunk", "gtmp", "w13", "w2p", "b13",
                    "xep", "hrout", "hshp", "yeBp", "pep", "s1p",
                    "combp", "ytp", "ps_o1", "ps_o3", "ps_l2", "ps_tr"))
            # ---- resident state ----
            # xtok_sb and xTb share one 2MB buffer (tag xb): xtok is dead
            # after the last gather; xTb is DMA'd into the same space then.
            xtok_sb = xbig.tile([P, TT, D], BF, name="xtok_sb", tag="xb")
            yacc = const1.tile([P, TT, D], F32)
            cw = const1.tile([P, TT, E], F32)
            posb_all = const1.tile([P, TT, E], F32)
            ye_sb = const1.tile([P, E * NP, D], BF)
            gwt_sb = const1.tile([P, KD, E], F32)
            gb_sb = const1.tile([1, E], F32)
            ones_sb = const1.tile([1, P], BF)
            onesc_sb = const1.tile([P, 1], BF)
            lt_sb = const1.tile([P, P], BF)
            id_sb = const1.tile([P, P], BF)
            iota_sb = const1.tile([P, capm], F32)
            b2c_sb = const1.tile([P, E * KD], F32)
            zerob = const1.tile([P, 1], F32)
            onesf = const1.tile([1, P], F32)

            # first two gate tiles + gate weights lead the DMA queues so
            # the gate starts without sitting behind the bulk prologue
            xc_pre = {}
            for m in range(2):
                xc = gchunk.tile([P, KD, P], F32, name="xchunk")
                nc.sync.dma_start(out=xc[:], in_=xT[:, :, m * P:(m + 1) * P])
                xc_pre[m] = xc
            nc.sync.dma_start(out=gwt_sb[:], in_=gwt[:])
            nc.sync.dma_start(out=gb_sb[:], in_=gb[:])
            # secondary consts flow on the Act queue in parallel so the
            # gate's per-tile xchunk stream on sync isn't delayed
            nc.scalar.dma_start(out=ones_sb[:], in_=ones1[:])
            nc.scalar.dma_start(out=onesc_sb[:], in_=onesc[:])
            nc.scalar.dma_start(out=lt_sb[:], in_=lt[:])
            nc.scalar.dma_start(out=id_sb[:], in_=ident[:])
            nc.scalar.dma_start(out=iota_sb[:], in_=iota[:])
            nc.scalar.dma_start(out=b2c_sb[:], in_=b2c[:])
            nc.vector.memset(zerob[:], 0.0)
            nc.vector.memset(onesf[:], 1.0)

            # prefetch expert 0's first L1 weight chunks
            pre_w = {}
            for hc in range(3):
                w1c = w13.tile([P, KD, P], BF, name="w1c", tag="w1c")
                nc.sync.dma_start(out=w1c[:], in_=w1t[0, hc])
                w3c = w13.tile([P, KD, P], BF, name="w3c", tag="w3c")
                nc.gpsimd.dma_start(out=w3c[:], in_=w3t[0, hc])
                pre_w[hc] = (w1c, w3c)
            # xtok hands off to the DMA engines late (pool-queue tail) so
            # its 2MB transfer neither starves the small gate-const loads
            # nor blocks the per-tile gate xchunk stream on sync; it is
            # first needed by expert 0's gather, well after the gate.
            nc.gpsimd.dma_start(out=xtok_sb[:], in_=xtok[:])

            # ---- gate + bucket positions, per 128-token tile ----
            cntb = None
            for m in range(TT):
                if m in xc_pre:
                    xchunk = xc_pre[m]
                else:
                    xchunk = gchunk.tile([P, KD, P], F32)
                    nc.sync.dma_start(out=xchunk[:],
                                      in_=xT[:, :, m * P:(m + 1) * P])

                pg = ps_l2.tile([P, E], F32, space="PSUM", name="pg", tag="l2")
                for k in range(KD):
                    nc.tensor.matmul(out=pg[:], lhsT=xchunk[:, k, :],
                                     rhs=gwt_sb[:, k, :],
                                     start=(k == 0), stop=False)
                nc.tensor.matmul(out=pg[:], lhsT=onesf[:], rhs=gb_sb[:],
                                 start=False, stop=True)

                lg = gtmp.tile([P, E], F32)
                nc.scalar.activation(lg[:], pg[:], AF.Copy)
                m8 = gtmp.tile([P, 8], F32)
                nc.vector.max(m8[:], lg[:])
                ex = gtmp.tile([P, E], F32)
                nc.vector.tensor_scalar(out=ex[:], in0=lg[:],
                                        scalar1=m8[:, 0:1], scalar2=None,
                                        op0=OP.subtract)
                nc.scalar.activation(ex[:], ex[:], AF.Exp, bias=zerob[:])
                mask = gtmp.tile([P, E], F32)
                nc.vector.tensor_scalar(out=mask[:], in0=lg[:],
                                        scalar1=m8[:, 1:2], scalar2=None,
                                        op0=OP.is_ge)
                e2 = gtmp.tile([P, 1], F32)
                nc.vector.tensor_tensor(out=e2[:], in0=m8[:, 1:2],
                                        in1=m8[:, 0:1], op=OP.subtract)
                nc.scalar.activation(e2[:], e2[:], AF.Exp, bias=zerob[:])
                den = gtmp.tile([P, 1], F32)
                nc.vector.tensor_scalar(out=den[:], in0=e2[:], scalar1=1.0,
                                        scalar2=None, op0=OP.add)
                rec = gtmp.tile([P, 1], F32)
                nc.vector.reciprocal(rec[:], den[:])
                cwm = gtmp.tile([P, E], F32)
                nc.vector.tensor_mul(cwm[:], ex[:], mask[:])
                nc.vector.tensor_scalar(out=cw[:, m, :], in0=cwm[:],
                                        scalar1=rec[:, 0:1], scalar2=None,
                                        op0=OP.mult)

                # bucket-local slot: pair prefix(mask) - mask; OOB unrouted
                maskb = gtmp.tile([P, E], BF)
                nc.vector.tensor_copy(maskb[:], mask[:])
                pp = ps_tr.tile([P, E], F32, space="PSUM", name="pp", tag="tr")
                if m % 2 == 0:
                    nc.tensor.matmul(out=pp[:], lhsT=lt_sb[:],
                                     rhs=maskb[:], start=True, stop=True)
                    cnt_ps = ps_tr.tile([1, E], F32, space="PSUM",
                                        name="cntp", tag="tr")
                    nc.tensor.matmul(out=cnt_ps[:], lhsT=onesc_sb[:],
                                     rhs=maskb[:], start=True, stop=True)
                    cntb = gtmp.tile([1, E], BF, name="cntb")
                    nc.scalar.activation(cntb[:], cnt_ps[:], AF.Copy)
                else:
                    nc.tensor.matmul(out=pp[:], lhsT=lt_sb[:],
                                     rhs=maskb[:], start=True, stop=False)
                    nc.tensor.matmul(out=pp[:], lhsT=ones_sb[:],
                                     rhs=cntb[:], start=False, stop=True)
                t1m = gtmp.tile([P, E], F32)
                nc.vector.scalar_tensor_tensor(out=t1m[:], in0=mask[:],
                                               scalar=-1.0, in1=pp[:],
                                               op0=OP.mult, op1=OP.add)
                notm = gtmp.tile([P, E], F32)
                nc.vector.tensor_scalar(out=notm[:], in0=mask[:],
                                        scalar1=-1.0, scalar2=1.0,
                                        op0=OP.mult, op1=OP.add)
                nc.vector.scalar_tensor_tensor(out=posb_all[:, m, :],
                                               in0=notm[:], scalar=OOB,
                                               in1=t1m[:],
                                               op0=OP.mult, op1=OP.add)

            # combine groups: stack pairs of experts with capm <= 64 into
            # one K<=128 chunk (second member at partition base 64 — PE
            # writes only allow bases 0/32/64). Gap rows are zeroed once.
            small = [e for e in range(E) if cfg.capms[e] <= 64]
            big = [e for e in range(E) if cfg.capms[e] > 64]
            groups = []
            for i in range(0, len(small) - 1, 2):
                groups.append((small[i], small[i + 1]))
            if len(small) % 2:
                groups.append((small[-1],))
            groups.extend((e,) for e in big)
            home = {}
            yoff = {}
            gap_zero = []
            for g in groups:
                for i, e in enumerate(g):
                    home[e] = g[0]
                    yoff[e] = 64 * i
                if len(g) == 2 and cfg.capms[g[0]] < 64:
                    gap_zero.append((cfg.capms[g[0]], g[0]))
            grp_k = {g: (64 + cfg.capms[g[1]] if len(g) == 2
                         else cfg.capms[g[0]]) for g in groups}

            # zero the ye/one-hot gap rows [capm_a, 64) of paired tiles so
            # the stacked K=64+capm_b combine chains read zeros there
            for cap_a, hm in gap_zero:
                nc.vector.memset(
                    ye_sb[cap_a:64, hm * NP:(hm + 1) * NP, :], 0.0)

            # ---- routed experts over dispatched slots ----
            pre_sh = {}
            for e in range(E):
                capm_e = cfg.capms[e]
                CAPE = NP * capm_e
                b1sb = b13.tile([P, HCN], F32, name="b1sb", tag="b1")
                nc.sync.dma_start(out=b1sb[:], in_=b1a[e])
                b3sb = b13.tile([P, HCN], F32, name="b3sb", tag="b3")
                nc.sync.dma_start(out=b3sb[:], in_=b3a[e])

                # prefetch the first two w2 quarters; they land during L1
                w2qs_pre = []
                for dq in range(2):
                    w2q0 = w2p.tile([P, HCN, 256], BF, name="w2q", tag="w2q")
                    nc.sync.dma_start(
                        out=w2q0[:],
                        in_=w2t[e][:, :, dq * 256:(dq + 1) * 256])
                    w2qs_pre.append(w2q0)

                # one-hot dispatch tiles for all 8 token tiles
                pe_all = pep.tile([P, TT, capm_e], BF, name="pe_all", tag="pe")
                for m in range(TT):
                    nc.vector.tensor_scalar(
                        out=pe_all[:, m, :], in0=iota_sb[:, :capm_e],
                        scalar1=posb_all[:, m, e:e + 1],
                        scalar2=None, op0=OP.is_equal)

                # matmul gather: xeT[k][d, slot] = sum_m x_m^T @ Pe_m
                xeT = xep.tile([P, KD, CAPE], BF, name="xeT", tag="xeT")
                for k in range(KD):
                    gxp, gxt = (ps_l2, "l2") if k % 2 == 0 else (ps_tr, "tr")
                    gx = gxp.tile([P, CAPE], F32, space="PSUM",
                                  name="gx", tag=gxt)
                    for pr in range(NP):
                        for h in range(2):
                            m = 2 * pr + h
                            nc.tensor.matmul(
                                out=gx[:, pr * capm_e:(pr + 1) * capm_e],
                                lhsT=xtok_sb[:, m, k * P:(k + 1) * P],
                                rhs=pe_all[:, m, :],
                                start=(h == 0), stop=(h == 1))
                    nc.vector.tensor_copy(xeT[:, k, :], gx[:])

                if e == E - 1:
                    # last xtok reader just emitted: reload the shared
                    # buffer with d-major x for the tail's shared expert;
                    # the transfer hides under expert 7's L1/L2.
                    xTb = xbig.tile([P, KD, T], BF, name="xTb", tag="xb")
                    nc.scalar.dma_start(out=xTb[:], in_=xtb[:])

                # L1: hT[h, slot] = silu(W1 xe + b1) * (W3 xe + b3)
                hT = hrout.tile([P, HCN, CAPE], BF, name="hT", tag="hT")
                for hc in range(HCN):
                    if e == 0 and hc in pre_w:
                        w1c, w3c = pre_w[hc]
                    else:
                        w1c = w13.tile([P, KD, P], BF, name="w1c", tag="w1c")
                        nc.sync.dma_start(out=w1c[:], in_=w1t[e, hc])
                        w3c = w13.tile([P, KD, P], BF, name="w3c", tag="w3c")
                        nc.gpsimd.dma_start(out=w3c[:], in_=w3t[e, hc])
                    o1 = ps_o1.tile([P, CAPE], F32, space="PSUM",
                                    name="o1", tag="o1")
                    for k in range(KD):
                        nc.tensor.matmul(out=o1[:], lhsT=w1c[:, k, :],
                                         rhs=xeT[:, k, :],
                                         start=(k == 0), stop=(k == KD - 1))
                    s1 = s1p.tile([P, CAPE], F32, name="s1", tag="s1")
                    nc.scalar.activation(s1[:], o1[:], AF.Sigmoid,
                                         bias=b1sb[:, hc:hc + 1])
                    t1 = s1p.tile([P, CAPE], F32, name="t1", tag="t1")
                    nc.vector.scalar_tensor_tensor(
                        out=t1[:], in0=o1[:],
                        scalar=b1sb[:, hc:hc + 1], in1=s1[:],
                        op0=OP.add, op1=OP.mult)
                    o3 = ps_o3.tile([P, CAPE], F32, space="PSUM",
                                    name="o3", tag="o3")
                    for k in range(KD):
                        nc.tensor.matmul(out=o3[:], lhsT=w3c[:, k, :],
                                         rhs=xeT[:, k, :],
                                         start=(k == 0), stop=(k == KD - 1))
                    nc.vector.scalar_tensor_tensor(
                        out=hT[:, hc, :], in0=o3[:],
                        scalar=b3sb[:, hc:hc + 1], in1=t1[:],
                        op0=OP.add, op1=OP.mult)

                # L2 (d-partition orientation) + bias, then transpose to
                # slot-major ye tiles; transposes staggered one dc behind
                # the chains so their yeB reads never stall the PE.
                yeB = yeBp.tile([P, KD, CAPE], BF, name="yeB", tag="yeB")
                pend = []

                def emit_transp(dc, e=e, yeB=yeB, capm_e=capm_e):
                    o = yoff[e]
                    tgt = home[e] * NP
                    for pr in range(NP):
                        p2t = ps_tr.tile([P, P], BF, space="PSUM",
                                         name="p2t", tag="tr")
                        nc.tensor.transpose(
                            out=p2t[o:o + capm_e, :],
                            in_=yeB[:, dc, pr * capm_e:(pr + 1) * capm_e],
                            identity=id_sb[:])
                        nc.vector.tensor_copy(
                            ye_sb[o:o + capm_e, tgt + pr, dc * P:(dc + 1) * P],
                            p2t[o:o + capm_e, :])

                for dq in range(4):
                    w2q = w2qs_pre[dq]
                    for dc2 in range(2):
                        dc = dq * 2 + dc2
                        pl2 = ps_l2.tile([P, CAPE], F32, space="PSUM",
                                         name="pl2", tag="l2")
                        for hc in range(HCN):
                            nc.tensor.matmul(
                                out=pl2[:],
                                lhsT=w2q[:, hc, dc2 * P:(dc2 + 1) * P],
                                rhs=hT[:, hc, :],
                                start=(hc == 0), stop=(hc == HCN - 1))
                        nc.vector.tensor_scalar(
                            out=yeB[:, dc, :], in0=pl2[:],
                            scalar1=b2c_sb[:, e * KD + dc:e * KD + dc + 1],
                            scalar2=None, op0=OP.add)
                        if pend:
                            emit_transp(pend.pop())
                        pend.append(dc)
                    if dq + 2 < 4:
                        # refill two quarters ahead (this quarter's chains
                        # just freed the buffer, so the queue-head wait is
                        # short; only next-expert w1c prefetches sit behind)
                        w2n = w2p.tile([P, HCN, 256], BF, name="w2q",
                                       tag="w2q")
                        nc.sync.dma_start(
                            out=w2n[:],
                            in_=w2t[e][:, :, (dq + 2) * 256:(dq + 3) * 256])
                        w2qs_pre.append(w2n)
                while pend:
                    emit_transp(pend.pop())

            # ---- tail: shared sub-experts (hidden 512 each) + combine ----
            def emit_shared_l1_unit(s, hcl):
                sv = E + s // 4
                hcg = (s % 4) * HQ + hcl
                if (s, hcl) in pre_sh:
                    w1c, w3c = pre_sh[(s, hcl)]
                else:
                    w1c = w13.tile([P, KD, P], BF, name="w1c", tag="w1c")
                    nc.sync.dma_start(out=w1c[:], in_=w1t[sv, hcg])
                    w3c = w13.tile([P, KD, P], BF, name="w3c", tag="w3c")
                    nc.gpsimd.dma_start(out=w3c[:], in_=w3t[sv, hcg])
                hT_s = hts[s]
                for ft in range(FT):
                    fsl = slice(ft * 512, (ft + 1) * 512)
                    o1 = ps_o1.tile([P, 512], F32, space="PSUM",
                                    name="o1", tag="o1")
                    for k in range(KD):
                        nc.tensor.matmul(out=o1[:], lhsT=w1c[:, k, :],
                                         rhs=xTb[:, k, fsl],
                                         start=(k == 0), stop=(k == KD - 1))
                    s1 = s1p.tile([P, 512], F32, name="s1", tag="s1")
                    nc.scalar.activation(s1[:], o1[:], AF.Sigmoid,
                                         bias=bsh1[s // 4][:, hcg:hcg + 1])
                    t1 = s1p.tile([P, 512], F32, name="t1", tag="t1")
                    nc.vector.scalar_tensor_tensor(
                        out=t1[:], in0=o1[:],
                        scalar=bsh1[s // 4][:, hcg:hcg + 1], in1=s1[:],
                        op0=OP.add, op1=OP.mult)
                    o3 = ps_o3.tile([P, 512], F32, space="PSUM",
                                    name="o3", tag="o3")
                    for k in range(KD):
                        nc.tensor.matmul(out=o3[:], lhsT=w3c[:, k, :],
                                         rhs=xTb[:, k, fsl],
                                         start=(k == 0), stop=(k == KD - 1))
                    nc.vector.scalar_tensor_tensor(
                        out=hT_s[:, hcl, fsl], in0=o3[:],
                        scalar=bsh3[s // 4][:, hcg:hcg + 1], in1=t1[:],
                        op0=OP.add, op1=OP.mult)

            def emit_combine(m):
                pr = m // 2
                p2s_l = []
                for g in groups:
                    kk = grp_k[g]
                    p2c = ps_tr.tile([P, P], BF, space="PSUM",
                                     name="p2c", tag="tr")
                    for e in g:
                        capm_e = cfg.capms[e]
                        o = yoff[e]
                        pe2 = gtmp.tile([P, capm_e], BF, name="pe2")
                        nc.vector.tensor_scalar(
                            out=pe2[:], in0=iota_sb[:, :capm_e],
                            scalar1=posb_all[:, m, e:e + 1],
                            scalar2=None, op0=OP.is_equal)
                        pew = gtmp.tile([P, capm_e], BF, name="pew")
                        nc.vector.tensor_scalar(out=pew[:], in0=pe2[:],
                                                scalar1=cw[:, m, e:e + 1],
                                                scalar2=None, op0=OP.mult)
                        nc.tensor.transpose(out=p2c[o:o + capm_e, :],
                                            in_=pew[:], identity=id_sb[:])
                    p2s = combp.tile([kk, P], BF, name="p2s")
                    nc.scalar.activation(p2s[:], p2c[:kk, :], AF.Copy)
                    if len(g) == 2 and cfg.capms[g[0]] < 64:
                        # rows [capm_a, 64) came from uninitialized PSUM;
                        # zero them (their ye rows are zero too, but NaN
                        # garbage would still poison the product)
                        nc.vector.memset(p2s[cfg.capms[g[0]]:64, :], 0.0)
                    p2s_l.append((p2s, kk))
                for dt in range(DT):
                    dsl = slice(dt * 512, (dt + 1) * 512)
                    yp = ps_l2.tile([P, 512], F32, space="PSUM",
                                    name="yp", tag="l2")
                    for i, g in enumerate(groups):
                        p2s, kk = p2s_l[i]
                        nc.tensor.matmul(
                            out=yp[:], lhsT=p2s[:],
                            rhs=ye_sb[0:kk, home[g[0]] * NP + pr, dsl],
                            start=(i == 0), stop=(i == len(groups) - 1))
                    nc.vector.tensor_copy(yacc[:, m, dsl], yp[:])

            def emit_shared_l2(s):
                sv = E + s // 4
                hcg0 = (s % 4) * HQ
                w2q = w2p.tile([P, HQ, D], BF, name="w2qs", tag="w2q")
                nc.scalar.dma_start(out=w2q[:],
                                    in_=w2t[sv][:, hcg0:hcg0 + HQ, :])
                hT_s = hts[s]
                for tt in range(TT):
                    tsl = slice(tt * P, (tt + 1) * P)
                    for dt in range(DT):
                        dsl = slice(dt * 512, (dt + 1) * 512)
                        if s >= NSH - 2:
                            # L1 is done by now: o1/o3 banks are free, use a
                            # deeper 3-pool rotation so chains never wait on
                            # the yacc-add evictions
                            pl, tg = [(ps_l2, "l2"), (ps_tr, "tr"),
                                      (ps_o1, "o1")][(tt * DT + dt) % 3]
                        else:
                            pl, tg = ((ps_l2, "l2") if (tt * DT + dt) % 2 == 0
                                      else (ps_tr, "tr"))
                        yp2 = pl.tile([P, 512], F32, space="PSUM",
                                      name="yp2", tag=tg)
                        for hcl in range(HQ):
                            nc.tensor.matmul(
                                out=yp2[:], lhsT=hT_s[:, hcl, tsl],
                                rhs=w2q[:, hcl, dsl],
                                start=(hcl == 0),
                                stop=(hcl == HQ - 1))
                        if s < NSH - 1:
                            nc.vector.tensor_add(yacc[:, tt, dsl],
                                                 yacc[:, tt, dsl], yp2[:])
                        else:
                            yt = ytp.tile([P, 512], BF, name="yt", bufs=4)
                            nc.vector.tensor_add(yt[:], yacc[:, tt, dsl],
                                                 yp2[:])
                            nc.sync.dma_start(out=y[:, tt, dsl], in_=yt[:])

            bsh1 = []
            bsh3 = []
            for sv in range(2):
                b1s = b13.tile([P, HCN], F32, name="b1sh", tag="b1sh")
                nc.sync.dma_start(out=b1s[:], in_=b1a[E + sv])
                b3s = b13.tile([P, HCN], F32, name="b3sh", tag="b3sh")
                nc.sync.dma_start(out=b3s[:], in_=b3a[E + sv])
                bsh1.append(b1s)
                bsh3.append(b3s)

            hts = {}
            for s in range(NSH):
                if s < 2:
                    hts[s] = hshp.tile([P, HQ, T], BF, name=f"hTs{s}",
                                       tag="hTs")
            # interleave first two shared sub-experts' L1 with combine;
            # combine leads: its inputs (ye, cw, pos) are ready at routed
            # end, covering the xTb/w1c arrival for the shared L1
            ci = 0
            for s in range(2):
                for hcl in range(HQ):
                    emit_combine(ci)
                    ci += 1
                    emit_shared_l1_unit(s, hcl)
            # pipeline: L2(s) || L1(s+2)
            for s in range(NSH):
                emit_shared_l2(s)
                if s + 2 < NSH:
                    hts[s + 2] = hshp.tile([P, HQ, T], BF, name=f"hTs{s+2}",
                                           tag="hTs")
                    for hcl in range(HQ):
                        emit_shared_l1_unit(s + 2, hcl)

    nc.compile()
    return nc


# ---------------- host-side packing ----------------

def pack_static(cfg: Cfg, gate_w, gate_b, w1, b1, w2, b2, w3, b3,
                sw1, sb1, sw2, sb2, sw3, sb3):
    D, H, E, NV, n_sh = cfg.D, cfg.H, cfg.E, cfg.NV, cfg.n_sh
    KD, HCN = cfg.KD, cfg.HCN

    w1T = np.transpose(w1, (0, 2, 1))                      # [E, D, H]
    w3T = np.transpose(w3, (0, 2, 1))
    w2T = np.transpose(w2, (0, 2, 1))                      # [E, H, D]
    s1T = sw1.T.reshape(D, n_sh, H).transpose(1, 0, 2)     # [n_sh, D, H]
    s3T = sw3.T.reshape(D, n_sh, H).transpose(1, 0, 2)
    s2T = sw2.T.reshape(n_sh, H, D)                        # [n_sh, H, D]
    w1T_all = np.concatenate([w1T, s1T], 0)                # [NV, D, H]
    w3T_all = np.concatenate([w3T, s3T], 0)
    w2T_all = np.concatenate([w2T, s2T], 0)                # [NV, H, D]

    w1t = np.ascontiguousarray(
        w1T_all.reshape(NV, KD, P, HCN, P).transpose(0, 3, 2, 1, 4)).astype(BF16)
    w3t = np.ascontiguousarray(
        w3T_all.reshape(NV, KD, P, HCN, P).transpose(0, 3, 2, 1, 4)).astype(BF16)
    w2t = np.ascontiguousarray(
        w2T_all.reshape(NV, HCN, P, D).transpose(0, 2, 1, 3)).astype(BF16)

    b1_all = np.concatenate([b1, sb1.reshape(n_sh, H)], 0)  # [NV, H]
    b3_all = np.concatenate([b3, sb3.reshape(n_sh, H)], 0)
    b1a = np.ascontiguousarray(
        b1_all.reshape(NV, HCN, P).transpose(0, 2, 1)).astype(np.float32)
    b3a = np.ascontiguousarray(
        b3_all.reshape(NV, HCN, P).transpose(0, 2, 1)).astype(np.float32)

    # routed b2 in d-partition layout, with the shared expert's sb2 folded
    # in: the renormalized top-2 weights sum to exactly 1, so adding sb2 to
    # every routed expert's bias reproduces the unconditional +sb2 exactly.
    b2f = b2 + sb2[None]
    b2c = np.ascontiguousarray(
        b2f.reshape(E, KD, P).transpose(2, 0, 1).reshape(P, E * KD)
    ).astype(np.float32)

    gwt = np.ascontiguousarray(
        gate_w.T.reshape(KD, P, E).transpose(1, 0, 2)).astype(np.float32)
    gb = gate_b[None].astype(np.float32)
    ones1 = np.ones((1, P), BF16)
    onesc = np.ones((P, 1), BF16)
    lt = np.triu(np.ones((P, P))).astype(BF16)
    ident = np.eye(P).astype(BF16)
    iota = np.tile(np.arange(cfg.capm, dtype=np.float32), (P, 1))

    return dict(w1t=w1t, w3t=w3t, w2t=w2t, b1a=b1a, b3a=b3a,
                b2c=b2c, gwt=gwt, gb=gb, ones1=ones1, onesc=onesc,
                lt=lt, ident=ident, iota=iota)


def pack_xtok(cfg: Cfg, x_tokens):
    T, D = x_tokens.shape
    xt = x_tokens.reshape(cfg.TT, P, D).transpose(1, 0, 2)
    return np.ascontiguousarray(xt).astype(BF16)


def pack_xT(cfg: Cfg, x_tokens):
    T, D = x_tokens.shape
    xT = x_tokens.T.reshape(cfg.KD, P, T).transpose(1, 0, 2)
    return np.ascontiguousarray(xT).astype(np.float32)


def unpack_y(cfg: Cfg, y_dev):
    return np.ascontiguousarray(
        y_dev.transpose(1, 0, 2).reshape(cfg.T, cfg.D)).astype(np.float32)


def balance_tokens(xf, gate_w, gate_b, E=8, margin=1):
    """Assign tokens to 256-token buckets so per-(bucket, expert) routed
    counts are near their per-expert means. Returns (perm, capms):
    bucket-major token order and per-expert slot capacities."""
    N = xf.shape[0]
    NB = N // 256
    logits = xf @ gate_w.T + gate_b
    idx = np.argsort(-logits, axis=1)[:, :2]
    tgt = np.zeros(E)
    for e in range(E):
        tgt[e] = ((idx[:, 0] == e) | (idx[:, 1] == e)).sum() / NB
    tgt = np.maximum(tgt, 1.0)
    cnt = np.zeros((NB, E), np.float64)
    fill = np.zeros(NB, np.int64)
    assign = np.empty(N, np.int32)
    rng = np.random.RandomState(0)
    BIG = 1 << 40
    for t in rng.permutation(N):
        a, b = idx[t]
        s = np.maximum((cnt[:, a] + 1) / tgt[a],
                       (cnt[:, b] + 1) / tgt[b]) * 4096 + fill
        s[fill >= 256] = BIG
        bb = int(np.argmin(s))
        assign[t] = bb
        cnt[bb, a] += 1
        cnt[bb, b] += 1
        fill[bb] += 1
    perm = np.argsort(assign.astype(np.int64) * N + np.arange(N))
    capms = tuple(int(c) + margin for c in cnt.max(0))
    return perm, capms


_CACHE = {}


def _get_nc(cfg: Cfg):
    key = (cfg.D, cfg.H, cfg.E, cfg.n_sh, cfg.T, cfg.capms)
    if key not in _CACHE:
        _CACHE[key] = build_nc_v2(cfg)
    return _CACHE[key]


def plan_cfg(inputs):
    """Balance tokens from the actual routing; returns (cfg, perm)."""
    x = np.asarray(inputs["x"], np.float32)
    B, S, D = x.shape
    xf = x.reshape(-1, D)
    perm, capms = balance_tokens(
        xf, np.asarray(inputs["gate_w"], np.float32),
        np.asarray(inputs["gate_b"], np.float32))
    cfg = Cfg(D=D, T=(B * S) // 8, n_cores=8, capms=capms)
    return cfg, perm


def make_in_maps(cfg: Cfg, inputs, perm):
    static = pack_static(
        cfg,
        np.asarray(inputs["gate_w"], np.float32), np.asarray(inputs["gate_b"], np.float32),
        np.asarray(inputs["w1"], np.float32), np.asarray(inputs["b1"], np.float32),
        np.asarray(inputs["w2"], np.float32), np.asarray(inputs["b2"], np.float32),
        np.asarray(inputs["w3"], np.float32), np.asarray(inputs["b3"], np.float32),
        np.asarray(inputs["sw1"], np.float32), np.asarray(inputs["sb1"], np.float32),
        np.asarray(inputs["sw2"], np.float32), np.asarray(inputs["sb2"], np.float32),
        np.asarray(inputs["sw3"], np.float32), np.asarray(inputs["sb3"], np.float32),
    )
    x = np.asarray(inputs["x"], np.float32)
    B, S, D = x.shape
    xp = x.reshape(-1, D)[perm]
    in_maps = []
    for c in range(cfg.n_cores):
        mm = dict(static)
        xc = xp[c * cfg.T:(c + 1) * cfg.T]
        mm["xT"] = pack_xT(cfg, xc)
        mm["xtok"] = pack_xtok(cfg, xc)
        mm["xtb"] = mm["xT"].astype(BF16)
        in_maps.append(mm)
    return in_maps


def kernel(**inputs) -> np.ndarray:
    x = np.asarray(inputs["x"], np.float32)
    B, S, D = x.shape
    cfg, perm = plan_cfg(inputs)
    nc = _get_nc(cfg)
    in_maps = make_in_maps(cfg, inputs, perm)
    res = run_bass_kernel_spmd(nc, in_maps, list(range(cfg.n_cores)))
    yp = np.concatenate(
        [unpack_y(cfg, res.results[c]["y"]) for c in range(cfg.n_cores)], 0)
    out = np.empty_like(yp)
    out[perm] = yp
    return out.reshape(B, S, D)


# revision 86
# speedup vs baseline: 1.2766x; 1.0080x over previous
"""MoE (8 routed experts, top-2, + shared expert) on 8 NeuronCores.

Data-parallel over tokens (1024/core), weights replicated. The host
load-balances the token->bucket assignment (any sharding is allowed) so
each (256-token bucket, expert) routed count sits at its per-expert
mean, letting the capacity-dispatched kernel run with per-expert
capacities capm_e = max bucket count + 1 (59..74 on this routing)
instead of the binomial-tail uniform 96. Combine chains for expert
pairs with capm <= 64 are stacked into single K<=128 chunks.

Device kernel (per core):
  1. Gate in fp32 (matches reference routing bit-for-bit for the
     observed >=1.7e-4 top-2/3 logit gaps), renormalized top-2 combine
     weights cw, and bucket-local slot positions via triangular-matmul
     prefix sums.
  2. Routed experts e=0..7: one-hot matmul gather of x into CAPE=296
     slots, SwiGLU L1 (feature-major, free dim = slots), L2 in
     d-partition orientation (out[d, slot], bias via activation), then
     PE transposes to slot-major ye tiles held in SBUF.
  3. Tail: shared expert (8 sub-experts of hidden 512) interleaved with
     the scatter-combine (transposed scaled one-hots x ye), everything
     accumulating into a token-major f32 yacc; last shared sub-expert's
     L2 fuses the final add and streams y out.

Matmuls are bf16 with fp32 accumulation; weight DMA is split across the
SP/Pool/Act queues to avoid head-of-line blocking on one DMA queue.
"""

import numpy as np
import ml_dtypes

import concourse.bacc as bacc
import concourse.bass as bass
import concourse.tile as tile
import concourse.mybir as mybir
from concourse.bass_utils import run_bass_kernel_spmd

BF16 = ml_dtypes.bfloat16
F32 = mybir.dt.float32
BF = mybir.dt.bfloat16
AF = mybir.ActivationFunctionType
OP = mybir.AluOpType

P = 128


class Cfg:
    def __init__(self, D=1024, H=2048, E=8, n_sh=2, T=1024, n_cores=8,
                 capms=(74,) * 8):
        self.D, self.H, self.E, self.n_sh, self.T = D, H, E, n_sh, T
        self.NV = E + n_sh          # packed weight rows (8 routed + 2 shared)
        self.HS = n_sh * H          # shared hidden total (4096)
        self.KD = D // P            # contraction chunks over D
        self.HCN = H // P           # h chunks per packed VE
        self.TT = T // P            # token 128-tiles per core
        self.FT = T // 512          # shared L1 free 512-tiles
        self.DT = D // 512          # 512-wide d tiles
        self.n_cores = n_cores
        self.capms = tuple(capms)   # slots per (256-token bucket, expert)
        self.capm = max(self.capms)  # iota / tile sizing width
        self.NP = self.TT // 2      # buckets per core (pair of tiles)
        self.NSH = 8                # shared sub-experts
        self.HQ = (self.HS // P) // self.NSH  # h-chunks per sub-expert (4)


def build_nc_v2(cfg: Cfg):
    D, H, E, T = cfg.D, cfg.H, cfg.E, cfg.T
    KD, HCN, TT, FT, DT = cfg.KD, cfg.HCN, cfg.TT, cfg.FT, cfg.DT
    capm, NP = cfg.capm, cfg.NP
    NSH, HQ = cfg.NSH, cfg.HQ

    nc = bacc.Bacc("TRN2", target_bir_lowering=False)

    xT = nc.dram_tensor("xT", [P, KD, T], F32, kind="ExternalInput")
    xtok = nc.dram_tensor("xtok", [P, TT, D], BF, kind="ExternalInput")
    xtb = nc.dram_tensor("xtb", [P, KD, T], BF, kind="ExternalInput")
    w1t = nc.dram_tensor("w1t", [cfg.NV, HCN, P, KD, P], BF, kind="ExternalInput")
    w3t = nc.dram_tensor("w3t", [cfg.NV, HCN, P, KD, P], BF, kind="ExternalInput")
    w2t = nc.dram_tensor("w2t", [cfg.NV, P, HCN, D], BF, kind="ExternalInput")
    b1a = nc.dram_tensor("b1a", [cfg.NV, P, HCN], F32, kind="ExternalInput")
    b3a = nc.dram_tensor("b3a", [cfg.NV, P, HCN], F32, kind="ExternalInput")
    b2c = nc.dram_tensor("b2c", [P, E * KD], F32, kind="ExternalInput")
    gwt = nc.dram_tensor("gwt", [P, KD, E], F32, kind="ExternalInput")
    gb = nc.dram_tensor("gb", [1, E], F32, kind="ExternalInput")
    ones1 = nc.dram_tensor("ones1", [1, P], BF, kind="ExternalInput")
    onesc = nc.dram_tensor("onesc", [P, 1], BF, kind="ExternalInput")
    lt = nc.dram_tensor("lt", [P, P], BF, kind="ExternalInput")
    ident = nc.dram_tensor("ident", [P, P], BF, kind="ExternalInput")
    iota = nc.dram_tensor("iota", [P, capm], F32, kind="ExternalInput")
    y = nc.dram_tensor("y", [P, TT, D], BF, kind="ExternalOutput")

    OOB = 3.0e6

    from contextlib import ExitStack
    with tile.TileContext(nc) as tc:
        with ExitStack() as stack:
            pool_specs = dict(
                const1=dict(bufs=1), xbig=dict(bufs=1),
                gchunk=dict(bufs=2), gtmp=dict(bufs=4),
                w13=dict(bufs=4), w2p=dict(bufs=2), b13=dict(bufs=2),
                xep=dict(bufs=1), hrout=dict(bufs=1), hshp=dict(bufs=2),
                yeBp=dict(bufs=1), pep=dict(bufs=2), s1p=dict(bufs=2),
                combp=dict(bufs=8), ytp=dict(bufs=2),
                ps_o1=dict(bufs=2, space="PSUM"),
                ps_o3=dict(bufs=2, space="PSUM"),
                ps_l2=dict(bufs=2, space="PSUM"),
                ps_tr=dict(bufs=2, space="PSUM"),
            )
            pools = {n: stack.enter_context(tc.tile_pool(name=n, **kw))
                     for n, kw in pool_specs.items()}
            (const1, xbig, gchunk, gtmp, w13, w2p, b13, xep, hrout,
             hshp, yeBp, pep, s1p, combp, ytp, ps_o1, ps_o3, ps_l2,
             ps_tr) = (
                pools[n] for n in (
                    "const1", "xbig", "gchunk", "gtmp", "w13", "w2p", "b13",
                    "xep", "hrout", "hshp", "yeBp", "pep", "s1p",
                    "combp", "ytp", "ps_o1", "ps_o3", "ps_l2", "ps_tr"))
            # ---- resident state ----
            # xtok_sb and xTb share one 2MB buffer (tag xb): xtok is dead
            # after the last gather; xTb is DMA'd into the same space then.
            xtok_sb = xbig.tile([P, TT, D], BF, name="xtok_sb", tag="xb")
            yacc = const1.tile([P, TT, D], F32)
            cw = const1.tile([P, TT, E], F32)
            posb_all = const1.tile([P, TT, E], F32)
            ye_sb = const1.tile([P, 6 * NP, D], BF)
            gwt_sb = const1.tile([P, KD, E], F32)
            gb_sb = const1.tile([1, E], F32)
            ones_sb = const1.tile([1, P], BF)
            onesc_sb = const1.tile([P, 1], BF)
            lt_sb = const1.tile([P, P], BF)
            id_sb = const1.tile([P, P], BF)
            iota_sb = const1.tile([P, capm], F32)
            b2c_sb = const1.tile([P, E * KD], F32)
            zerob = const1.tile([P, 1], F32)
            onesf = const1.tile([1, P], F32)

            # first two gate tiles + gate weights lead the DMA queues so
            # the gate starts without sitting behind the bulk prologue
            xc_pre = {}
            for m in range(2):
                xc = gchunk.tile([P, KD, P], F32, name="xchunk")
                nc.sync.dma_start(out=xc[:], in_=xT[:, :, m * P:(m + 1) * P])
                xc_pre[m] = xc
            nc.sync.dma_start(out=gwt_sb[:], in_=gwt[:])
            nc.sync.dma_start(out=gb_sb[:], in_=gb[:])
            # secondary consts flow on the Act queue in parallel so the
            # gate's per-tile xchunk stream on sync isn't delayed
            nc.scalar.dma_start(out=ones_sb[:], in_=ones1[:])
            nc.scalar.dma_start(out=onesc_sb[:], in_=onesc[:])
            nc.scalar.dma_start(out=lt_sb[:], in_=lt[:])
            nc.scalar.dma_start(out=id_sb[:], in_=ident[:])
            nc.scalar.dma_start(out=iota_sb[:], in_=iota[:])
            nc.scalar.dma_start(out=b2c_sb[:], in_=b2c[:])
            nc.vector.memset(zerob[:], 0.0)
            nc.vector.memset(onesf[:], 1.0)

            # prefetch expert 0's first L1 weight chunks
            pre_w = {}
            for hc in range(3):
                w1c = w13.tile([P, KD, P], BF, name="w1c", tag="w1c")
                nc.sync.dma_start(out=w1c[:], in_=w1t[0, hc])
                w3c = w13.tile([P, KD, P], BF, name="w3c", tag="w3c")
                nc.gpsimd.dma_start(out=w3c[:], in_=w3t[0, hc])
                pre_w[hc] = (w1c, w3c)
            # xtok hands off to the DMA engines late (pool-queue tail) so
            # its 2MB transfer neither starves the small gate-const loads
            # nor blocks the per-tile gate xchunk stream on sync; it is
            # first needed by expert 0's gather, well after the gate.
            nc.gpsimd.dma_start(out=xtok_sb[:], in_=xtok[:])

            # ---- gate + bucket positions, per 128-token tile ----
            cntb = None
            for m in range(TT):
                if m in xc_pre:
                    xchunk = xc_pre[m]
                else:
                    xchunk = gchunk.tile([P, KD, P], F32)
                    nc.sync.dma_start(out=xchunk[:],
                                      in_=xT[:, :, m * P:(m + 1) * P])

                pg = ps_l2.tile([P, E], F32, space="PSUM", name="pg", tag="l2")
                for k in range(KD):
                    nc.tensor.matmul(out=pg[:], lhsT=xchunk[:, k, :],
                                     rhs=gwt_sb[:, k, :],
                                     start=(k == 0), stop=False)
                nc.tensor.matmul(out=pg[:], lhsT=onesf[:], rhs=gb_sb[:],
                                 start=False, stop=True)

                lg = gtmp.tile([P, E], F32)
                nc.scalar.activation(lg[:], pg[:], AF.Copy)
                m8 = gtmp.tile([P, 8], F32)
                nc.vector.max(m8[:], lg[:])
                ex = gtmp.tile([P, E], F32)
                nc.vector.tensor_scalar(out=ex[:], in0=lg[:],
                                        scalar1=m8[:, 0:1], scalar2=None,
                                        op0=OP.subtract)
                nc.scalar.activation(ex[:], ex[:], AF.Exp, bias=zerob[:])
                mask = gtmp.tile([P, E], F32)
                nc.vector.tensor_scalar(out=mask[:], in0=lg[:],
                                        scalar1=m8[:, 1:2], scalar2=None,
                                        op0=OP.is_ge)
                e2 = gtmp.tile([P, 1], F32)
                nc.vector.tensor_tensor(out=e2[:], in0=m8[:, 1:2],
                                        in1=m8[:, 0:1], op=OP.subtract)
                nc.scalar.activation(e2[:], e2[:], AF.Exp, bias=zerob[:])
                den = gtmp.tile([P, 1], F32)
                nc.vector.tensor_scalar(out=den[:], in0=e2[:], scalar1=1.0,
                                        scalar2=None, op0=OP.add)
                rec = gtmp.tile([P, 1], F32)
                nc.vector.reciprocal(rec[:], den[:])
                cwm = gtmp.tile([P, E], F32)
                nc.vector.tensor_mul(cwm[:], ex[:], mask[:])
                nc.vector.tensor_scalar(out=cw[:, m, :], in0=cwm[:],
                                        scalar1=rec[:, 0:1], scalar2=None,
                                        op0=OP.mult)

                # bucket-local slot: pair prefix(mask) - mask; OOB unrouted
                maskb = gtmp.tile([P, E], BF)
                nc.vector.tensor_copy(maskb[:], mask[:])
                pp = ps_tr.tile([P, E], F32, space="PSUM", name="pp", tag="tr")
                if m % 2 == 0:
                    nc.tensor.matmul(out=pp[:], lhsT=lt_sb[:],
                                     rhs=maskb[:], start=True, stop=True)
                    cnt_ps = ps_tr.tile([1, E], F32, space="PSUM",
                                        name="cntp", tag="tr")
                    nc.tensor.matmul(out=cnt_ps[:], lhsT=onesc_sb[:],
                                     rhs=maskb[:], start=True, stop=True)
                    cntb = gtmp.tile([1, E], BF, name="cntb")
                    nc.scalar.activation(cntb[:], cnt_ps[:], AF.Copy)
                else:
                    nc.tensor.matmul(out=pp[:], lhsT=lt_sb[:],
                                     rhs=maskb[:], start=True, stop=False)
                    nc.tensor.matmul(out=pp[:], lhsT=ones_sb[:],
                                     rhs=cntb[:], start=False, stop=True)
                t1m = gtmp.tile([P, E], F32)
                nc.vector.scalar_tensor_tensor(out=t1m[:], in0=mask[:],
                                               scalar=-1.0, in1=pp[:],
                                               op0=OP.mult, op1=OP.add)
                notm = gtmp.tile([P, E], F32)
                nc.vector.tensor_scalar(out=notm[:], in0=mask[:],
                                        scalar1=-1.0, scalar2=1.0,
                                        op0=OP.mult, op1=OP.add)
                nc.vector.scalar_tensor_tensor(out=posb_all[:, m, :],
                                               in0=notm[:], scalar=OOB,
                                               in1=t1m[:],
                                               op0=OP.mult, op1=OP.add)

            # combine groups: stack pairs of experts with capm <= 64 into
            # one K<=128 chunk (second member at partition base 64 — PE
            # writes only allow bases 0/32/64). Gap rows are zeroed once.
            small = [e for e in range(E) if cfg.capms[e] <= 64]
            big = [e for e in range(E) if cfg.capms[e] > 64]
            groups = []
            for i in range(0, len(small) - 1, 2):
                groups.append((small[i], small[i + 1]))
            if len(small) % 2:
                groups.append((small[-1],))
            groups.extend((e,) for e in big)
            home = {}
            yoff = {}
            gap_zero = []
            for g in groups:
                for i, e in enumerate(g):
                    home[e] = g[0]
                    yoff[e] = 64 * i
                if len(g) == 2 and cfg.capms[g[0]] < 64:
                    gap_zero.append((cfg.capms[g[0]], g[0]))
            grp_k = {g: (64 + cfg.capms[g[1]] if len(g) == 2
                         else cfg.capms[g[0]]) for g in groups}
            tile_of = {g[0]: i for i, g in enumerate(groups)}

            # zero the ye/one-hot gap rows [capm_a, 64) of paired tiles so
            # the stacked K=64+capm_b combine chains read zeros there
            for cap_a, hm in gap_zero:
                ti = tile_of[hm]
                nc.vector.memset(
                    ye_sb[cap_a:64, ti * NP:(ti + 1) * NP, :], 0.0)

            # ---- routed experts over dispatched slots ----
            pre_sh = {}
            for e in range(E):
                capm_e = cfg.capms[e]
                CAPE = NP * capm_e
                b1sb = b13.tile([P, HCN], F32, name="b1sb", tag="b1")
                nc.sync.dma_start(out=b1sb[:], in_=b1a[e])
                b3sb = b13.tile([P, HCN], F32, name="b3sb", tag="b3")
                nc.sync.dma_start(out=b3sb[:], in_=b3a[e])

                # prefetch the first two w2 quarters; they land during L1
                w2qs_pre = []
                for dq in range(2):
                    w2q0 = w2p.tile([P, HCN, 256], BF, name="w2q", tag="w2q")
                    nc.sync.dma_start(
                        out=w2q0[:],
                        in_=w2t[e][:, :, dq * 256:(dq + 1) * 256])
                    w2qs_pre.append(w2q0)

                # one-hot dispatch tiles for all 8 token tiles
                pe_all = pep.tile([P, TT, capm_e], BF, name="pe_all", tag="pe")
                for m in range(TT):
                    nc.vector.tensor_scalar(
                        out=pe_all[:, m, :], in0=iota_sb[:, :capm_e],
                        scalar1=posb_all[:, m, e:e + 1],
                        scalar2=None, op0=OP.is_equal)

                # matmul gather: xeT[k][d, slot] = sum_m x_m^T @ Pe_m
                xeT = xep.tile([P, KD, CAPE], BF, name="xeT", tag="xeT")
                for k in range(KD):
                    gxp, gxt = (ps_l2, "l2") if k % 2 == 0 else (ps_tr, "tr")
                    gx = gxp.tile([P, CAPE], F32, space="PSUM",
                                  name="gx", tag=gxt)
                    for pr in range(NP):
                        for h in range(2):
                            m = 2 * pr + h
                            nc.tensor.matmul(
                                out=gx[:, pr * capm_e:(pr + 1) * capm_e],
                                lhsT=xtok_sb[:, m, k * P:(k + 1) * P],
                                rhs=pe_all[:, m, :],
                                start=(h == 0), stop=(h == 1))
                    nc.vector.tensor_copy(xeT[:, k, :], gx[:])

                if e == E - 1:
                    # last xtok reader just emitted: reload the shared
                    # buffer with d-major x for the tail's shared expert;
                    # the transfer hides under expert 7's L1/L2.
                    xTb = xbig.tile([P, KD, T], BF, name="xTb", tag="xb")
                    nc.scalar.dma_start(out=xTb[:], in_=xtb[:])

                # L1: hT[h, slot] = silu(W1 xe + b1) * (W3 xe + b3)
                hT = hrout.tile([P, HCN, CAPE], BF, name="hT", tag="hT")
                for hc in range(HCN):
                    if e == 0 and hc in pre_w:
                        w1c, w3c = pre_w[hc]
                    else:
                        w1c = w13.tile([P, KD, P], BF, name="w1c", tag="w1c")
                        nc.sync.dma_start(out=w1c[:], in_=w1t[e, hc])
                        w3c = w13.tile([P, KD, P], BF, name="w3c", tag="w3c")
                        nc.gpsimd.dma_start(out=w3c[:], in_=w3t[e, hc])
                    o1 = ps_o1.tile([P, CAPE], F32, space="PSUM",
                                    name="o1", tag="o1")
                    for k in range(KD):
                        nc.tensor.matmul(out=o1[:], lhsT=w1c[:, k, :],
                                         rhs=xeT[:, k, :],
                                         start=(k == 0), stop=(k == KD - 1))
                    s1 = s1p.tile([P, CAPE], F32, name="s1", tag="s1")
                    nc.scalar.activation(s1[:], o1[:], AF.Sigmoid,
                                         bias=b1sb[:, hc:hc + 1])
                    t1 = s1p.tile([P, CAPE], F32, name="t1", tag="t1")
                    nc.vector.scalar_tensor_tensor(
                        out=t1[:], in0=o1[:],
                        scalar=b1sb[:, hc:hc + 1], in1=s1[:],
                        op0=OP.add, op1=OP.mult)
                    o3 = ps_o3.tile([P, CAPE], F32, space="PSUM",
                                    name="o3", tag="o3")
                    for k in range(KD):
                        nc.tensor.matmul(out=o3[:], lhsT=w3c[:, k, :],
                                         rhs=xeT[:, k, :],
                                         start=(k == 0), stop=(k == KD - 1))
                    nc.vector.scalar_tensor_tensor(
                        out=hT[:, hc, :], in0=o3[:],
                        scalar=b3sb[:, hc:hc + 1], in1=t1[:],
                        op0=OP.add, op1=OP.mult)

                # L2 (d-partition orientation) + bias, then transpose to
                # slot-major ye tiles; transposes staggered one dc behind
                # the chains so their yeB reads never stall the PE.
                yeB = yeBp.tile([P, KD, CAPE], BF, name="yeB", tag="yeB")
                pend = []

                def emit_transp(dc, e=e, yeB=yeB, capm_e=capm_e):
                    o = yoff[e]
                    tgt = tile_of[home[e]] * NP
                    for pr in range(NP):
                        p2t = ps_tr.tile([P, P], BF, space="PSUM",
                                         name="p2t", tag="tr")
                        nc.tensor.transpose(
                            out=p2t[o:o + capm_e, :],
                            in_=yeB[:, dc, pr * capm_e:(pr + 1) * capm_e],
                            identity=id_sb[:])
                        nc.vector.tensor_copy(
                            ye_sb[o:o + capm_e, tgt + pr, dc * P:(dc + 1) * P],
                            p2t[o:o + capm_e, :])

                for dq in range(4):
                    w2q = w2qs_pre[dq]
                    for dc2 in range(2):
                        dc = dq * 2 + dc2
                        pl2 = ps_l2.tile([P, CAPE], F32, space="PSUM",
                                         name="pl2", tag="l2")
                        for hc in range(HCN):
                            nc.tensor.matmul(
                                out=pl2[:],
                                lhsT=w2q[:, hc, dc2 * P:(dc2 + 1) * P],
                                rhs=hT[:, hc, :],
                                start=(hc == 0), stop=(hc == HCN - 1))
                        nc.vector.tensor_scalar(
                            out=yeB[:, dc, :], in0=pl2[:],
                            scalar1=b2c_sb[:, e * KD + dc:e * KD + dc + 1],
                            scalar2=None, op0=OP.add)
                        if pend:
                            emit_transp(pend.pop())
                        pend.append(dc)
                    if dq + 2 < 4:
                        # refill two quarters ahead (this quarter's chains
                        # just freed the buffer, so the queue-head wait is
                        # short; only next-expert w1c prefetches sit behind)
                        w2n = w2p.tile([P, HCN, 256], BF, name="w2q",
                                       tag="w2q")
                        nc.sync.dma_start(
                            out=w2n[:],
                            in_=w2t[e][:, :, (dq + 2) * 256:(dq + 3) * 256])
                        w2qs_pre.append(w2n)
                while pend:
                    emit_transp(pend.pop())

            # ---- tail: shared sub-experts (hidden 512 each) + combine ----
            def emit_shared_l1_unit(s, hcl):
                sv = E + s // 4
                hcg = (s % 4) * HQ + hcl
                if (s, hcl) in pre_sh:
                    w1c, w3c = pre_sh[(s, hcl)]
                else:
                    w1c = w13.tile([P, KD, P], BF, name="w1c", tag="w1c")
                    nc.sync.dma_start(out=w1c[:], in_=w1t[sv, hcg])
                    w3c = w13.tile([P, KD, P], BF, name="w3c", tag="w3c")
                    nc.gpsimd.dma_start(out=w3c[:], in_=w3t[sv, hcg])
                hT_s = hts[s]
                for ft in range(FT):
                    fsl = slice(ft * 512, (ft + 1) * 512)
                    o1 = ps_o1.tile([P, 512], F32, space="PSUM",
                                    name="o1", tag="o1")
                    for k in range(KD):
                        nc.tensor.matmul(out=o1[:], lhsT=w1c[:, k, :],
                                         rhs=xTb[:, k, fsl],
                                         start=(k == 0), stop=(k == KD - 1))
                    s1 = s1p.tile([P, 512], F32, name="s1", tag="s1")
                    nc.scalar.activation(s1[:], o1[:], AF.Sigmoid,
                                         bias=bsh1[s // 4][:, hcg:hcg + 1])
                    t1 = s1p.tile([P, 512], F32, name="t1", tag="t1")
                    nc.vector.scalar_tensor_tensor(
                        out=t1[:], in0=o1[:],
                        scalar=bsh1[s // 4][:, hcg:hcg + 1], in1=s1[:],
                        op0=OP.add, op1=OP.mult)
                    o3 = ps_o3.tile([P, 512], F32, space="PSUM",
                                    name="o3", tag="o3")
                    for k in range(KD):
                        nc.tensor.matmul(out=o3[:], lhsT=w3c[:, k, :],
                                         rhs=xTb[:, k, fsl],
                                         start=(k == 0), stop=(k == KD - 1))
                    nc.vector.scalar_tensor_tensor(
                        out=hT_s[:, hcl, fsl], in0=o3[:],
                        scalar=bsh3[s // 4][:, hcg:hcg + 1], in1=t1[:],
                        op0=OP.add, op1=OP.mult)

            def emit_combine(m):
                pr = m // 2
                p2s_l = []
                for g in groups:
                    kk = grp_k[g]
                    p2c = ps_tr.tile([P, P], BF, space="PSUM",
                                     name="p2c", tag="tr")
                    for e in g:
                        capm_e = cfg.capms[e]
                        o = yoff[e]
                        pe2 = gtmp.tile([P, capm_e], BF, name="pe2")
                        nc.vector.tensor_scalar(
                            out=pe2[:], in0=iota_sb[:, :capm_e],
                            scalar1=posb_all[:, m, e:e + 1],
                            scalar2=None, op0=OP.is_equal)
                        pew = gtmp.tile([P, capm_e], BF, name="pew")
                        nc.vector.tensor_scalar(out=pew[:], in0=pe2[:],
                                                scalar1=cw[:, m, e:e + 1],
                                                scalar2=None, op0=OP.mult)
                        nc.tensor.transpose(out=p2c[o:o + capm_e, :],
                                            in_=pew[:], identity=id_sb[:])
                    p2s = combp.tile([kk, P], BF, name="p2s")
                    nc.scalar.activation(p2s[:], p2c[:kk, :], AF.Copy)
                    if len(g) == 2 and cfg.capms[g[0]] < 64:
                        # rows [capm_a, 64) came from uninitialized PSUM;
                        # zero them (their ye rows are zero too, but NaN
                        # garbage would still poison the product)
                        nc.vector.memset(p2s[cfg.capms[g[0]]:64, :], 0.0)
                    p2s_l.append((p2s, kk))
                for dt in range(DT):
                    dsl = slice(dt * 512, (dt + 1) * 512)
                    yp = ps_l2.tile([P, 512], F32, space="PSUM",
                                    name="yp", tag="l2")
                    for i, g in enumerate(groups):
                        p2s, kk = p2s_l[i]
                        nc.tensor.matmul(
                            out=yp[:], lhsT=p2s[:],
                            rhs=ye_sb[0:kk, tile_of[g[0]] * NP + pr, dsl],
                            start=(i == 0), stop=(i == len(groups) - 1))
                    nc.vector.tensor_copy(yacc[:, m, dsl], yp[:])

            def emit_shared_l2(s):
                sv = E + s // 4
                hcg0 = (s % 4) * HQ
                w2q = w2p.tile([P, HQ, D], BF, name="w2qs", tag="w2q")
                nc.scalar.dma_start(out=w2q[:],
                                    in_=w2t[sv][:, hcg0:hcg0 + HQ, :])
                hT_s = hts[s]
                for tt in range(TT):
                    tsl = slice(tt * P, (tt + 1) * P)
                    for dt in range(DT):
                        dsl = slice(dt * 512, (dt + 1) * 512)
                        if s >= NSH - 2:
                            # L1 is done by now: o1/o3 banks are free, use a
                            # deeper 3-pool rotation so chains never wait on
                            # the yacc-add evictions
                            pl, tg = [(ps_l2, "l2"), (ps_tr, "tr"),
                                      (ps_o1, "o1")][(tt * DT + dt) % 3]
                        else:
                            pl, tg = ((ps_l2, "l2") if (tt * DT + dt) % 2 == 0
                                      else (ps_tr, "tr"))
                        yp2 = pl.tile([P, 512], F32, space="PSUM",
                                      name="yp2", tag=tg)
                        for hcl in range(HQ):
                            nc.tensor.matmul(
                                out=yp2[:], lhsT=hT_s[:, hcl, tsl],
                                rhs=w2q[:, hcl, dsl],
                                start=(hcl == 0),
                                stop=(hcl == HQ - 1))
                        if s < NSH - 1:
                            nc.vector.tensor_add(yacc[:, tt, dsl],
                                                 yacc[:, tt, dsl], yp2[:])
                        else:
                            yt = ytp.tile([P, 512], BF, name="yt", bufs=4)
                            nc.vector.tensor_add(yt[:], yacc[:, tt, dsl],
                                                 yp2[:])
                            nc.sync.dma_start(out=y[:, tt, dsl], in_=yt[:])

            bsh1 = []
            bsh3 = []
            for sv in range(2):
                b1s = b13.tile([P, HCN], F32, name="b1sh", tag="b1sh")
                nc.sync.dma_start(out=b1s[:], in_=b1a[E + sv])
                b3s = b13.tile([P, HCN], F32, name="b3sh", tag="b3sh")
                nc.sync.dma_start(out=b3s[:], in_=b3a[E + sv])
                bsh1.append(b1s)
                bsh3.append(b3s)

            hts = {}
            for s in range(NSH):
                if s < 2:
                    hts[s] = hshp.tile([P, HQ, T], BF, name=f"hTs{s}",
                                       tag="hTs")
            # interleave first two shared sub-experts' L1 with combine;
            # combine leads: its inputs (ye, cw, pos) are ready at routed
            # end, covering the xTb/w1c arrival for the shared L1
            ci = 0
            for s in range(2):
                for hcl in range(HQ):
                    emit_combine(ci)
                    ci += 1
                    emit_shared_l1_unit(s, hcl)
            # pipeline: L2(s) || L1(s+2)
            for s in range(NSH):
                emit_shared_l2(s)
                if s + 2 < NSH:
                    hts[s + 2] = hshp.tile([P, HQ, T], BF, name=f"hTs{s+2}",
                                           tag="hTs")
                    for hcl in range(HQ):
                        emit_shared_l1_unit(s + 2, hcl)

    nc.compile()
    return nc


# ---------------- host-side packing ----------------

def pack_static(cfg: Cfg, gate_w, gate_b, w1, b1, w2, b2, w3, b3,
                sw1, sb1, sw2, sb2, sw3, sb3):
    D, H, E, NV, n_sh = cfg.D, cfg.H, cfg.E, cfg.NV, cfg.n_sh
    KD, HCN = cfg.KD, cfg.HCN

    w1T = np.transpose(w1, (0, 2, 1))                      # [E, D, H]
    w3T = np.transpose(w3, (0, 2, 1))
    w2T = np.transpose(w2, (0, 2, 1))                      # [E, H, D]
    s1T = sw1.T.reshape(D, n_sh, H).transpose(1, 0, 2)     # [n_sh, D, H]
    s3T = sw3.T.reshape(D, n_sh, H).transpose(1, 0, 2)
    s2T = sw2.T.reshape(n_sh, H, D)                        # [n_sh, H, D]
    w1T_all = np.concatenate([w1T, s1T], 0)                # [NV, D, H]
    w3T_all = np.concatenate([w3T, s3T], 0)
    w2T_all = np.concatenate([w2T, s2T], 0)                # [NV, H, D]

    w1t = np.ascontiguousarray(
        w1T_all.reshape(NV, KD, P, HCN, P).transpose(0, 3, 2, 1, 4)).astype(BF16)
    w3t = np.ascontiguousarray(
        w3T_all.reshape(NV, KD, P, HCN, P).transpose(0, 3, 2, 1, 4)).astype(BF16)
    w2t = np.ascontiguousarray(
        w2T_all.reshape(NV, HCN, P, D).transpose(0, 2, 1, 3)).astype(BF16)

    b1_all = np.concatenate([b1, sb1.reshape(n_sh, H)], 0)  # [NV, H]
    b3_all = np.concatenate([b3, sb3.reshape(n_sh, H)], 0)
    b1a = np.ascontiguousarray(
        b1_all.reshape(NV, HCN, P).transpose(0, 2, 1)).astype(np.float32)
    b3a = np.ascontiguousarray(
        b3_all.reshape(NV, HCN, P).transpose(0, 2, 1)).astype(np.float32)

    # routed b2 in d-partition layout, with the shared expert's sb2 folded
    # in: the renormalized top-2 weights sum to exactly 1, so adding sb2 to
    # every routed expert's bias reproduces the unconditional +sb2 exactly.
    b2f = b2 + sb2[None]
    b2c = np.ascontiguousarray(
        b2f.reshape(E, KD, P).transpose(2, 0, 1).reshape(P, E * KD)
    ).astype(np.float32)

    gwt = np.ascontiguousarray(
        gate_w.T.reshape(KD, P, E).transpose(1, 0, 2)).astype(np.float32)
    gb = gate_b[None].astype(np.float32)
    ones1 = np.ones((1, P), BF16)
    onesc = np.ones((P, 1), BF16)
    lt = np.triu(np.ones((P, P))).astype(BF16)
    ident = np.eye(P).astype(BF16)
    iota = np.tile(np.arange(cfg.capm, dtype=np.float32), (P, 1))

    return dict(w1t=w1t, w3t=w3t, w2t=w2t, b1a=b1a, b3a=b3a,
                b2c=b2c, gwt=gwt, gb=gb, ones1=ones1, onesc=onesc,
                lt=lt, ident=ident, iota=iota)


def pack_xtok(cfg: Cfg, x_tokens):
    T, D = x_tokens.shape
    xt = x_tokens.reshape(cfg.TT, P, D).transpose(1, 0, 2)
    return np.ascontiguousarray(xt).astype(BF16)


def pack_xT(cfg: Cfg, x_tokens):
    T, D = x_tokens.shape
    xT = x_tokens.T.reshape(cfg.KD, P, T).transpose(1, 0, 2)
    return np.ascontiguousarray(xT).astype(np.float32)


def unpack_y(cfg: Cfg, y_dev):
    return np.ascontiguousarray(
        y_dev.transpose(1, 0, 2).reshape(cfg.T, cfg.D)).astype(np.float32)


def balance_tokens(xf, gate_w, gate_b, E=8, margin=1):
    """Assign tokens to 256-token buckets so per-(bucket, expert) routed
    counts are near their per-expert means. Returns (perm, capms):
    bucket-major token order and per-expert slot capacities."""
    N = xf.shape[0]
    NB = N // 256
    logits = xf @ gate_w.T + gate_b
    idx = np.argsort(-logits, axis=1)[:, :2]
    tgt = np.zeros(E)
    for e in range(E):
        tgt[e] = ((idx[:, 0] == e) | (idx[:, 1] == e)).sum() / NB
    tgt = np.maximum(tgt, 1.0)
    cnt = np.zeros((NB, E), np.float64)
    fill = np.zeros(NB, np.int64)
    assign = np.empty(N, np.int32)
    rng = np.random.RandomState(0)
    BIG = 1 << 40
    for t in rng.permutation(N):
        a, b = idx[t]
        s = np.maximum((cnt[:, a] + 1) / tgt[a],
                       (cnt[:, b] + 1) / tgt[b]) * 4096 + fill
        s[fill >= 256] = BIG
        bb = int(np.argmin(s))
        assign[t] = bb
        cnt[bb, a] += 1
        cnt[bb, b] += 1
        fill[bb] += 1
    perm = np.argsort(assign.astype(np.int64) * N + np.arange(N))
    capms = tuple(int(c) + margin for c in cnt.max(0))
    return perm, capms


_CACHE = {}


def _get_nc(cfg: Cfg):
    key = (cfg.D, cfg.H, cfg.E, cfg.n_sh, cfg.T, cfg.capms)
    if key not in _CACHE:
        _CACHE[key] = build_nc_v2(cfg)
    return _CACHE[key]


def plan_cfg(inputs):
    """Balance tokens from the actual routing; returns (cfg, perm)."""
    x = np.asarray(inputs["x"], np.float32)
    B, S, D = x.shape
    xf = x.reshape(-1, D)
    perm, capms = balance_tokens(
        xf, np.asarray(inputs["gate_w"], np.float32),
        np.asarray(inputs["gate_b"], np.float32))
    cfg = Cfg(D=D, T=(B * S) // 8, n_cores=8, capms=capms)
    return cfg, perm


def make_in_maps(cfg: Cfg, inputs, perm):
    static = pack_static(
        cfg,
        np.asarray(inputs["gate_w"], np.float32), np.asarray(inputs["gate_b"], np.float32),
        np.asarray(inputs["w1"], np.float32), np.asarray(inputs["b1"], np.float32),
        np.asarray(inputs["w2"], np.float32), np.asarray(inputs["b2"], np.float32),
        np.asarray(inputs["w3"], np.float32), np.asarray(inputs["b3"], np.float32),
        np.asarray(inputs["sw1"], np.float32), np.asarray(inputs["sb1"], np.float32),
        np.asarray(inputs["sw2"], np.float32), np.asarray(inputs["sb2"], np.float32),
        np.asarray(inputs["sw3"], np.float32), np.asarray(inputs["sb3"], np.float32),
    )
    x = np.asarray(inputs["x"], np.float32)
    B, S, D = x.shape
    xp = x.reshape(-1, D)[perm]
    in_maps = []
    for c in range(cfg.n_cores):
        mm = dict(static)
        xc = xp[c * cfg.T:(c + 1) * cfg.T]
        mm["xT"] = pack_xT(cfg, xc)
        mm["xtok"] = pack_xtok(cfg, xc)
        mm["xtb"] = mm["xT"].astype(BF16)
        in_maps.append(mm)
    return in_maps


def kernel(**inputs) -> np.ndarray:
    x = np.asarray(inputs["x"], np.float32)
    B, S, D = x.shape
    cfg, perm = plan_cfg(inputs)
    nc = _get_nc(cfg)
    in_maps = make_in_maps(cfg, inputs, perm)
    res = run_bass_kernel_spmd(nc, in_maps, list(range(cfg.n_cores)))
    yp = np.concatenate(
        [unpack_y(cfg, res.results[c]["y"]) for c in range(cfg.n_cores)], 0)
    out = np.empty_like(yp)
    out[perm] = yp
    return out.reshape(B, S, D)


# revision 87
# speedup vs baseline: 1.2907x; 1.0110x over previous
"""MoE (8 routed experts, top-2, + shared expert) on 8 NeuronCores.

Data-parallel over tokens (1024/core), weights replicated. The host
load-balances the token->bucket assignment (any sharding is allowed) so
each (256-token bucket, expert) routed count sits at its per-expert
mean, letting the capacity-dispatched kernel run with per-expert
capacities capm_e = max bucket count + 1 (59..74 on this routing)
instead of the binomial-tail uniform 96. Combine chains for expert
pairs with capm <= 64 are stacked into single K<=128 chunks.

Device kernel (per core):
  1. Gate in fp32 (matches reference routing bit-for-bit for the
     observed >=1.7e-4 top-2/3 logit gaps), renormalized top-2 combine
     weights cw, and bucket-local slot positions via triangular-matmul
     prefix sums.
  2. Routed experts e=0..7: one-hot matmul gather of x into CAPE=296
     slots, SwiGLU L1 (feature-major, free dim = slots), L2 in
     d-partition orientation (out[d, slot], bias via activation), then
     PE transposes to slot-major ye tiles held in SBUF.
  3. Tail: shared expert (8 sub-experts of hidden 512) interleaved with
     the scatter-combine (transposed scaled one-hots x ye), everything
     accumulating into a token-major f32 yacc; last shared sub-expert's
     L2 fuses the final add and streams y out.

Matmuls are bf16 with fp32 accumulation; weight DMA is split across the
SP/Pool/Act queues to avoid head-of-line blocking on one DMA queue.
"""

import numpy as np
import ml_dtypes

import concourse.bacc as bacc
import concourse.bass as bass
import concourse.tile as tile
import concourse.mybir as mybir
from concourse.bass_utils import run_bass_kernel_spmd

BF16 = ml_dtypes.bfloat16
F32 = mybir.dt.float32
BF = mybir.dt.bfloat16
AF = mybir.ActivationFunctionType
OP = mybir.AluOpType

P = 128


class Cfg:
    def __init__(self, D=1024, H=2048, E=8, n_sh=2, T=1024, n_cores=8,
                 capms=(74,) * 8):
        self.D, self.H, self.E, self.n_sh, self.T = D, H, E, n_sh, T
        self.NV = E + n_sh          # packed weight rows (8 routed + 2 shared)
        self.HS = n_sh * H          # shared hidden total (4096)
        self.KD = D // P            # contraction chunks over D
        self.HCN = H // P           # h chunks per packed VE
        self.TT = T // P            # token 128-tiles per core
        self.FT = T // 512          # shared L1 free 512-tiles
        self.DT = D // 512          # 512-wide d tiles
        self.n_cores = n_cores
        self.capms = tuple(capms)   # slots per (256-token bucket, expert)
        self.capm = max(self.capms)  # iota / tile sizing width
        self.NP = self.TT // 2      # buckets per core (pair of tiles)
        self.NSH = 8                # shared sub-experts
        self.HQ = (self.HS // P) // self.NSH  # h-chunks per sub-expert (4)


def build_nc_v2(cfg: Cfg):
    D, H, E, T = cfg.D, cfg.H, cfg.E, cfg.T
    KD, HCN, TT, FT, DT = cfg.KD, cfg.HCN, cfg.TT, cfg.FT, cfg.DT
    capm, NP = cfg.capm, cfg.NP
    NSH, HQ = cfg.NSH, cfg.HQ

    nc = bacc.Bacc("TRN2", target_bir_lowering=False)

    xT = nc.dram_tensor("xT", [P, KD, T], F32, kind="ExternalInput")
    xtok = nc.dram_tensor("xtok", [P, TT, D], BF, kind="ExternalInput")
    xtb = nc.dram_tensor("xtb", [P, KD, T], BF, kind="ExternalInput")
    w1t = nc.dram_tensor("w1t", [cfg.NV, HCN, P, KD, P], BF, kind="ExternalInput")
    w3t = nc.dram_tensor("w3t", [cfg.NV, HCN, P, KD, P], BF, kind="ExternalInput")
    w2t = nc.dram_tensor("w2t", [cfg.NV, P, HCN, D], BF, kind="ExternalInput")
    b1a = nc.dram_tensor("b1a", [cfg.NV, P, HCN], F32, kind="ExternalInput")
    b3a = nc.dram_tensor("b3a", [cfg.NV, P, HCN], F32, kind="ExternalInput")
    b2c = nc.dram_tensor("b2c", [P, E * KD], F32, kind="ExternalInput")
    gwt = nc.dram_tensor("gwt", [P, KD, E], F32, kind="ExternalInput")
    gb = nc.dram_tensor("gb", [1, E], F32, kind="ExternalInput")
    ones1 = nc.dram_tensor("ones1", [1, P], BF, kind="ExternalInput")
    onesc = nc.dram_tensor("onesc", [P, 1], BF, kind="ExternalInput")
    lt = nc.dram_tensor("lt", [P, P], BF, kind="ExternalInput")
    ident = nc.dram_tensor("ident", [P, P], BF, kind="ExternalInput")
    iota = nc.dram_tensor("iota", [P, capm], F32, kind="ExternalInput")
    y = nc.dram_tensor("y", [P, TT, D], BF, kind="ExternalOutput")

    OOB = 3.0e6

    from contextlib import ExitStack
    with tile.TileContext(nc) as tc:
        with ExitStack() as stack:
            pool_specs = dict(
                const1=dict(bufs=1), xbig=dict(bufs=1),
                gchunk=dict(bufs=2), gtmp=dict(bufs=4),
                w13=dict(bufs=5), w2p=dict(bufs=3), b13=dict(bufs=2),
                xep=dict(bufs=1), hrout=dict(bufs=1), hshp=dict(bufs=2),
                yeBp=dict(bufs=1), pep=dict(bufs=2), s1p=dict(bufs=2),
                combp=dict(bufs=8), ytp=dict(bufs=2),
                ps_o1=dict(bufs=2, space="PSUM"),
                ps_o3=dict(bufs=2, space="PSUM"),
                ps_l2=dict(bufs=2, space="PSUM"),
                ps_tr=dict(bufs=2, space="PSUM"),
            )
            pools = {n: stack.enter_context(tc.tile_pool(name=n, **kw))
                     for n, kw in pool_specs.items()}
            (const1, xbig, gchunk, gtmp, w13, w2p, b13, xep, hrout,
             hshp, yeBp, pep, s1p, combp, ytp, ps_o1, ps_o3, ps_l2,
             ps_tr) = (
                pools[n] for n in (
                    "const1", "xbig", "gchunk", "gtmp", "w13", "w2p", "b13",
                    "xep", "hrout", "hshp", "yeBp", "pep", "s1p",
                    "combp", "ytp", "ps_o1", "ps_o3", "ps_l2", "ps_tr"))
            # ---- resident state ----
            # xtok_sb and xTb share one 2MB buffer (tag xb): xtok is dead
            # after the last gather; xTb is DMA'd into the same space then.
            xtok_sb = xbig.tile([P, TT, D], BF, name="xtok_sb", tag="xb")
            yacc = const1.tile([P, TT, D], F32)
            cw = const1.tile([P, TT, E], F32)
            posb_all = const1.tile([P, TT, E], F32)
            ye_sb = const1.tile([P, 6 * NP, D], BF)
            gwt_sb = const1.tile([P, KD, E], F32)
            gb_sb = const1.tile([1, E], F32)
            ones_sb = const1.tile([1, P], BF)
            onesc_sb = const1.tile([P, 1], BF)
            lt_sb = const1.tile([P, P], BF)
            id_sb = const1.tile([P, P], BF)
            iota_sb = const1.tile([P, capm], F32)
            b2c_sb = const1.tile([P, E * KD], F32)
            zerob = const1.tile([P, 1], F32)
            onesf = const1.tile([1, P], F32)

            # first two gate tiles + gate weights lead the DMA queues so
            # the gate starts without sitting behind the bulk prologue
            xc_pre = {}
            for m in range(2):
                xc = gchunk.tile([P, KD, P], F32, name="xchunk")
                nc.sync.dma_start(out=xc[:], in_=xT[:, :, m * P:(m + 1) * P])
                xc_pre[m] = xc
            nc.sync.dma_start(out=gwt_sb[:], in_=gwt[:])
            nc.sync.dma_start(out=gb_sb[:], in_=gb[:])
            # secondary consts flow on the Act queue in parallel so the
            # gate's per-tile xchunk stream on sync isn't delayed
            nc.scalar.dma_start(out=ones_sb[:], in_=ones1[:])
            nc.scalar.dma_start(out=onesc_sb[:], in_=onesc[:])
            nc.scalar.dma_start(out=lt_sb[:], in_=lt[:])
            nc.scalar.dma_start(out=id_sb[:], in_=ident[:])
            nc.scalar.dma_start(out=iota_sb[:], in_=iota[:])
            nc.scalar.dma_start(out=b2c_sb[:], in_=b2c[:])
            nc.vector.memset(zerob[:], 0.0)
            nc.vector.memset(onesf[:], 1.0)

            # prefetch expert 0's first L1 weight chunks
            pre_w = {}
            for hc in range(3):
                w1c = w13.tile([P, KD, P], BF, name="w1c", tag="w1c")
                nc.sync.dma_start(out=w1c[:], in_=w1t[0, hc])
                w3c = w13.tile([P, KD, P], BF, name="w3c", tag="w3c")
                nc.gpsimd.dma_start(out=w3c[:], in_=w3t[0, hc])
                pre_w[hc] = (w1c, w3c)
            # xtok hands off to the DMA engines late (pool-queue tail) so
            # its 2MB transfer neither starves the small gate-const loads
            # nor blocks the per-tile gate xchunk stream on sync; it is
            # first needed by expert 0's gather, well after the gate.
            nc.gpsimd.dma_start(out=xtok_sb[:], in_=xtok[:])

            # ---- gate + bucket positions, per 128-token tile ----
            cntb = None
            for m in range(TT):
                if m in xc_pre:
                    xchunk = xc_pre[m]
                else:
                    xchunk = gchunk.tile([P, KD, P], F32)
                    nc.sync.dma_start(out=xchunk[:],
                                      in_=xT[:, :, m * P:(m + 1) * P])

                pg = ps_l2.tile([P, E], F32, space="PSUM", name="pg", tag="l2")
                for k in range(KD):
                    nc.tensor.matmul(out=pg[:], lhsT=xchunk[:, k, :],
                                     rhs=gwt_sb[:, k, :],
                                     start=(k == 0), stop=False)
                nc.tensor.matmul(out=pg[:], lhsT=onesf[:], rhs=gb_sb[:],
                                 start=False, stop=True)

                lg = gtmp.tile([P, E], F32)
                nc.scalar.activation(lg[:], pg[:], AF.Copy)
                m8 = gtmp.tile([P, 8], F32)
                nc.vector.max(m8[:], lg[:])
                ex = gtmp.tile([P, E], F32)
                nc.vector.tensor_scalar(out=ex[:], in0=lg[:],
                                        scalar1=m8[:, 0:1], scalar2=None,
                                        op0=OP.subtract)
                nc.scalar.activation(ex[:], ex[:], AF.Exp, bias=zerob[:])
                mask = gtmp.tile([P, E], F32)
                nc.vector.tensor_scalar(out=mask[:], in0=lg[:],
                                        scalar1=m8[:, 1:2], scalar2=None,
                                        op0=OP.is_ge)
                e2 = gtmp.tile([P, 1], F32)
                nc.vector.tensor_tensor(out=e2[:], in0=m8[:, 1:2],
                                        in1=m8[:, 0:1], op=OP.subtract)
                nc.scalar.activation(e2[:], e2[:], AF.Exp, bias=zerob[:])
                den = gtmp.tile([P, 1], F32)
                nc.vector.tensor_scalar(out=den[:], in0=e2[:], scalar1=1.0,
                                        scalar2=None, op0=OP.add)
                rec = gtmp.tile([P, 1], F32)
                nc.vector.reciprocal(rec[:], den[:])
                cwm = gtmp.tile([P, E], F32)
                nc.vector.tensor_mul(cwm[:], ex[:], mask[:])
                nc.vector.tensor_scalar(out=cw[:, m, :], in0=cwm[:],
                                        scalar1=rec[:, 0:1], scalar2=None,
                                        op0=OP.mult)

                # bucket-local slot: pair prefix(mask) - mask; OOB unrouted
                maskb = gtmp.tile([P, E], BF)
                nc.vector.tensor_copy(maskb[:], mask[:])
                pp = ps_tr.tile([P, E], F32, space="PSUM", name="pp", tag="tr")
                if m % 2 == 0:
                    nc.tensor.matmul(out=pp[:], lhsT=lt_sb[:],
                                     rhs=maskb[:], start=True, stop=True)
                    cnt_ps = ps_tr.tile([1, E], F32, space="PSUM",
                                        name="cntp", tag="tr")
                    nc.tensor.matmul(out=cnt_ps[:], lhsT=onesc_sb[:],
                                     rhs=maskb[:], start=True, stop=True)
                    cntb = gtmp.tile([1, E], BF, name="cntb")
                    nc.scalar.activation(cntb[:], cnt_ps[:], AF.Copy)
                else:
                    nc.tensor.matmul(out=pp[:], lhsT=lt_sb[:],
                                     rhs=maskb[:], start=True, stop=False)
                    nc.tensor.matmul(out=pp[:], lhsT=ones_sb[:],
                                     rhs=cntb[:], start=False, stop=True)
                t1m = gtmp.tile([P, E], F32)
                nc.vector.scalar_tensor_tensor(out=t1m[:], in0=mask[:],
                                               scalar=-1.0, in1=pp[:],
                                               op0=OP.mult, op1=OP.add)
                notm = gtmp.tile([P, E], F32)
                nc.vector.tensor_scalar(out=notm[:], in0=mask[:],
                                        scalar1=-1.0, scalar2=1.0,
                                        op0=OP.mult, op1=OP.add)
                nc.vector.scalar_tensor_tensor(out=posb_all[:, m, :],
                                               in0=notm[:], scalar=OOB,
                                               in1=t1m[:],
                                               op0=OP.mult, op1=OP.add)

            # combine groups: stack pairs of experts with capm <= 64 into
            # one K<=128 chunk (second member at partition base 64 — PE
            # writes only allow bases 0/32/64). Gap rows are zeroed once.
            small = [e for e in range(E) if cfg.capms[e] <= 64]
            big = [e for e in range(E) if cfg.capms[e] > 64]
            groups = []
            for i in range(0, len(small) - 1, 2):
                groups.append((small[i], small[i + 1]))
            if len(small) % 2:
                groups.append((small[-1],))
            groups.extend((e,) for e in big)
            home = {}
            yoff = {}
            gap_zero = []
            for g in groups:
                for i, e in enumerate(g):
                    home[e] = g[0]
                    yoff[e] = 64 * i
                if len(g) == 2 and cfg.capms[g[0]] < 64:
                    gap_zero.append((cfg.capms[g[0]], g[0]))
            grp_k = {g: (64 + cfg.capms[g[1]] if len(g) == 2
                         else cfg.capms[g[0]]) for g in groups}
            tile_of = {g[0]: i for i, g in enumerate(groups)}

            # zero the ye/one-hot gap rows [capm_a, 64) of paired tiles so
            # the stacked K=64+capm_b combine chains read zeros there
            for cap_a, hm in gap_zero:
                ti = tile_of[hm]
                nc.vector.memset(
                    ye_sb[cap_a:64, ti * NP:(ti + 1) * NP, :], 0.0)

            # ---- routed experts over dispatched slots ----
            pre_sh = {}
            for e in range(E):
                capm_e = cfg.capms[e]
                CAPE = NP * capm_e
                b1sb = b13.tile([P, HCN], F32, name="b1sb", tag="b1")
                nc.sync.dma_start(out=b1sb[:], in_=b1a[e])
                b3sb = b13.tile([P, HCN], F32, name="b3sb", tag="b3")
                nc.sync.dma_start(out=b3sb[:], in_=b3a[e])

                # prefetch the first two w2 quarters; they land during L1
                w2qs_pre = []
                for dq in range(2):
                    w2q0 = w2p.tile([P, HCN, 256], BF, name="w2q", tag="w2q")
                    nc.sync.dma_start(
                        out=w2q0[:],
                        in_=w2t[e][:, :, dq * 256:(dq + 1) * 256])
                    w2qs_pre.append(w2q0)

                # one-hot dispatch tiles for all 8 token tiles
                pe_all = pep.tile([P, TT, capm_e], BF, name="pe_all", tag="pe")
                for m in range(TT):
                    nc.vector.tensor_scalar(
                        out=pe_all[:, m, :], in0=iota_sb[:, :capm_e],
                        scalar1=posb_all[:, m, e:e + 1],
                        scalar2=None, op0=OP.is_equal)

                # matmul gather: xeT[k][d, slot] = sum_m x_m^T @ Pe_m
                xeT = xep.tile([P, KD, CAPE], BF, name="xeT", tag="xeT")
                for k in range(KD):
                    gxp, gxt = (ps_l2, "l2") if k % 2 == 0 else (ps_tr, "tr")
                    gx = gxp.tile([P, CAPE], F32, space="PSUM",
                                  name="gx", tag=gxt)
                    for pr in range(NP):
                        for h in range(2):
                            m = 2 * pr + h
                            nc.tensor.matmul(
                                out=gx[:, pr * capm_e:(pr + 1) * capm_e],
                                lhsT=xtok_sb[:, m, k * P:(k + 1) * P],
                                rhs=pe_all[:, m, :],
                                start=(h == 0), stop=(h == 1))
                    nc.vector.tensor_copy(xeT[:, k, :], gx[:])

                if e == E - 1:
                    # last xtok reader just emitted: reload the shared
                    # buffer with d-major x for the tail's shared expert;
                    # the transfer hides under expert 7's L1/L2.
                    xTb = xbig.tile([P, KD, T], BF, name="xTb", tag="xb")
                    nc.scalar.dma_start(out=xTb[:], in_=xtb[:])

                # L1: hT[h, slot] = silu(W1 xe + b1) * (W3 xe + b3)
                hT = hrout.tile([P, HCN, CAPE], BF, name="hT", tag="hT")
                for hc in range(HCN):
                    if e == 0 and hc in pre_w:
                        w1c, w3c = pre_w[hc]
                    else:
                        w1c = w13.tile([P, KD, P], BF, name="w1c", tag="w1c")
                        nc.sync.dma_start(out=w1c[:], in_=w1t[e, hc])
                        w3c = w13.tile([P, KD, P], BF, name="w3c", tag="w3c")
                        nc.gpsimd.dma_start(out=w3c[:], in_=w3t[e, hc])
                    o1 = ps_o1.tile([P, CAPE], F32, space="PSUM",
                                    name="o1", tag="o1")
                    for k in range(KD):
                        nc.tensor.matmul(out=o1[:], lhsT=w1c[:, k, :],
                                         rhs=xeT[:, k, :],
                                         start=(k == 0), stop=(k == KD - 1))
                    s1 = s1p.tile([P, CAPE], F32, name="s1", tag="s1")
                    nc.scalar.activation(s1[:], o1[:], AF.Sigmoid,
                                         bias=b1sb[:, hc:hc + 1])
                    t1 = s1p.tile([P, CAPE], F32, name="t1", tag="t1")
                    nc.vector.scalar_tensor_tensor(
                        out=t1[:], in0=o1[:],
                        scalar=b1sb[:, hc:hc + 1], in1=s1[:],
                        op0=OP.add, op1=OP.mult)
                    o3 = ps_o3.tile([P, CAPE], F32, space="PSUM",
                                    name="o3", tag="o3")
                    for k in range(KD):
                        nc.tensor.matmul(out=o3[:], lhsT=w3c[:, k, :],
                                         rhs=xeT[:, k, :],
                                         start=(k == 0), stop=(k == KD - 1))
                    nc.vector.scalar_tensor_tensor(
                        out=hT[:, hc, :], in0=o3[:],
                        scalar=b3sb[:, hc:hc + 1], in1=t1[:],
                        op0=OP.add, op1=OP.mult)

                # L2 (d-partition orientation) + bias, then transpose to
                # slot-major ye tiles; transposes staggered one dc behind
                # the chains so their yeB reads never stall the PE.
                yeB = yeBp.tile([P, KD, CAPE], BF, name="yeB", tag="yeB")
                pend = []

                def emit_transp(dc, e=e, yeB=yeB, capm_e=capm_e):
                    o = yoff[e]
                    tgt = tile_of[home[e]] * NP
                    for pr in range(NP):
                        p2t = ps_tr.tile([P, P], BF, space="PSUM",
                                         name="p2t", tag="tr")
                        nc.tensor.transpose(
                            out=p2t[o:o + capm_e, :],
                            in_=yeB[:, dc, pr * capm_e:(pr + 1) * capm_e],
                            identity=id_sb[:])
                        nc.vector.tensor_copy(
                            ye_sb[o:o + capm_e, tgt + pr, dc * P:(dc + 1) * P],
                            p2t[o:o + capm_e, :])

                for dq in range(4):
                    w2q = w2qs_pre[dq]
                    for dc2 in range(2):
                        dc = dq * 2 + dc2
                        pl2 = ps_l2.tile([P, CAPE], F32, space="PSUM",
                                         name="pl2", tag="l2")
                        for hc in range(HCN):
                            nc.tensor.matmul(
                                out=pl2[:],
                                lhsT=w2q[:, hc, dc2 * P:(dc2 + 1) * P],
                                rhs=hT[:, hc, :],
                                start=(hc == 0), stop=(hc == HCN - 1))
                        nc.vector.tensor_scalar(
                            out=yeB[:, dc, :], in0=pl2[:],
                            scalar1=b2c_sb[:, e * KD + dc:e * KD + dc + 1],
                            scalar2=None, op0=OP.add)
                        if pend:
                            emit_transp(pend.pop())
                        pend.append(dc)
                    if dq + 2 < 4:
                        # refill two quarters ahead (this quarter's chains
                        # just freed the buffer, so the queue-head wait is
                        # short; only next-expert w1c prefetches sit behind)
                        w2n = w2p.tile([P, HCN, 256], BF, name="w2q",
                                       tag="w2q")
                        nc.sync.dma_start(
                            out=w2n[:],
                            in_=w2t[e][:, :, (dq + 2) * 256:(dq + 3) * 256])
                        w2qs_pre.append(w2n)
                while pend:
                    emit_transp(pend.pop())

            # ---- tail: shared sub-experts (hidden 512 each) + combine ----
            def emit_shared_l1_unit(s, hcl):
                sv = E + s // 4
                hcg = (s % 4) * HQ + hcl
                if (s, hcl) in pre_sh:
                    w1c, w3c = pre_sh[(s, hcl)]
                else:
                    w1c = w13.tile([P, KD, P], BF, name="w1c", tag="w1c")
                    nc.sync.dma_start(out=w1c[:], in_=w1t[sv, hcg])
                    w3c = w13.tile([P, KD, P], BF, name="w3c", tag="w3c")
                    nc.gpsimd.dma_start(out=w3c[:], in_=w3t[sv, hcg])
                hT_s = hts[s]
                for ft in range(FT):
                    fsl = slice(ft * 512, (ft + 1) * 512)
                    o1 = ps_o1.tile([P, 512], F32, space="PSUM",
                                    name="o1", tag="o1")
                    for k in range(KD):
                        nc.tensor.matmul(out=o1[:], lhsT=w1c[:, k, :],
                                         rhs=xTb[:, k, fsl],
                                         start=(k == 0), stop=(k == KD - 1))
                    s1 = s1p.tile([P, 512], F32, name="s1", tag="s1")
                    nc.scalar.activation(s1[:], o1[:], AF.Sigmoid,
                                         bias=bsh1[s // 4][:, hcg:hcg + 1])
                    t1 = s1p.tile([P, 512], F32, name="t1", tag="t1")
                    nc.vector.scalar_tensor_tensor(
                        out=t1[:], in0=o1[:],
                        scalar=bsh1[s // 4][:, hcg:hcg + 1], in1=s1[:],
                        op0=OP.add, op1=OP.mult)
                    o3 = ps_o3.tile([P, 512], F32, space="PSUM",
                                    name="o3", tag="o3")
                    for k in range(KD):
                        nc.tensor.matmul(out=o3[:], lhsT=w3c[:, k, :],
                                         rhs=xTb[:, k, fsl],
                                         start=(k == 0), stop=(k == KD - 1))
                    nc.vector.scalar_tensor_tensor(
                        out=hT_s[:, hcl, fsl], in0=o3[:],
                        scalar=bsh3[s // 4][:, hcg:hcg + 1], in1=t1[:],
                        op0=OP.add, op1=OP.mult)

            def emit_combine(m):
                pr = m // 2
                p2s_l = []
                for g in groups:
                    kk = grp_k[g]
                    p2c = ps_tr.tile([P, P], BF, space="PSUM",
                                     name="p2c", tag="tr")
                    for e in g:
                        capm_e = cfg.capms[e]
                        o = yoff[e]
                        pe2 = gtmp.tile([P, capm_e], BF, name="pe2")
                        nc.vector.tensor_scalar(
                            out=pe2[:], in0=iota_sb[:, :capm_e],
                            scalar1=posb_all[:, m, e:e + 1],
                            scalar2=None, op0=OP.is_equal)
                        pew = gtmp.tile([P, capm_e], BF, name="pew")
                        nc.vector.tensor_scalar(out=pew[:], in0=pe2[:],
                                                scalar1=cw[:, m, e:e + 1],
                                                scalar2=None, op0=OP.mult)
                        nc.tensor.transpose(out=p2c[o:o + capm_e, :],
                                            in_=pew[:], identity=id_sb[:])
                    p2s = combp.tile([kk, P], BF, name="p2s")
                    nc.scalar.activation(p2s[:], p2c[:kk, :], AF.Copy)
                    if len(g) == 2 and cfg.capms[g[0]] < 64:
                        # rows [capm_a, 64) came from uninitialized PSUM;
                        # zero them (their ye rows are zero too, but NaN
                        # garbage would still poison the product)
                        nc.vector.memset(p2s[cfg.capms[g[0]]:64, :], 0.0)
                    p2s_l.append((p2s, kk))
                for dt in range(DT):
                    dsl = slice(dt * 512, (dt + 1) * 512)
                    yp = ps_l2.tile([P, 512], F32, space="PSUM",
                                    name="yp", tag="l2")
                    for i, g in enumerate(groups):
                        p2s, kk = p2s_l[i]
                        nc.tensor.matmul(
                            out=yp[:], lhsT=p2s[:],
                            rhs=ye_sb[0:kk, tile_of[g[0]] * NP + pr, dsl],
                            start=(i == 0), stop=(i == len(groups) - 1))
                    nc.vector.tensor_copy(yacc[:, m, dsl], yp[:])

            def emit_shared_l2(s):
                sv = E + s // 4
                hcg0 = (s % 4) * HQ
                w2q = w2p.tile([P, HQ, D], BF, name="w2qs", tag="w2q")
                nc.scalar.dma_start(out=w2q[:],
                                    in_=w2t[sv][:, hcg0:hcg0 + HQ, :])
                hT_s = hts[s]
                for tt in range(TT):
                    tsl = slice(tt * P, (tt + 1) * P)
                    for dt in range(DT):
                        dsl = slice(dt * 512, (dt + 1) * 512)
                        if s >= NSH - 2:
                            # L1 is done by now: o1/o3 banks are free, use a
                            # deeper 3-pool rotation so chains never wait on
                            # the yacc-add evictions
                            pl, tg = [(ps_l2, "l2"), (ps_tr, "tr"),
                                      (ps_o1, "o1")][(tt * DT + dt) % 3]
                        else:
                            pl, tg = ((ps_l2, "l2") if (tt * DT + dt) % 2 == 0
                                      else (ps_tr, "tr"))
                        yp2 = pl.tile([P, 512], F32, space="PSUM",
                                      name="yp2", tag=tg)
                        for hcl in range(HQ):
                            nc.tensor.matmul(
                                out=yp2[:], lhsT=hT_s[:, hcl, tsl],
                                rhs=w2q[:, hcl, dsl],
                                start=(hcl == 0),
                                stop=(hcl == HQ - 1))
                        if s < NSH - 1:
                            nc.vector.tensor_add(yacc[:, tt, dsl],
                                                 yacc[:, tt, dsl], yp2[:])
                        else:
                            yt = ytp.tile([P, 512], BF, name="yt", bufs=4)
                            nc.vector.tensor_add(yt[:], yacc[:, tt, dsl],
                                                 yp2[:])
                            nc.sync.dma_start(out=y[:, tt, dsl], in_=yt[:])

            bsh1 = []
            bsh3 = []
            for sv in range(2):
                b1s = b13.tile([P, HCN], F32, name="b1sh", tag="b1sh")
                nc.sync.dma_start(out=b1s[:], in_=b1a[E + sv])
                b3s = b13.tile([P, HCN], F32, name="b3sh", tag="b3sh")
                nc.sync.dma_start(out=b3s[:], in_=b3a[E + sv])
                bsh1.append(b1s)
                bsh3.append(b3s)

            hts = {}
            for s in range(NSH):
                if s < 2:
                    hts[s] = hshp.tile([P, HQ, T], BF, name=f"hTs{s}",
                                       tag="hTs")
            # interleave first two shared sub-experts' L1 with combine;
            # combine leads: its inputs (ye, cw, pos) are ready at routed
            # end, covering the xTb/w1c arrival for the shared L1
            ci = 0
            for s in range(2):
                for hcl in range(HQ):
                    emit_combine(ci)
                    ci += 1
                    emit_shared_l1_unit(s, hcl)
            # pipeline: L2(s) || L1(s+2)
            for s in range(NSH):
                emit_shared_l2(s)
                if s + 2 < NSH:
                    hts[s + 2] = hshp.tile([P, HQ, T], BF, name=f"hTs{s+2}",
                                           tag="hTs")
                    for hcl in range(HQ):
                        emit_shared_l1_unit(s + 2, hcl)

    nc.compile()
    return nc


# ---------------- host-side packing ----------------

def pack_static(cfg: Cfg, gate_w, gate_b, w1, b1, w2, b2, w3, b3,
                sw1, sb1, sw2, sb2, sw3, sb3):
    D, H, E, NV, n_sh = cfg.D, cfg.H, cfg.E, cfg.NV, cfg.n_sh
    KD, HCN = cfg.KD, cfg.HCN

    w1T = np.transpose(w1, (0, 2, 1))                      # [E, D, H]
    w3T = np.transpose(w3, (0, 2, 1))
    w2T = np.transpose(w2, (0, 2, 1))                      # [E, H, D]
    s1T = sw1.T.reshape(D, n_sh, H).transpose(1, 0, 2)     # [n_sh, D, H]
    s3T = sw3.T.reshape(D, n_sh, H).transpose(1, 0, 2)
    s2T = sw2.T.reshape(n_sh, H, D)                        # [n_sh, H, D]
    w1T_all = np.concatenate([w1T, s1T], 0)                # [NV, D, H]
    w3T_all = np.concatenate([w3T, s3T], 0)
    w2T_all = np.concatenate([w2T, s2T], 0)                # [NV, H, D]

    w1t = np.ascontiguousarray(
        w1T_all.reshape(NV, KD, P, HCN, P).transpose(0, 3, 2, 1, 4)).astype(BF16)
    w3t = np.ascontiguousarray(
        w3T_all.reshape(NV, KD, P, HCN, P).transpose(0, 3, 2, 1, 4)).astype(BF16)
    w2t = np.ascontiguousarray(
        w2T_all.reshape(NV, HCN, P, D).transpose(0, 2, 1, 3)).astype(BF16)

    b1_all = np.concatenate([b1, sb1.reshape(n_sh, H)], 0)  # [NV, H]
    b3_all = np.concatenate([b3, sb3.reshape(n_sh, H)], 0)
    b1a = np.ascontiguousarray(
        b1_all.reshape(NV, HCN, P).transpose(0, 2, 1)).astype(np.float32)
    b3a = np.ascontiguousarray(
        b3_all.reshape(NV, HCN, P).transpose(0, 2, 1)).astype(np.float32)

    # routed b2 in d-partition layout, with the shared expert's sb2 folded
    # in: the renormalized top-2 weights sum to exactly 1, so adding sb2 to
    # every routed expert's bias reproduces the unconditional +sb2 exactly.
    b2f = b2 + sb2[None]
    b2c = np.ascontiguousarray(
        b2f.reshape(E, KD, P).transpose(2, 0, 1).reshape(P, E * KD)
    ).astype(np.float32)

    gwt = np.ascontiguousarray(
        gate_w.T.reshape(KD, P, E).transpose(1, 0, 2)).astype(np.float32)
    gb = gate_b[None].astype(np.float32)
    ones1 = np.ones((1, P), BF16)
    onesc = np.ones((P, 1), BF16)
    lt = np.triu(np.ones((P, P))).astype(BF16)
    ident = np.eye(P).astype(BF16)
    iota = np.tile(np.arange(cfg.capm, dtype=np.float32), (P, 1))

    return dict(w1t=w1t, w3t=w3t, w2t=w2t, b1a=b1a, b3a=b3a,
                b2c=b2c, gwt=gwt, gb=gb, ones1=ones1, onesc=onesc,
                lt=lt, ident=ident, iota=iota)


def pack_xtok(cfg: Cfg, x_tokens):
    T, D = x_tokens.shape
    xt = x_tokens.reshape(cfg.TT, P, D).transpose(1, 0, 2)
    return np.ascontiguousarray(xt).astype(BF16)


def pack_xT(cfg: Cfg, x_tokens):
    T, D = x_tokens.shape
    xT = x_tokens.T.reshape(cfg.KD, P, T).transpose(1, 0, 2)
    return np.ascontiguousarray(xT).astype(np.float32)


def unpack_y(cfg: Cfg, y_dev):
    return np.ascontiguousarray(
        y_dev.transpose(1, 0, 2).reshape(cfg.T, cfg.D)).astype(np.float32)


def balance_tokens(xf, gate_w, gate_b, E=8, margin=1):
    """Assign tokens to 256-token buckets so per-(bucket, expert) routed
    counts are near their per-expert means. Returns (perm, capms):
    bucket-major token order and per-expert slot capacities."""
    N = xf.shape[0]
    NB = N // 256
    logits = xf @ gate_w.T + gate_b
    idx = np.argsort(-logits, axis=1)[:, :2]
    tgt = np.zeros(E)
    for e in range(E):
        tgt[e] = ((idx[:, 0] == e) | (idx[:, 1] == e)).sum() / NB
    tgt = np.maximum(tgt, 1.0)
    cnt = np.zeros((NB, E), np.float64)
    fill = np.zeros(NB, np.int64)
    assign = np.empty(N, np.int32)
    rng = np.random.RandomState(0)
    BIG = 1 << 40
    for t in rng.permutation(N):
        a, b = idx[t]
        s = np.maximum((cnt[:, a] + 1) / tgt[a],
                       (cnt[:, b] + 1) / tgt[b]) * 4096 + fill
        s[fill >= 256] = BIG
        bb = int(np.argmin(s))
        assign[t] = bb
        cnt[bb, a] += 1
        cnt[bb, b] += 1
        fill[bb] += 1
    perm = np.argsort(assign.astype(np.int64) * N + np.arange(N))
    capms = tuple(int(c) + margin for c in cnt.max(0))
    return perm, capms


_CACHE = {}


def _get_nc(cfg: Cfg):
    key = (cfg.D, cfg.H, cfg.E, cfg.n_sh, cfg.T, cfg.capms)
    if key not in _CACHE:
        _CACHE[key] = build_nc_v2(cfg)
    return _CACHE[key]


def plan_cfg(inputs):
    """Balance tokens from the actual routing; returns (cfg, perm)."""
    x = np.asarray(inputs["x"], np.float32)
    B, S, D = x.shape
    xf = x.reshape(-1, D)
    perm, capms = balance_tokens(
        xf, np.asarray(inputs["gate_w"], np.float32),
        np.asarray(inputs["gate_b"], np.float32))
    cfg = Cfg(D=D, T=(B * S) // 8, n_cores=8, capms=capms)
    return cfg, perm


def make_in_maps(cfg: Cfg, inputs, perm):
    static = pack_static(
        cfg,
        np.asarray(inputs["gate_w"], np.float32), np.asarray(inputs["gate_b"], np.float32),
        np.asarray(inputs["w1"], np.float32), np.asarray(inputs["b1"], np.float32),
        np.asarray(inputs["w2"], np.float32), np.asarray(inputs["b2"], np.float32),
        np.asarray(inputs["w3"], np.float32), np.asarray(inputs["b3"], np.float32),
        np.asarray(inputs["sw1"], np.float32), np.asarray(inputs["sb1"], np.float32),
        np.asarray(inputs["sw2"], np.float32), np.asarray(inputs["sb2"], np.float32),
        np.asarray(inputs["sw3"], np.float32), np.asarray(inputs["sb3"], np.float32),
    )
    x = np.asarray(inputs["x"], np.float32)
    B, S, D = x.shape
    xp = x.reshape(-1, D)[perm]
    in_maps = []
    for c in range(cfg.n_cores):
        mm = dict(static)
        xc = xp[c * cfg.T:(c + 1) * cfg.T]
        mm["xT"] = pack_xT(cfg, xc)
        mm["xtok"] = pack_xtok(cfg, xc)
        mm["xtb"] = mm["xT"].astype(BF16)
        in_maps.append(mm)
    return in_maps


def kernel(**inputs) -> np.ndarray:
    x = np.asarray(inputs["x"], np.float32)
    B, S, D = x.shape
    cfg, perm = plan_cfg(inputs)
    nc = _get_nc(cfg)
    in_maps = make_in_maps(cfg, inputs, perm)
    res = run_bass_kernel_spmd(nc, in_maps, list(range(cfg.n_cores)))
    yp = np.concatenate(
        [unpack_y(cfg, res.results[c]["y"]) for c in range(cfg.n_cores)], 0)
    out = np.empty_like(yp)
    out[perm] = yp
    return out.reshape(B, S, D)
